# revision 1
# baseline (speedup 1.0000x reference)
"""Trainium2 Bass kernel for chunked recurrent causal linear attention.

Problem: b=2, h=8, n=2048, d=128, e=64, chunk=128, two branches (plain +
rotary) sharing one denominator.

Math (per (b,h), per chunk c, token t in chunk, with running state
S[d,e], Z[d] per branch):
    AT[s,t]   = k_s . q_t                  (s,t in chunk; masked to s<=t)
    num[t,:]  = sum_s ATm[s,t] v_s + q_t @ S      (both branches summed)
    den[t]    = sum_s ATm[s,t]   + q_t . Z        (both branches summed)
    out[t,:]  = num[t,:] / den[t]
    S += k_chunk^T v_chunk ;  Z += sum_s k_s

Sharding: 16 (b,h) pairs over 8 cores, 2 pairs per core. Host ships
pre-transposed copies of q/k/q_rot/k_rot (so no on-device transposes are
needed) plus natural-layout k/k_rot (stationary operand of the state
update) and v with a ones-column appended (fuses the denominator into
the numerator matmuls).
"""

import contextlib
import sys

_nullctx = contextlib.nullcontext

if "/opt/trn_rl_repo" not in sys.path:
    sys.path.insert(0, "/opt/trn_rl_repo")

import numpy as np

import concourse.bass as bass
import concourse.tile as tile
from concourse import bacc, mybir
from concourse.bass_utils import run_bass_kernel_spmd

F32 = mybir.dt.float32
F32R = mybir.dt.float32r

N_CORES = 8
PAIRS_PER_CORE = 2
N = 2048           # sequence length per (b,h)
D = 128            # qk head dim
E = 64             # v head dim
E1 = E + 1         # v plus ones column
C = 128            # chunk size
NCHUNK = N // C    # 16
SLAB = 4           # chunks per DMA slab
SLAB_BUFS = 6      # slab pool buffers
NROWS = PAIRS_PER_CORE * N  # 4096

_cached = {}


def build_kernel(repeat=1, loop_k=None, dma_only=False, reuse_slab=False,
                 probe_no_at=False, probe_no_state=False, transpose_k=False,
                 pipe=1, host_norm=False, dma_split=False, taper=False,
                 big_bufs=False, load_reorder=False, bank_42=False,
                 stagger=False, probe_pe_only=False, mm_f32r=False,
                 f32r=False, fast_start=False, ilv=True):
    nc = bacc.Bacc("TRN2", target_bir_lowering=False, debug=False,
                   num_devices=N_CORES)

    MT = F32  # typed-f32r rejected by walrus codegen (odd-N ISA check)

    def mm(out_ap, lhsT_ap, rhs_ap, **kw):
        if mm_f32r:
            lhsT_ap = lhsT_ap.bitcast(F32R)
            rhs_ap = rhs_ap.bitcast(F32R)
        return nc.tensor.matmul(out_ap, lhsT_ap, rhs_ap, **kw)

    qT = nc.dram_tensor("qT", [D, NROWS], MT, kind="ExternalInput").ap()
    kT = nc.dram_tensor("kT", [D, NROWS], MT, kind="ExternalInput").ap()
    qrT = nc.dram_tensor("qrT", [D, NROWS], MT, kind="ExternalInput").ap()
    krT = nc.dram_tensor("krT", [D, NROWS], MT, kind="ExternalInput").ap()
    if not transpose_k:
        kn = nc.dram_tensor("kn", [NROWS, D], MT, kind="ExternalInput").ap()
        krn = nc.dram_tensor("krn", [NROWS, D], MT, kind="ExternalInput").ap()
    else:
        ident = nc.dram_tensor("ident", [C, C], F32, kind="ExternalInput").ap()
    v1 = nc.dram_tensor("v1", [NROWS, E1], MT, kind="ExternalInput").ap()
    mask = nc.dram_tensor("mask", [C, C], F32, kind="ExternalInput").ap()
    EO = E1 if host_norm else E
    out = nc.dram_tensor("out", [NROWS, EO], F32, kind="ExternalOutput").ap()

    if taper:
        plans = [{0: 2, 2: 4, 6: 4, 10: 4, 14: 2}] * PAIRS_PER_CORE
    elif stagger:
        plans = [{c0: SLAB for c0 in range(0, NCHUNK, SLAB)},
                 {0: 2, 2: 4, 6: 4, 10: 4, 14: 2}]
    else:
        plans = [{c0: SLAB for c0 in range(0, NCHUNK, SLAB)}] * PAIRS_PER_CORE
    slab_of = []
    for pp in range(PAIRS_PER_CORE):
        m = {}
        for c0, ln in plans[pp].items():
            for c in range(c0, c0 + ln):
                m[c] = (c0, ln)
        slab_of.append(m)

    with tile.TileContext(nc) as tc:
        with (
            tc.tile_pool(name="const", bufs=1) as constp,
            tc.tile_pool(name="slabs", bufs=SLAB_BUFS) as slabs,
            tc.tile_pool(name="atm", bufs=(6 if big_bufs else (4 if pipe == 1 else 6))) as atmp,
            tc.tile_pool(name="ssb", bufs=(12 if big_bufs else 8)) as ssbp,
            tc.tile_pool(name="dinv", bufs=(12 if big_bufs else 8)) as dinvp,
            tc.tile_pool(name="pat", bufs=(2 if (transpose_k or bank_42) else 3),
                         space="PSUM") as patp,
            tc.tile_pool(name="pout", bufs=(2 if transpose_k else (4 if bank_42 else 3)),
                         space="PSUM") as poutp,
            tc.tile_pool(name="pst", bufs=2, space="PSUM") as pstp,
            tc.tile_pool(name="ktr", bufs=2, space="PSUM") as ktrp,
            tc.tile_pool(name="kns", bufs=4) as knsp,
        ):
            mask_t = constp.tile([C, C], F32, tag="mask")
            nc.sync.dma_start(mask_t[:], mask[:])
            if transpose_k:
                ident_t = constp.tile([C, C], F32, tag="ident")
                nc.sync.dma_start(ident_t[:], ident[:])

            for rep in range(repeat):
              with (tc.For_i(0, loop_k, 1, hint_engines=(
                        mybir.EngineType.PE, mybir.EngineType.DVE,
                        mybir.EngineType.Activation, mybir.EngineType.SP))
                    if (loop_k is not None and loop_k > 1)
                    else _nullctx()):
                  # per-pair state accumulator in one PSUM bank:
                  # cols 0:65 -> branch 0 [S|Z], cols 66:131 -> branch 1
                  pS = {}
                  for p in range(PAIRS_PER_CORE):
                      pS[p] = pstp.tile([D, 2 * E1 + 2], F32, tag="pS", name=f"pS_{rep}_{p}")

                  slab_t = [None] * PAIRS_PER_CORE   # per pair: dict of slab tiles
                  S_sbuf = {}                        # (p, br) -> sbuf state tile

                  # Software pipeline, one chunk deep: the "front" stage of
                  # chunk c emits loads, the state update (PE), and AT+mask
                  # (PE then DVE); the "back" stage consumes chunk c-1's
                  # masked AT for the numerator/denominator matmuls. This
                  # gives every cross-engine hop a full stage of slack, so
                  # the PE never head-of-line blocks on DVE/ACT latency.
                  fifo = []
                  for cc in range(NCHUNK + pipe):
                    pending = {}
                    back = {}
                    if cc >= pipe:
                        back = fifo.pop(0)
                    if cc < NCHUNK:
                        fifo.append(pending)
                    if cc < NCHUNK:
                      c = cc
                      for p in range(PAIRS_PER_CORE):
                          c0, slen = slab_of[p][c]
                          if (c == c0) and not (reuse_slab and c > 0):
                              base = p * N + c * C
                              cols = slice(base, base + slen * C)
                              dmae = nc.gpsimd if (dma_split and p == 1) else nc.sync
                              st = {"len": slen}
                              if not load_reorder:
                                  st["qT"] = slabs.tile([D, slen * C], MT, tag="qT", name=f"qTs_{rep}_{p}_{c}")
                                  st["kT"] = slabs.tile([D, slen * C], MT, tag="kT", name=f"kTs_{rep}_{p}_{c}")
                                  if fast_start and c == 0:
                                      # split the very first q/k loads so chunk
                                      # 0's AT matmul starts after 128KB, not
                                      # a full slab (range-level tile deps)
                                      dmae.dma_start(st["qT"][:, 0:C], qT[:, base:base + C])
                                      dmae.dma_start(st["kT"][:, 0:C], kT[:, base:base + C])
                                      dmae.dma_start(st["qT"][:, C:slen * C], qT[:, base + C:base + slen * C])
                                      dmae.dma_start(st["kT"][:, C:slen * C], kT[:, base + C:base + slen * C])
                                  else:
                                      dmae.dma_start(st["qT"][:], qT[:, cols])
                                      dmae.dma_start(st["kT"][:], kT[:, cols])
                                  st["qrT"] = slabs.tile([D, slen * C], MT, tag="qrT", name=f"qrTs_{rep}_{p}_{c}")
                                  dmae.dma_start(st["qrT"][:], qrT[:, cols])
                                  st["krT"] = slabs.tile([D, slen * C], MT, tag="krT", name=f"krTs_{rep}_{p}_{c}")
                                  dmae.dma_start(st["krT"][:], krT[:, cols])
                              # load the state-update inputs (kn/krn/v1)
                              # first: they feed the first PE ops of the chunk
                              if not transpose_k:
                                  st["kn"] = slabs.tile([C, slen, D], MT, tag="kn", name=f"kns_{rep}_{p}_{c}")
                                  dmae.dma_start(
                                      st["kn"][:],
                                      kn[cols, :].rearrange("(n p) d -> p n d", p=C))
                                  st["krn"] = slabs.tile([C, slen, D], MT, tag="krn", name=f"krns_{rep}_{p}_{c}")
                                  dmae.dma_start(
                                      st["krn"][:],
                                      krn[cols, :].rearrange("(n p) d -> p n d", p=C))
                              st["v1"] = slabs.tile([C, slen, E1], MT, tag="v1", name=f"v1s_{rep}_{p}_{c}")
                              dmae.dma_start(
                                  st["v1"][:],
                                  v1[cols, :].rearrange("(n p) e -> p n e", p=C))
                              if load_reorder:
                                  st["kT"] = slabs.tile([D, slen * C], MT, tag="kT", name=f"kTs_{rep}_{p}_{c}")
                                  dmae.dma_start(st["kT"][:], kT[:, cols])
                                  st["qT"] = slabs.tile([D, slen * C], MT, tag="qT", name=f"qTs_{rep}_{p}_{c}")
                                  dmae.dma_start(st["qT"][:], qT[:, cols])
                                  st["qrT"] = slabs.tile([D, slen * C], MT, tag="qrT", name=f"qrTs_{rep}_{p}_{c}")
                                  dmae.dma_start(st["qrT"][:], qrT[:, cols])
                                  st["krT"] = slabs.tile([D, slen * C], MT, tag="krT", name=f"krTs_{rep}_{p}_{c}")
                                  dmae.dma_start(st["krT"][:], krT[:, cols])
                              st["outs"] = slabs.tile([C, slen, EO], F32, tag="outs", name=f"outs_{rep}_{p}_{c}")
                              slab_t[p] = st

                          st = slab_t[p]
                          j = c - c0
                          qcT = st["qT"][:, j * C:(j + 1) * C]
                          kcT = st["kT"][:, j * C:(j + 1) * C]
                          qrcT = st["qrT"][:, j * C:(j + 1) * C]
                          krcT = st["krT"][:, j * C:(j + 1) * C]
                          vc = st["v1"][:, j, :]
                          knc = krnc = None
                          if not transpose_k:
                              knc = st["kn"][:, j, :]
                              krnc = st["krn"][:, j, :]

                          if dma_only:
                              continue

                          if probe_pe_only:
                              # pure matmul throughput probe: same 7 MMs as the
                              # real kernel, but no cross-engine deps at all
                              pat0 = patp.tile([C, C], F32, tag="pat")
                              mm(pat0[:], kcT, qcT, start=True, stop=False)
                              mm(pat0[:], krcT, qrcT, start=False, stop=True)
                              po = poutp.tile([C, E1], F32, tag="po")
                              mm(po[:], mask_t[:], vc, start=True, stop=False)
                              mm(po[:], qcT, mask_t[:, 0:E1], start=False, stop=False, skip_group_check=True)
                              mm(po[:], qrcT, mask_t[:, 0:E1], start=False, stop=True, skip_group_check=True)
                              mm(pS[p][:, 0:E1], knc, vc, start=(c == 0), stop=False, skip_group_check=True)
                              mm(pS[p][:, E1 + 1:2 * E1 + 1], krnc, vc, start=False, stop=(c == NCHUNK - 1), skip_group_check=True)
                              continue

                          prev_S = S_sbuf.get(p)

                          if ilv:
                              # MMs emitted pair-interleaved after this loop
                              pending[p] = dict(qcT=qcT, qrcT=qrcT, kcT=kcT,
                                                krcT=krcT, knc=knc, krnc=krnc,
                                                vc=vc, st=st, j=j, c=c,
                                                prev_S=prev_S, kns0=None,
                                                kns1=None, c0=c0,
                                                slen=st.get("len", SLAB))
                              continue

                          # State update: both branches share one PSUM bank
                          # (start=True on c0/br0 clears it; br1 overwrites its
                          # unwritten columns). Without transpose_k the natural-
                          # layout k arrives by DMA and the update is emitted
                          # here (front stage); with transpose_k the k tiles are
                          # transposed on the PE this stage and the state update
                          # moves to the back stage so the transpose->copy->
                          # matmul chain gets a stage of slack.
                          kns0 = kns1 = None
                          if transpose_k and not probe_no_state:
                              ktp0 = ktrp.tile([C, C], F32, tag="ktr")
                              nc.tensor.transpose(ktp0[:], kcT, ident_t[:])
                              kns0 = knsp.tile([C, C], F32, tag="kns")
                              nc.vector.tensor_copy(kns0[:], ktp0[:])
                              ktp1 = ktrp.tile([C, C], F32, tag="ktr")
                              nc.tensor.transpose(ktp1[:], krcT, ident_t[:])
                              kns1 = knsp.tile([C, C], F32, tag="kns")
                              nc.scalar.copy(kns1[:], ktp1[:])
                          if not transpose_k and not probe_no_state:
                              mm(pS[p][:, 0:E1], knc, vc,
                                               start=(c == 0), stop=False,
                                               skip_group_check=True)
                              mm(pS[p][:, E1 + 1:2 * E1 + 1], krnc, vc,
                                               start=False, stop=(c == NCHUNK - 1),
                                               skip_group_check=True)
                              if c < NCHUNK - 1:
                                  s01 = ssbp.tile([D, 2 * E1 + 2], MT, tag="ssb")
                                  nc.scalar.copy(s01[:], pS[p][:])
                                  S_sbuf[p] = s01

                          # AT = K0 Q0^T + K1 Q1^T (both branches accumulate in
                          # one PSUM bank), then one causal mask (s<=t)
                          if probe_no_at:
                              atm0 = mask_t
                          else:
                              pat0 = patp.tile([C, C], F32, tag="pat")
                              if f32r:
                                  mm(pat0[:], kcT.bitcast(F32R),
                                     qcT.bitcast(F32R), start=True, stop=False)
                                  mm(pat0[:], krcT.bitcast(F32R),
                                     qrcT.bitcast(F32R), start=False, stop=True)
                              else:
                                  mm(pat0[:], kcT, qcT, start=True, stop=False)
                                  mm(pat0[:], krcT, qrcT, start=False, stop=True)
                              atm0 = atmp.tile([C, C], MT, tag="atm")
                              nc.vector.tensor_mul(atm0[:], pat0[:], mask_t[:])

                          pending[p] = dict(atm=atm0, qcT=qcT, qrcT=qrcT,
                                            vc=vc, st=st, j=j, c=c,
                                            prev_S=prev_S, kns0=kns0, kns1=kns1,
                                            c0=c0, slen=st.get("len", SLAB))

                    if ilv and cc < NCHUNK and not dma_only and not probe_pe_only:
                        ps = sorted(pending.keys())
                        # state matmuls, pair-interleaved (consecutive MMs hit
                        # different PSUM banks)
                        for br in range(2):
                            for p in ps:
                                z = pending[p]
                                if br == 0:
                                    mm(pS[p][:, 0:E1], z["knc"], z["vc"],
                                       start=(c == 0), stop=False,
                                       skip_group_check=True)
                                else:
                                    mm(pS[p][:, E1 + 1:2 * E1 + 1], z["krnc"],
                                       z["vc"], start=False,
                                       stop=(c == NCHUNK - 1),
                                       skip_group_check=True)
                        for p in ps:
                            if c < NCHUNK - 1:
                                s01 = ssbp.tile([D, 2 * E1 + 2], MT, tag="ssb",
                                                name=f"s01i_{rep}_{p}_{c}")
                                nc.scalar.copy(s01[:], pS[p][:])
                                S_sbuf[p] = s01
                        pats = {}
                        for p in ps:
                            pats[p] = patp.tile([C, C], F32, tag="pat",
                                                name=f"pati_{rep}_{p}_{c}")
                        for br in range(2):
                            for p in ps:
                                z = pending[p]
                                if br == 0:
                                    mm(pats[p][:], z["kcT"], z["qcT"],
                                       start=True, stop=False,
                                       skip_group_check=True)
                                else:
                                    mm(pats[p][:], z["krcT"], z["qrcT"],
                                       start=False, stop=True,
                                       skip_group_check=True)
                        for p in ps:
                            atm = atmp.tile([C, C], MT, tag="atm",
                                            name=f"atmi_{rep}_{p}_{c}")
                            nc.vector.tensor_mul(atm[:], pats[p][:], mask_t[:])
                            pending[p]["atm"] = atm

                    if ilv:
                        items = sorted(back.items())
                        pos = {}
                        for p, z in items:
                            pos[p] = poutp.tile([C, E1], F32, tag="po",
                                                name=f"poi_{rep}_{p}_{z['c']}")
                        for p, z in items:
                            mm(pos[p][:], z["atm"][:], z["vc"], start=True,
                               stop=(z["c"] == 0 or z["prev_S"] is None),
                               skip_group_check=True)
                        for p, z in items:
                            if z["c"] > 0 and z["prev_S"] is not None:
                                mm(pos[p][:], z["qcT"], z["prev_S"][:, 0:E1],
                                   start=False, stop=False,
                                   skip_group_check=True)
                        for p, z in items:
                            if z["c"] > 0 and z["prev_S"] is not None:
                                mm(pos[p][:], z["qrcT"],
                                   z["prev_S"][:, E1 + 1:2 * E1 + 1],
                                   start=False, stop=True,
                                   skip_group_check=True)
                        for p, z in items:
                            po = pos[p]
                            dinv = dinvp.tile([C, 1], F32, tag="dinv",
                                              name=f"dinvi_{rep}_{p}_{z['c']}")
                            nc.vector.reciprocal(dinv[:], po[:, E:E1])
                            nc.scalar.mul(z["st"]["outs"][:, z["j"], :],
                                          po[:, 0:E], dinv[:])
                            if z["j"] == z["slen"] - 1:
                                base = p * N + z["c0"] * C
                                rows = slice(base, base + z["slen"] * C)
                                nc.sync.dma_start(
                                    out[rows, :].rearrange(
                                        "(n p) e -> p n e", p=C),
                                    z["st"]["outs"][:])
                        back = {}

                    for p, z in back.items():
                        cb = z["c"]
                        # with transpose_k the state update happens here, so
                        # the pre-update state must be captured here as well
                        if transpose_k:
                            z["prev_S"] = S_sbuf.get(p)
                        if transpose_k and z["kns0"] is not None:
                            mm(pS[p][:, 0:E1], z["kns0"][:],
                                             z["vc"], start=(cb == 0),
                                             stop=False, skip_group_check=True)
                            mm(pS[p][:, E1 + 1:2 * E1 + 1],
                                             z["kns1"][:], z["vc"],
                                             start=False,
                                             stop=(cb == NCHUNK - 1),
                                             skip_group_check=True)
                            if cb < NCHUNK - 1:
                                s01 = ssbp.tile([D, 2 * E1 + 2], MT, tag="ssb")
                                nc.scalar.copy(s01[:], pS[p][:])
                                S_sbuf[p] = s01
                        # numerator (cols 0..63) + denominator (col 64)
                        po = poutp.tile([C, E1], F32, tag="po")
                        mm(po[:], z["atm"][:], z["vc"],
                                         start=True,
                                         stop=(cb == 0 or z["prev_S"] is None))
                        if cb > 0 and z["prev_S"] is not None:
                            mm(po[:], z["qcT"],
                                             z["prev_S"][:, 0:E1],
                                             start=False, stop=False,
                                             skip_group_check=True)
                            mm(po[:], z["qrcT"],
                                             z["prev_S"][:, E1 + 1:2 * E1 + 1],
                                             start=False, stop=True,
                                             skip_group_check=True)

                        if host_norm:
                            # ship numerator and denominator; host divides
                            nc.scalar.copy(z["st"]["outs"][:, z["j"], :],
                                           po[:, 0:E1])
                        else:
                            # out[t,:] = num[t,:] / den[t]
                            dinv = dinvp.tile([C, 1], F32, tag="dinv")
                            nc.vector.reciprocal(dinv[:], po[:, E:E1])
                            nc.scalar.mul(z["st"]["outs"][:, z["j"], :],
                                          po[:, 0:E], dinv[:])

                        if z["j"] == z["slen"] - 1:
                            base = p * N + z["c0"] * C
                            rows = slice(base, base + z["slen"] * C)
                            nc.sync.dma_start(
                                out[rows, :].rearrange("(n p) e -> p n e", p=C),
                                z["st"]["outs"][:])

    nc.compile()
    return nc




# Column strides inside shared PSUM banks (8-byte aligned regions)
PW = 72            # per-pair region width in the output bank (>= E1)
SW = 66            # per-(pair,branch) region width in the state bank (>= E1)


def build_kernel_m(repeat=1, loop_k=None):
    """Pair-merged variant: both (b,h) pairs handled per core share single
    PSUM banks for AT, numerator/denominator, and state, so the causal mask,
    the state evacuation, and the reciprocal each run as ONE wide
    vector/scalar op per chunk instead of one per pair. Cuts the DVE/ACT
    instruction count (and their fixed per-op drain cost) roughly in half."""
    nc = bacc.Bacc("TRN2", target_bir_lowering=False, debug=False,
                   num_devices=N_CORES)

    MT = F32  # typed-f32r rejected by walrus codegen (odd-N ISA check)

    def mm(out_ap, lhsT_ap, rhs_ap, **kw):
        if mm_f32r:
            lhsT_ap = lhsT_ap.bitcast(F32R)
            rhs_ap = rhs_ap.bitcast(F32R)
        return nc.tensor.matmul(out_ap, lhsT_ap, rhs_ap, **kw)

    qT = nc.dram_tensor("qT", [D, NROWS], MT, kind="ExternalInput").ap()
    kT = nc.dram_tensor("kT", [D, NROWS], MT, kind="ExternalInput").ap()
    qrT = nc.dram_tensor("qrT", [D, NROWS], MT, kind="ExternalInput").ap()
    krT = nc.dram_tensor("krT", [D, NROWS], MT, kind="ExternalInput").ap()
    kn = nc.dram_tensor("kn", [NROWS, D], MT, kind="ExternalInput").ap()
    krn = nc.dram_tensor("krn", [NROWS, D], MT, kind="ExternalInput").ap()
    v1 = nc.dram_tensor("v1", [NROWS, E1], MT, kind="ExternalInput").ap()
    mask2 = nc.dram_tensor("mask2", [C, 2 * C], F32, kind="ExternalInput").ap()
    out = nc.dram_tensor("out", [NROWS, E], F32, kind="ExternalOutput").ap()

    NP = PAIRS_PER_CORE  # 2

    with tile.TileContext(nc) as tc:
        with (
            tc.tile_pool(name="const", bufs=1) as constp,
            tc.tile_pool(name="slabs", bufs=6) as slabs,
            tc.tile_pool(name="atm", bufs=3) as atmp,
            tc.tile_pool(name="ssb", bufs=4) as ssbp,
            tc.tile_pool(name="dinv", bufs=8) as dinvp,
            tc.tile_pool(name="pat", bufs=3, space="PSUM") as patp,
            tc.tile_pool(name="pout", bufs=3, space="PSUM") as poutp,
            tc.tile_pool(name="pst", bufs=1, space="PSUM") as pstp,
        ):
            mask_t = constp.tile([C, 2 * C], F32, tag="mask")
            nc.sync.dma_start(mask_t[:], mask2[:])

            for rep in range(repeat):
              with (tc.For_i(0, loop_k, 1, hint_engines=(
                        mybir.EngineType.PE, mybir.EngineType.DVE,
                        mybir.EngineType.Activation, mybir.EngineType.SP))
                    if (loop_k is not None and loop_k > 1)
                    else _nullctx()):
                  # one state bank: region (p, br) at cols (2p+br)*SW
                  pSt = pstp.tile([D, 2 * NP * SW], F32, tag="pS",
                                  name=f"pSm_{rep}")

                  slab_t = [None] * NP
                  S_sbuf = [None]     # boxed: current [D, 4*SW] sbuf state

                  pending = None
                  for cc in range(NCHUNK + 1):
                    back = pending
                    pending = None
                    if cc < NCHUNK:
                      c = cc
                      sl = {}
                      for p in range(NP):
                          if c % SLAB == 0:
                              base = p * N + c * C
                              cols = slice(base, base + SLAB * C)
                              st = {}
                              st["qT"] = slabs.tile([D, slen * C], F32, tag="qT", name=f"qTs_{rep}_{p}_{c}")
                              nc.sync.dma_start(st["qT"][:], qT[:, cols])
                              st["kT"] = slabs.tile([D, slen * C], F32, tag="kT", name=f"kTs_{rep}_{p}_{c}")
                              nc.sync.dma_start(st["kT"][:], kT[:, cols])
                              st["qrT"] = slabs.tile([D, slen * C], F32, tag="qrT", name=f"qrTs_{rep}_{p}_{c}")
                              nc.sync.dma_start(st["qrT"][:], qrT[:, cols])
                              st["krT"] = slabs.tile([D, slen * C], F32, tag="krT", name=f"krTs_{rep}_{p}_{c}")
                              nc.sync.dma_start(st["krT"][:], krT[:, cols])
                              st["kn"] = slabs.tile([C, slen, D], F32, tag="kn", name=f"kns_{rep}_{p}_{c}")
                              nc.sync.dma_start(
                                  st["kn"][:],
                                  kn[cols, :].rearrange("(n p) d -> p n d", p=C))
                              st["krn"] = slabs.tile([C, slen, D], F32, tag="krn", name=f"krns_{rep}_{p}_{c}")
                              nc.sync.dma_start(
                                  st["krn"][:],
                                  krn[cols, :].rearrange("(n p) d -> p n d", p=C))
                              st["v1"] = slabs.tile([C, slen, E1], F32, tag="v1", name=f"v1s_{rep}_{p}_{c}")
                              nc.sync.dma_start(
                                  st["v1"][:],
                                  v1[cols, :].rearrange("(n p) e -> p n e", p=C))
                              st["outs"] = slabs.tile([C, SLAB, E], F32, tag="outs", name=f"outs_{rep}_{p}_{c}")
                              slab_t[p] = st

                          st = slab_t[p]
                          j = c - c0
                          sl[p] = dict(
                              st=st, j=j,
                              qcT=st["qT"][:, j * C:(j + 1) * C],
                              kcT=st["kT"][:, j * C:(j + 1) * C],
                              qrcT=st["qrT"][:, j * C:(j + 1) * C],
                              krcT=st["krT"][:, j * C:(j + 1) * C],
                              knc=st["kn"][:, j, :],
                              krnc=st["krn"][:, j, :],
                              vc=st["v1"][:, j, :],
                          )

                      prev_S = S_sbuf[0]

                      # state updates, all four into one bank
                      for p in range(NP):
                          z = sl[p]
                          nc.tensor.matmul(
                              pSt[:, (2 * p) * SW:(2 * p) * SW + E1],
                              z["knc"], z["vc"],
                              start=(c == 0 and p == 0), stop=False,
                              skip_group_check=True)
                          nc.tensor.matmul(
                              pSt[:, (2 * p + 1) * SW:(2 * p + 1) * SW + E1],
                              z["krnc"], z["vc"],
                              start=False,
                              stop=(c == NCHUNK - 1 and p == NP - 1),
                              skip_group_check=True)
                      if c < NCHUNK - 1:
                          s01 = ssbp.tile([D, 2 * NP * SW], F32, tag="ssb")
                          nc.scalar.copy(s01[:], pSt[:])
                          S_sbuf[0] = s01

                      # AT for both pairs into one bank, one mask op
                      patb = patp.tile([C, 2 * C], F32, tag="pat")
                      for p in range(NP):
                          z = sl[p]
                          reg = patb[:, p * C:(p + 1) * C]
                          nc.tensor.matmul(reg, z["kcT"], z["qcT"],
                                           start=True, stop=False,
                                           skip_group_check=True)
                          nc.tensor.matmul(reg, z["krcT"], z["qrcT"],
                                           start=False, stop=True,
                                           skip_group_check=True)
                      atm = atmp.tile([C, 2 * C], F32, tag="atm")
                      nc.vector.tensor_mul(atm[:], patb[:], mask_t[:])

                      pending = dict(atm=atm, sl=sl, c=c, prev_S=prev_S)

                    if back is not None:
                        cb = back["c"]
                        pob = poutp.tile([C, NP * PW], F32, tag="po")
                        for p in range(NP):
                            z = back["sl"][p]
                            reg = pob[:, p * PW:p * PW + E1]
                            only = (cb == 0)
                            nc.tensor.matmul(
                                reg, back["atm"][:, p * C:(p + 1) * C],
                                z["vc"], start=True, stop=only,
                                skip_group_check=True)
                            if cb > 0:
                                pv = back["prev_S"]
                                nc.tensor.matmul(
                                    reg, z["qcT"],
                                    pv[:, (2 * p) * SW:(2 * p) * SW + E1],
                                    start=False, stop=False,
                                    skip_group_check=True)
                                nc.tensor.matmul(
                                    reg, z["qrcT"],
                                    pv[:, (2 * p + 1) * SW:(2 * p + 1) * SW + E1],
                                    start=False, stop=True,
                                    skip_group_check=True)

                        # one reciprocal for both pairs' denominators
                        dinv = dinvp.tile([C, NP], F32, tag="dinv")
                        nc.vector.reciprocal(
                            dinv[:], pob[:, E:NP * PW:PW])
                        for p in range(NP):
                            z = back["sl"][p]
                            nc.scalar.mul(z["st"]["outs"][:, z["j"], :],
                                          pob[:, p * PW:p * PW + E],
                                          dinv[:, p:p + 1])
                            if z["j"] == SLAB - 1:
                                base = p * N + (cb - SLAB + 1) * C
                                rows = slice(base, base + SLAB * C)
                                nc.sync.dma_start(
                                    out[rows, :].rearrange(
                                        "(n p) e -> p n e", p=C),
                                    z["st"]["outs"][:])

    nc.compile()
    return nc



def _prepare_in_maps(q, k, q_rot, k_rot, v, transpose_k=False, merged=False):
    b, h, n, d = q.shape
    e = v.shape[-1]
    nbh = b * h
    qf = np.ascontiguousarray(q.reshape(nbh, n, d).astype(np.float32))
    kf = np.ascontiguousarray(k.reshape(nbh, n, d).astype(np.float32))
    qrf = np.ascontiguousarray(q_rot.reshape(nbh, n, d).astype(np.float32))
    krf = np.ascontiguousarray(k_rot.reshape(nbh, n, d).astype(np.float32))
    vf = np.ascontiguousarray(v.reshape(nbh, n, e).astype(np.float32))
    mask = np.triu(np.ones((C, C), dtype=np.float32))

    in_maps = []
    for i in range(N_CORES):
        sel = [PAIRS_PER_CORE * i + p for p in range(PAIRS_PER_CORE)]
        qT = np.ascontiguousarray(
            np.concatenate([qf[s].T for s in sel], axis=1))
        kT = np.ascontiguousarray(
            np.concatenate([kf[s].T for s in sel], axis=1))
        qrT = np.ascontiguousarray(
            np.concatenate([qrf[s].T for s in sel], axis=1))
        krT = np.ascontiguousarray(
            np.concatenate([krf[s].T for s in sel], axis=1))
        knat = np.ascontiguousarray(np.concatenate([kf[s] for s in sel], axis=0))
        krnat = np.ascontiguousarray(np.concatenate([krf[s] for s in sel], axis=0))
        vcat = np.concatenate([vf[s] for s in sel], axis=0)
        v1 = np.ascontiguousarray(
            np.concatenate([vcat, np.ones((vcat.shape[0], 1), np.float32)],
                           axis=1))
        m = dict(qT=qT, kT=kT, qrT=qrT, krT=krT, v1=v1)
        if merged:
            m["mask2"] = np.ascontiguousarray(np.concatenate([mask, mask], axis=1))
        else:
            m["mask"] = mask
        if transpose_k:
            m["ident"] = np.eye(C, dtype=np.float32)
        else:
            m["kn"] = knat
            m["krn"] = krnat
        in_maps.append(m)
    return in_maps


def kernel(q, k, q_rot, k_rot, v, horizon=128, **run_kwargs):
    q = np.asarray(q)
    k = np.asarray(k)
    q_rot = np.asarray(q_rot)
    k_rot = np.asarray(k_rot)
    v = np.asarray(v)
    b, h, n, d = q.shape
    e = v.shape[-1]
    assert (b * h, n, d, e) == (N_CORES * PAIRS_PER_CORE, N, D, E), \
        "kernel is hardcoded for b*h=16, n=2048, d=128, e=64"

    if "nc" not in _cached:
        _cached["nc"] = build_kernel()
    nc = _cached["nc"]

    in_maps = _prepare_in_maps(q, k, q_rot, k_rot, v)
    res = run_bass_kernel_spmd(nc, in_maps, core_ids=list(range(N_CORES)),
                               **run_kwargs)

    outf = np.empty((b * h, n, e), dtype=np.float32)
    for i in range(N_CORES):
        o = res.results[i]["out"].reshape(PAIRS_PER_CORE, n, e)
        for p in range(PAIRS_PER_CORE):
            outf[PAIRS_PER_CORE * i + p] = o[p]
    if run_kwargs:
        kernel.last_results = res
    return outf.reshape(b, h, n, e)


if __name__ == "__main__":
    rng = np.random.default_rng(0)
    q = rng.random((2, 8, N, D), dtype=np.float32)
    k = rng.random((2, 8, N, D), dtype=np.float32)
    qr = rng.standard_normal((2, 8, N, D), dtype=np.float32)
    kr = rng.standard_normal((2, 8, N, D), dtype=np.float32)
    v = rng.random((2, 8, N, E), dtype=np.float32)
    o = kernel(q, k, qr, kr, v, 128)
    print("ok", o.shape, o.dtype, np.abs(o).mean())



# revision 3
# speedup vs baseline: 1.5598x; 1.5598x over previous
"""Trainium2 Bass kernel for chunked recurrent causal linear attention.

Problem: b=2, h=8, n=2048, d=128, e=64, chunk=128, two branches (plain +
rotary) sharing one denominator.

Math (per (b,h), per chunk c, token t in chunk, with running state
S[d,e], Z[d] per branch):
    AT[s,t]   = k_s . q_t                  (s,t in chunk; masked to s<=t)
    num[t,:]  = sum_s ATm[s,t] v_s + q_t @ S      (both branches summed)
    den[t]    = sum_s ATm[s,t]   + q_t . Z        (both branches summed)
    out[t,:]  = num[t,:] / den[t]
    S += k_chunk^T v_chunk ;  Z += sum_s k_s

Sharding: 16 (b,h) pairs over 8 cores, 2 pairs per core. Host ships
pre-transposed copies of q/k/q_rot/k_rot (so no on-device transposes are
needed) plus natural-layout k/k_rot (stationary operand of the state
update) and v with a ones-column appended (fuses the denominator into
the numerator matmuls).
"""

import contextlib
import sys

_nullctx = contextlib.nullcontext

if "/opt/trn_rl_repo" not in sys.path:
    sys.path.insert(0, "/opt/trn_rl_repo")

import numpy as np

import concourse.bass as bass
import concourse.tile as tile
from concourse import bacc, mybir
from concourse.bass_utils import run_bass_kernel_spmd

F32 = mybir.dt.float32
F32R = mybir.dt.float32r

N_CORES = 8
PAIRS_PER_CORE = 2
N = 2048           # sequence length per (b,h)
D = 128            # qk head dim
E = 64             # v head dim
E1 = E + 1         # v plus ones column
C = 128            # chunk size
NCHUNK = N // C    # 16
SLAB = 4           # chunks per DMA slab
SLAB_BUFS = 6      # slab pool buffers
NROWS = PAIRS_PER_CORE * N  # 4096

_cached = {}


def build_kernel(repeat=1, loop_k=None, dma_only=False, reuse_slab=False,
                 probe_no_at=False, probe_no_state=False, transpose_k=False,
                 pipe=1, host_norm=False, dma_split=False, taper=False,
                 big_bufs=False, load_reorder=False, bank_42=False,
                 stagger=False, probe_pe_only=False, mm_f32r=False,
                 f32r=False, fast_start=False, ilv=True):
    nc = bacc.Bacc("TRN2", target_bir_lowering=False, debug=False,
                   num_devices=N_CORES)

    MT = F32  # typed-f32r rejected by walrus codegen (odd-N ISA check)

    def mm(out_ap, lhsT_ap, rhs_ap, **kw):
        if mm_f32r:
            lhsT_ap = lhsT_ap.bitcast(F32R)
            rhs_ap = rhs_ap.bitcast(F32R)
        return nc.tensor.matmul(out_ap, lhsT_ap, rhs_ap, **kw)

    qT = nc.dram_tensor("qT", [D, NROWS], MT, kind="ExternalInput").ap()
    kT = nc.dram_tensor("kT", [D, NROWS], MT, kind="ExternalInput").ap()
    qrT = nc.dram_tensor("qrT", [D, NROWS], MT, kind="ExternalInput").ap()
    krT = nc.dram_tensor("krT", [D, NROWS], MT, kind="ExternalInput").ap()
    if not transpose_k:
        kn = nc.dram_tensor("kn", [NROWS, D], MT, kind="ExternalInput").ap()
        krn = nc.dram_tensor("krn", [NROWS, D], MT, kind="ExternalInput").ap()
    else:
        ident = nc.dram_tensor("ident", [C, C], F32, kind="ExternalInput").ap()
    v1 = nc.dram_tensor("v1", [NROWS, E1], MT, kind="ExternalInput").ap()
    mask = nc.dram_tensor("mask", [C, C], F32, kind="ExternalInput").ap()
    EO = E1 if host_norm else E
    out = nc.dram_tensor("out", [NROWS, EO], F32, kind="ExternalOutput").ap()

    if taper:
        plans = [{0: 2, 2: 4, 6: 4, 10: 4, 14: 2}] * PAIRS_PER_CORE
    elif stagger:
        plans = [{c0: SLAB for c0 in range(0, NCHUNK, SLAB)},
                 {0: 2, 2: 4, 6: 4, 10: 4, 14: 2}]
    else:
        plans = [{c0: SLAB for c0 in range(0, NCHUNK, SLAB)}] * PAIRS_PER_CORE
    slab_of = []
    for pp in range(PAIRS_PER_CORE):
        m = {}
        for c0, ln in plans[pp].items():
            for c in range(c0, c0 + ln):
                m[c] = (c0, ln)
        slab_of.append(m)

    with tile.TileContext(nc) as tc:
        with (
            tc.tile_pool(name="const", bufs=1) as constp,
            tc.tile_pool(name="slabs", bufs=SLAB_BUFS) as slabs,
            tc.tile_pool(name="atm", bufs=(6 if big_bufs else (4 if pipe == 1 else 6))) as atmp,
            tc.tile_pool(name="ssb", bufs=(12 if big_bufs else 8)) as ssbp,
            tc.tile_pool(name="dinv", bufs=(12 if big_bufs else 8)) as dinvp,
            tc.tile_pool(name="pat", bufs=(2 if (transpose_k or bank_42) else 3),
                         space="PSUM") as patp,
            tc.tile_pool(name="pout", bufs=(2 if transpose_k else (4 if bank_42 else 3)),
                         space="PSUM") as poutp,
            tc.tile_pool(name="pst", bufs=2, space="PSUM") as pstp,
            tc.tile_pool(name="ktr", bufs=2, space="PSUM") as ktrp,
            tc.tile_pool(name="kns", bufs=4) as knsp,
        ):
            mask_t = constp.tile([C, C], F32, tag="mask")
            nc.sync.dma_start(mask_t[:], mask[:])
            if transpose_k:
                ident_t = constp.tile([C, C], F32, tag="ident")
                nc.sync.dma_start(ident_t[:], ident[:])

            for rep in range(repeat):
              with (tc.For_i(0, loop_k, 1, hint_engines=(
                        mybir.EngineType.PE, mybir.EngineType.DVE,
                        mybir.EngineType.Activation, mybir.EngineType.SP))
                    if (loop_k is not None and loop_k > 1)
                    else _nullctx()):
                  # per-pair state accumulator in one PSUM bank:
                  # cols 0:65 -> branch 0 [S|Z], cols 66:131 -> branch 1
                  pS = {}
                  for p in range(PAIRS_PER_CORE):
                      pS[p] = pstp.tile([D, 2 * E1 + 2], F32, tag="pS", name=f"pS_{rep}_{p}")

                  slab_t = [None] * PAIRS_PER_CORE   # per pair: dict of slab tiles
                  S_sbuf = {}                        # (p, br) -> sbuf state tile

                  # Software pipeline, one chunk deep: the "front" stage of
                  # chunk c emits loads, the state update (PE), and AT+mask
                  # (PE then DVE); the "back" stage consumes chunk c-1's
                  # masked AT for the numerator/denominator matmuls. This
                  # gives every cross-engine hop a full stage of slack, so
                  # the PE never head-of-line blocks on DVE/ACT latency.
                  fifo = []
                  for cc in range(NCHUNK + pipe):
                    pending = {}
                    back = {}
                    if cc >= pipe:
                        back = fifo.pop(0)
                    if cc < NCHUNK:
                        fifo.append(pending)
                    if cc < NCHUNK:
                      c = cc
                      for p in range(PAIRS_PER_CORE):
                          c0, slen = slab_of[p][c]
                          if (c == c0) and not (reuse_slab and c > 0):
                              base = p * N + c * C
                              cols = slice(base, base + slen * C)
                              dmae = nc.gpsimd if (dma_split and p == 1) else nc.sync
                              st = {"len": slen}
                              if not load_reorder:
                                  st["qT"] = slabs.tile([D, slen * C], MT, tag="qT", name=f"qTs_{rep}_{p}_{c}")
                                  st["kT"] = slabs.tile([D, slen * C], MT, tag="kT", name=f"kTs_{rep}_{p}_{c}")
                                  if fast_start and c == 0:
                                      # split the very first q/k loads so chunk
                                      # 0's AT matmul starts after 128KB, not
                                      # a full slab (range-level tile deps)
                                      dmae.dma_start(st["qT"][:, 0:C], qT[:, base:base + C])
                                      dmae.dma_start(st["kT"][:, 0:C], kT[:, base:base + C])
                                      dmae.dma_start(st["qT"][:, C:slen * C], qT[:, base + C:base + slen * C])
                                      dmae.dma_start(st["kT"][:, C:slen * C], kT[:, base + C:base + slen * C])
                                  else:
                                      dmae.dma_start(st["qT"][:], qT[:, cols])
                                      dmae.dma_start(st["kT"][:], kT[:, cols])
                                  st["qrT"] = slabs.tile([D, slen * C], MT, tag="qrT", name=f"qrTs_{rep}_{p}_{c}")
                                  dmae.dma_start(st["qrT"][:], qrT[:, cols])
                                  st["krT"] = slabs.tile([D, slen * C], MT, tag="krT", name=f"krTs_{rep}_{p}_{c}")
                                  dmae.dma_start(st["krT"][:], krT[:, cols])
                              # load the state-update inputs (kn/krn/v1)
                              # first: they feed the first PE ops of the chunk
                              if not transpose_k:
                                  st["kn"] = slabs.tile([C, slen, D], MT, tag="kn", name=f"kns_{rep}_{p}_{c}")
                                  dmae.dma_start(
                                      st["kn"][:],
                                      kn[cols, :].rearrange("(n p) d -> p n d", p=C))
                                  st["krn"] = slabs.tile([C, slen, D], MT, tag="krn", name=f"krns_{rep}_{p}_{c}")
                                  dmae.dma_start(
                                      st["krn"][:],
                                      krn[cols, :].rearrange("(n p) d -> p n d", p=C))
                              st["v1"] = slabs.tile([C, slen, E1], MT, tag="v1", name=f"v1s_{rep}_{p}_{c}")
                              dmae.dma_start(
                                  st["v1"][:],
                                  v1[cols, :].rearrange("(n p) e -> p n e", p=C))
                              if load_reorder:
                                  st["kT"] = slabs.tile([D, slen * C], MT, tag="kT", name=f"kTs_{rep}_{p}_{c}")
                                  dmae.dma_start(st["kT"][:], kT[:, cols])
                                  st["qT"] = slabs.tile([D, slen * C], MT, tag="qT", name=f"qTs_{rep}_{p}_{c}")
                                  dmae.dma_start(st["qT"][:], qT[:, cols])
                                  st["qrT"] = slabs.tile([D, slen * C], MT, tag="qrT", name=f"qrTs_{rep}_{p}_{c}")
                                  dmae.dma_start(st["qrT"][:], qrT[:, cols])
                                  st["krT"] = slabs.tile([D, slen * C], MT, tag="krT", name=f"krTs_{rep}_{p}_{c}")
                                  dmae.dma_start(st["krT"][:], krT[:, cols])
                              st["outs"] = slabs.tile([C, slen, EO], F32, tag="outs", name=f"outs_{rep}_{p}_{c}")
                              slab_t[p] = st

                          st = slab_t[p]
                          j = c - c0
                          qcT = st["qT"][:, j * C:(j + 1) * C]
                          kcT = st["kT"][:, j * C:(j + 1) * C]
                          qrcT = st["qrT"][:, j * C:(j + 1) * C]
                          krcT = st["krT"][:, j * C:(j + 1) * C]
                          vc = st["v1"][:, j, :]
                          knc = krnc = None
                          if not transpose_k:
                              knc = st["kn"][:, j, :]
                              krnc = st["krn"][:, j, :]

                          if dma_only:
                              continue

                          if probe_pe_only:
                              # pure matmul throughput probe: same 7 MMs as the
                              # real kernel, but no cross-engine deps at all
                              pat0 = patp.tile([C, C], F32, tag="pat")
                              mm(pat0[:], kcT, qcT, start=True, stop=False)
                              mm(pat0[:], krcT, qrcT, start=False, stop=True)
                              po = poutp.tile([C, E1], F32, tag="po")
                              mm(po[:], mask_t[:], vc, start=True, stop=False)
                              mm(po[:], qcT, mask_t[:, 0:E1], start=False, stop=False, skip_group_check=True)
                              mm(po[:], qrcT, mask_t[:, 0:E1], start=False, stop=True, skip_group_check=True)
                              mm(pS[p][:, 0:E1], knc, vc, start=(c == 0), stop=False, skip_group_check=True)
                              mm(pS[p][:, E1 + 1:2 * E1 + 1], krnc, vc, start=False, stop=(c == NCHUNK - 1), skip_group_check=True)
                              continue

                          prev_S = S_sbuf.get(p)

                          if ilv:
                              # MMs emitted pair-interleaved after this loop
                              pending[p] = dict(qcT=qcT, qrcT=qrcT, kcT=kcT,
                                                krcT=krcT, knc=knc, krnc=krnc,
                                                vc=vc, st=st, j=j, c=c,
                                                prev_S=prev_S, kns0=None,
                                                kns1=None, c0=c0,
                                                slen=st.get("len", SLAB))
                              continue

                          # State update: both branches share one PSUM bank
                          # (start=True on c0/br0 clears it; br1 overwrites its
                          # unwritten columns). Without transpose_k the natural-
                          # layout k arrives by DMA and the update is emitted
                          # here (front stage); with transpose_k the k tiles are
                          # transposed on the PE this stage and the state update
                          # moves to the back stage so the transpose->copy->
                          # matmul chain gets a stage of slack.
                          kns0 = kns1 = None
                          if transpose_k and not probe_no_state:
                              ktp0 = ktrp.tile([C, C], F32, tag="ktr")
                              nc.tensor.transpose(ktp0[:], kcT, ident_t[:])
                              kns0 = knsp.tile([C, C], F32, tag="kns")
                              nc.vector.tensor_copy(kns0[:], ktp0[:])
                              ktp1 = ktrp.tile([C, C], F32, tag="ktr")
                              nc.tensor.transpose(ktp1[:], krcT, ident_t[:])
                              kns1 = knsp.tile([C, C], F32, tag="kns")
                              nc.scalar.copy(kns1[:], ktp1[:])
                          if not transpose_k and not probe_no_state:
                              mm(pS[p][:, 0:E1], knc, vc,
                                               start=(c == 0), stop=False,
                                               skip_group_check=True)
                              mm(pS[p][:, E1 + 1:2 * E1 + 1], krnc, vc,
                                               start=False, stop=(c == NCHUNK - 1),
                                               skip_group_check=True)
                              if c < NCHUNK - 1:
                                  s01 = ssbp.tile([D, 2 * E1 + 2], MT, tag="ssb")
                                  nc.scalar.copy(s01[:], pS[p][:])
                                  S_sbuf[p] = s01

                          # AT = K0 Q0^T + K1 Q1^T (both branches accumulate in
                          # one PSUM bank), then one causal mask (s<=t)
                          if probe_no_at:
                              atm0 = mask_t
                          else:
                              pat0 = patp.tile([C, C], F32, tag="pat")
                              if f32r:
                                  mm(pat0[:], kcT.bitcast(F32R),
                                     qcT.bitcast(F32R), start=True, stop=False)
                                  mm(pat0[:], krcT.bitcast(F32R),
                                     qrcT.bitcast(F32R), start=False, stop=True)
                              else:
                                  mm(pat0[:], kcT, qcT, start=True, stop=False)
                                  mm(pat0[:], krcT, qrcT, start=False, stop=True)
                              atm0 = atmp.tile([C, C], MT, tag="atm")
                              nc.vector.tensor_mul(atm0[:], pat0[:], mask_t[:])

                          pending[p] = dict(atm=atm0, qcT=qcT, qrcT=qrcT,
                                            vc=vc, st=st, j=j, c=c,
                                            prev_S=prev_S, kns0=kns0, kns1=kns1,
                                            c0=c0, slen=st.get("len", SLAB))

                    if ilv and cc < NCHUNK and not dma_only and not probe_pe_only:
                        ps = sorted(pending.keys())
                        # state matmuls, pair-interleaved (consecutive MMs hit
                        # different PSUM banks)
                        for br in range(2):
                            for p in ps:
                                z = pending[p]
                                if br == 0:
                                    mm(pS[p][:, 0:E1], z["knc"], z["vc"],
                                       start=(c == 0), stop=False,
                                       skip_group_check=True)
                                else:
                                    mm(pS[p][:, E1 + 1:2 * E1 + 1], z["krnc"],
                                       z["vc"], start=False,
                                       stop=(c == NCHUNK - 1),
                                       skip_group_check=True)
                        for p in ps:
                            if c < NCHUNK - 1:
                                s01 = ssbp.tile([D, 2 * E1 + 2], MT, tag="ssb",
                                                name=f"s01i_{rep}_{p}_{c}")
                                nc.scalar.copy(s01[:], pS[p][:])
                                S_sbuf[p] = s01
                        pats = {}
                        for p in ps:
                            pats[p] = patp.tile([C, C], F32, tag="pat",
                                                name=f"pati_{rep}_{p}_{c}")
                        for br in range(2):
                            for p in ps:
                                z = pending[p]
                                if br == 0:
                                    mm(pats[p][:], z["kcT"], z["qcT"],
                                       start=True, stop=False,
                                       skip_group_check=True)
                                else:
                                    mm(pats[p][:], z["krcT"], z["qrcT"],
                                       start=False, stop=True,
                                       skip_group_check=True)
                        for p in ps:
                            atm = atmp.tile([C, C], MT, tag="atm",
                                            name=f"atmi_{rep}_{p}_{c}")
                            nc.vector.tensor_mul(atm[:], pats[p][:], mask_t[:])
                            pending[p]["atm"] = atm

                    if ilv:
                        items = sorted(back.items())
                        pos = {}
                        for p, z in items:
                            pos[p] = poutp.tile([C, E1], F32, tag="po",
                                                name=f"poi_{rep}_{p}_{z['c']}")
                        for p, z in items:
                            mm(pos[p][:], z["atm"][:], z["vc"], start=True,
                               stop=(z["c"] == 0 or z["prev_S"] is None),
                               skip_group_check=True)
                        for p, z in items:
                            if z["c"] > 0 and z["prev_S"] is not None:
                                mm(pos[p][:], z["qcT"], z["prev_S"][:, 0:E1],
                                   start=False, stop=False,
                                   skip_group_check=True)
                        for p, z in items:
                            if z["c"] > 0 and z["prev_S"] is not None:
                                mm(pos[p][:], z["qrcT"],
                                   z["prev_S"][:, E1 + 1:2 * E1 + 1],
                                   start=False, stop=True,
                                   skip_group_check=True)
                        for p, z in items:
                            po = pos[p]
                            dinv = dinvp.tile([C, 1], F32, tag="dinv",
                                              name=f"dinvi_{rep}_{p}_{z['c']}")
                            nc.vector.reciprocal(dinv[:], po[:, E:E1])
                            nc.scalar.mul(z["st"]["outs"][:, z["j"], :],
                                          po[:, 0:E], dinv[:])
                            if z["j"] == z["slen"] - 1:
                                base = p * N + z["c0"] * C
                                rows = slice(base, base + z["slen"] * C)
                                nc.sync.dma_start(
                                    out[rows, :].rearrange(
                                        "(n p) e -> p n e", p=C),
                                    z["st"]["outs"][:])
                        back = {}

                    for p, z in back.items():
                        cb = z["c"]
                        # with transpose_k the state update happens here, so
                        # the pre-update state must be captured here as well
                        if transpose_k:
                            z["prev_S"] = S_sbuf.get(p)
                        if transpose_k and z["kns0"] is not None:
                            mm(pS[p][:, 0:E1], z["kns0"][:],
                                             z["vc"], start=(cb == 0),
                                             stop=False, skip_group_check=True)
                            mm(pS[p][:, E1 + 1:2 * E1 + 1],
                                             z["kns1"][:], z["vc"],
                                             start=False,
                                             stop=(cb == NCHUNK - 1),
                                             skip_group_check=True)
                            if cb < NCHUNK - 1:
                                s01 = ssbp.tile([D, 2 * E1 + 2], MT, tag="ssb")
                                nc.scalar.copy(s01[:], pS[p][:])
                                S_sbuf[p] = s01
                        # numerator (cols 0..63) + denominator (col 64)
                        po = poutp.tile([C, E1], F32, tag="po")
                        mm(po[:], z["atm"][:], z["vc"],
                                         start=True,
                                         stop=(cb == 0 or z["prev_S"] is None))
                        if cb > 0 and z["prev_S"] is not None:
                            mm(po[:], z["qcT"],
                                             z["prev_S"][:, 0:E1],
                                             start=False, stop=False,
                                             skip_group_check=True)
                            mm(po[:], z["qrcT"],
                                             z["prev_S"][:, E1 + 1:2 * E1 + 1],
                                             start=False, stop=True,
                                             skip_group_check=True)

                        if host_norm:
                            # ship numerator and denominator; host divides
                            nc.scalar.copy(z["st"]["outs"][:, z["j"], :],
                                           po[:, 0:E1])
                        else:
                            # out[t,:] = num[t,:] / den[t]
                            dinv = dinvp.tile([C, 1], F32, tag="dinv")
                            nc.vector.reciprocal(dinv[:], po[:, E:E1])
                            nc.scalar.mul(z["st"]["outs"][:, z["j"], :],
                                          po[:, 0:E], dinv[:])

                        if z["j"] == z["slen"] - 1:
                            base = p * N + z["c0"] * C
                            rows = slice(base, base + z["slen"] * C)
                            nc.sync.dma_start(
                                out[rows, :].rearrange("(n p) e -> p n e", p=C),
                                z["st"]["outs"][:])

    nc.compile()
    return nc




F16 = mybir.dt.float16


def build_kernel16(repeat=1, loop_k=None, dma_only=False, probe_pe_only=False,
                   slab=SLAB, slab_bufs=SLAB_BUFS):
    """fp16 variant. All inputs ship as fp16; natural-layout tensors
    (kn/krn/v1) and the output use a chunk-major [C, nchunk*f] DRAM layout so
    every DMA descriptor is a contiguous >=512B run. fp16 matmuls run at 1
    cycle/row on the PE (vs 4 for fp32), accumulation stays f32 in PSUM.
    Host un-permutes the output and upcasts to f32."""
    nc = bacc.Bacc("TRN2", target_bir_lowering=False, debug=False,
                   num_devices=N_CORES)

    NPC = PAIRS_PER_CORE
    qT = nc.dram_tensor("qT", [D, NROWS], F16, kind="ExternalInput").ap()
    kT = nc.dram_tensor("kT", [D, NROWS], F16, kind="ExternalInput").ap()
    qrT = nc.dram_tensor("qrT", [D, NROWS], F16, kind="ExternalInput").ap()
    krT = nc.dram_tensor("krT", [D, NROWS], F16, kind="ExternalInput").ap()
    kn = nc.dram_tensor("kn", [C, NPC * NCHUNK * D], F16, kind="ExternalInput").ap()
    krn = nc.dram_tensor("krn", [C, NPC * NCHUNK * D], F16, kind="ExternalInput").ap()
    v1 = nc.dram_tensor("v1", [C, NPC * NCHUNK * E1], F16, kind="ExternalInput").ap()
    mask = nc.dram_tensor("mask", [C, C], F32, kind="ExternalInput").ap()
    out = nc.dram_tensor("out", [C, NPC * NCHUNK * E], F16, kind="ExternalOutput").ap()

    plans = [{c0: slab for c0 in range(0, NCHUNK, slab)}] * NPC
    slab_of = []
    for pp in range(NPC):
        m = {}
        for c0, ln in plans[pp].items():
            for c in range(c0, c0 + ln):
                m[c] = (c0, ln)
        slab_of.append(m)

    with tile.TileContext(nc) as tc:
        with (
            tc.tile_pool(name="const", bufs=1) as constp,
            tc.tile_pool(name="slabs", bufs=slab_bufs) as slabs,
            tc.tile_pool(name="atm", bufs=4) as atmp,
            tc.tile_pool(name="ssb", bufs=8) as ssbp,
            tc.tile_pool(name="dinv", bufs=8) as dinvp,
            tc.tile_pool(name="pat", bufs=3, space="PSUM") as patp,
            tc.tile_pool(name="pout", bufs=3, space="PSUM") as poutp,
            tc.tile_pool(name="pst", bufs=2, space="PSUM") as pstp,
        ):
            mask_t = constp.tile([C, C], F32, tag="mask")
            nc.sync.dma_start(mask_t[:], mask[:])

            for rep in range(repeat):
              with (tc.For_i(0, loop_k, 1, hint_engines=(
                        mybir.EngineType.PE, mybir.EngineType.DVE,
                        mybir.EngineType.Activation, mybir.EngineType.SP))
                    if (loop_k is not None and loop_k > 1)
                    else _nullctx()):
                  pS = {}
                  for p in range(NPC):
                      pS[p] = pstp.tile([D, 2 * E1 + 2], F32, tag="pS",
                                        name=f"pS16_{rep}_{p}")

                  slab_t = [None] * NPC
                  S_sbuf = {}

                  fifo = []
                  for cc in range(NCHUNK + 1):
                    pending = {}
                    back = {}
                    if cc >= 1:
                        back = fifo.pop(0)
                    if cc < NCHUNK:
                        fifo.append(pending)
                    if cc < NCHUNK:
                      c = cc
                      for p in range(NPC):
                          c0, slen = slab_of[p][c]
                          if c == c0:
                              base = p * N + c * C
                              cols = slice(base, base + slen * C)
                              ncols = slice((p * NCHUNK + c) * D,
                                            (p * NCHUNK + c + slen) * D)
                              vcols = slice((p * NCHUNK + c) * E1,
                                            (p * NCHUNK + c + slen) * E1)
                              st = {"len": slen}
                              st["qT"] = slabs.tile([D, slen * C], F16, tag="qT", name=f"qTs16_{rep}_{p}_{c}")
                              nc.sync.dma_start(st["qT"][:], qT[:, cols])
                              st["kT"] = slabs.tile([D, slen * C], F16, tag="kT", name=f"kTs16_{rep}_{p}_{c}")
                              nc.sync.dma_start(st["kT"][:], kT[:, cols])
                              st["qrT"] = slabs.tile([D, slen * C], F16, tag="qrT", name=f"qrTs16_{rep}_{p}_{c}")
                              nc.sync.dma_start(st["qrT"][:], qrT[:, cols])
                              st["krT"] = slabs.tile([D, slen * C], F16, tag="krT", name=f"krTs16_{rep}_{p}_{c}")
                              nc.sync.dma_start(st["krT"][:], krT[:, cols])
                              st["kn"] = slabs.tile([C, slen * D], F16, tag="kn", name=f"kns16_{rep}_{p}_{c}")
                              nc.sync.dma_start(st["kn"][:], kn[:, ncols])
                              st["krn"] = slabs.tile([C, slen * D], F16, tag="krn", name=f"krns16_{rep}_{p}_{c}")
                              nc.sync.dma_start(st["krn"][:], krn[:, ncols])
                              st["v1"] = slabs.tile([C, slen * E1], F16, tag="v1", name=f"v1s16_{rep}_{p}_{c}")
                              nc.sync.dma_start(st["v1"][:], v1[:, vcols])
                              st["outs"] = slabs.tile([C, slen * E], F16, tag="outs", name=f"outs16_{rep}_{p}_{c}")
                              slab_t[p] = st

                          st = slab_t[p]
                          j = c - c0
                          if dma_only:
                              continue
                          z = dict(
                              qcT=st["qT"][:, j * C:(j + 1) * C],
                              kcT=st["kT"][:, j * C:(j + 1) * C],
                              qrcT=st["qrT"][:, j * C:(j + 1) * C],
                              krcT=st["krT"][:, j * C:(j + 1) * C],
                              knc=st["kn"][:, j * D:(j + 1) * D],
                              krnc=st["krn"][:, j * D:(j + 1) * D],
                              vc=st["v1"][:, j * E1:(j + 1) * E1],
                              st=st, j=j, c=c, c0=c0, slen=slen,
                              prev_S=S_sbuf.get(p))
                          pending[p] = z

                      if probe_pe_only and pending:
                          for p, z in sorted(pending.items()):
                              pat0 = patp.tile([C, C], F32, tag="pat")
                              nc.tensor.matmul(pat0[:], z["kcT"], z["qcT"], start=True, stop=False)
                              nc.tensor.matmul(pat0[:], z["krcT"], z["qrcT"], start=False, stop=True)
                              po = poutp.tile([C, E1], F32, tag="po")
                              nc.tensor.matmul(po[:], z["qcT"], mask_t[:, 0:E1].bitcast(F16)[:, 0:E1], start=True, stop=False, skip_group_check=True)
                              nc.tensor.matmul(po[:], z["qrcT"], mask_t[:, 0:E1].bitcast(F16)[:, 0:E1], start=False, stop=False, skip_group_check=True)
                              nc.tensor.matmul(po[:], z["kcT"], mask_t[:, 0:E1].bitcast(F16)[:, 0:E1], start=False, stop=True, skip_group_check=True)
                              nc.tensor.matmul(pS[p][:, 0:E1], z["knc"], z["vc"], start=(z["c"] == 0), stop=False, skip_group_check=True)
                              nc.tensor.matmul(pS[p][:, E1 + 1:2 * E1 + 1], z["krnc"], z["vc"], start=False, stop=(z["c"] == NCHUNK - 1), skip_group_check=True)
                          continue

                      if pending and not dma_only:
                        ps = sorted(pending.keys())
                        for br in range(2):
                            for p in ps:
                                z = pending[p]
                                if br == 0:
                                    nc.tensor.matmul(pS[p][:, 0:E1], z["knc"],
                                                     z["vc"], start=(c == 0),
                                                     stop=False,
                                                     skip_group_check=True)
                                else:
                                    nc.tensor.matmul(pS[p][:, E1 + 1:2 * E1 + 1],
                                                     z["krnc"], z["vc"],
                                                     start=False,
                                                     stop=(c == NCHUNK - 1),
                                                     skip_group_check=True)
                        for p in ps:
                            if c < NCHUNK - 1:
                                s01 = ssbp.tile([D, 2 * E1 + 2], F16, tag="ssb",
                                                name=f"s01h_{rep}_{p}_{c}")
                                nc.scalar.copy(s01[:], pS[p][:])
                                S_sbuf[p] = s01
                        pats = {}
                        for p in ps:
                            pats[p] = patp.tile([C, C], F32, tag="pat",
                                                name=f"path_{rep}_{p}_{c}")
                        for br in range(2):
                            for p in ps:
                                z = pending[p]
                                if br == 0:
                                    nc.tensor.matmul(pats[p][:], z["kcT"],
                                                     z["qcT"], start=True,
                                                     stop=False,
                                                     skip_group_check=True)
                                else:
                                    nc.tensor.matmul(pats[p][:], z["krcT"],
                                                     z["qrcT"], start=False,
                                                     stop=True,
                                                     skip_group_check=True)
                        for p in ps:
                            atm = atmp.tile([C, C], F16, tag="atm",
                                            name=f"atmh_{rep}_{p}_{c}")
                            nc.vector.tensor_mul(atm[:], pats[p][:], mask_t[:])
                            pending[p]["atm"] = atm

                    if back and not dma_only and not probe_pe_only:
                        items = sorted(back.items())
                        pos = {}
                        for p, z in items:
                            pos[p] = poutp.tile([C, E1], F32, tag="po",
                                                name=f"poh_{rep}_{p}_{z['c']}")
                        for p, z in items:
                            nc.tensor.matmul(pos[p][:], z["atm"][:], z["vc"],
                                             start=True,
                                             stop=(z["c"] == 0 or z["prev_S"] is None),
                                             skip_group_check=True)
                        for p, z in items:
                            if z["c"] > 0 and z["prev_S"] is not None:
                                nc.tensor.matmul(pos[p][:], z["qcT"],
                                                 z["prev_S"][:, 0:E1],
                                                 start=False, stop=False,
                                                 skip_group_check=True)
                        for p, z in items:
                            if z["c"] > 0 and z["prev_S"] is not None:
                                nc.tensor.matmul(pos[p][:], z["qrcT"],
                                                 z["prev_S"][:, E1 + 1:2 * E1 + 1],
                                                 start=False, stop=True,
                                                 skip_group_check=True)
                        for p, z in items:
                            po = pos[p]
                            dinv = dinvp.tile([C, 1], F32, tag="dinv",
                                              name=f"dinvh_{rep}_{p}_{z['c']}")
                            nc.vector.reciprocal(dinv[:], po[:, E:E1])
                            nc.scalar.mul(
                                z["st"]["outs"][:, z["j"] * E:(z["j"] + 1) * E],
                                po[:, 0:E], dinv[:])
                            if z["j"] == z["slen"] - 1:
                                ocols = slice((p * NCHUNK + z["c0"]) * E,
                                              (p * NCHUNK + z["c0"] + z["slen"]) * E)
                                nc.sync.dma_start(out[:, ocols],
                                                  z["st"]["outs"][:])

    nc.compile()
    return nc


def _prepare_in_maps16(q, k, q_rot, k_rot, v):
    b, h, n, d = q.shape
    e = v.shape[-1]
    nbh = b * h
    qf = q.reshape(nbh, n, d)
    kf = k.reshape(nbh, n, d)
    qrf = q_rot.reshape(nbh, n, d)
    krf = k_rot.reshape(nbh, n, d)
    vf = v.reshape(nbh, n, e)
    mask = np.triu(np.ones((C, C), dtype=np.float32))

    def chunk_major(x):
        # [n, f] -> [C, NCHUNK * f]: column-major-by-chunk on-chip layout
        f = x.shape[-1]
        return x.reshape(NCHUNK, C, f).transpose(1, 0, 2).reshape(C, NCHUNK * f)

    in_maps = []
    for i in range(N_CORES):
        sel = [PAIRS_PER_CORE * i + p for p in range(PAIRS_PER_CORE)]
        qT = np.concatenate([qf[s].T for s in sel], axis=1).astype(np.float16)
        kT = np.concatenate([kf[s].T for s in sel], axis=1).astype(np.float16)
        qrT = np.concatenate([qrf[s].T for s in sel], axis=1).astype(np.float16)
        krT = np.concatenate([krf[s].T for s in sel], axis=1).astype(np.float16)
        kn = np.concatenate([chunk_major(kf[s]) for s in sel], axis=1).astype(np.float16)
        krn = np.concatenate([chunk_major(krf[s]) for s in sel], axis=1).astype(np.float16)
        v1 = np.concatenate(
            [chunk_major(np.concatenate(
                [vf[s], np.ones((n, 1), vf.dtype)], axis=1)) for s in sel],
            axis=1).astype(np.float16)
        in_maps.append(dict(qT=np.ascontiguousarray(qT),
                            kT=np.ascontiguousarray(kT),
                            qrT=np.ascontiguousarray(qrT),
                            krT=np.ascontiguousarray(krT),
                            kn=np.ascontiguousarray(kn),
                            krn=np.ascontiguousarray(krn),
                            v1=np.ascontiguousarray(v1),
                            mask=mask))
    return in_maps


def kernel16(q, k, q_rot, k_rot, v, horizon=128, **run_kwargs):
    q = np.asarray(q)
    k = np.asarray(k)
    q_rot = np.asarray(q_rot)
    k_rot = np.asarray(k_rot)
    v = np.asarray(v)
    b, h, n, d = q.shape
    e = v.shape[-1]
    assert (b * h, n, d, e) == (N_CORES * PAIRS_PER_CORE, N, D, E)

    if "nc16" not in _cached:
        _cached["nc16"] = build_kernel16()
    nc = _cached["nc16"]

    in_maps = _prepare_in_maps16(q, k, q_rot, k_rot, v)
    res = run_bass_kernel_spmd(nc, in_maps, core_ids=list(range(N_CORES)),
                               **run_kwargs)

    outf = np.empty((b * h, n, e), dtype=np.float32)
    for i in range(N_CORES):
        o = res.results[i]["out"]  # [C, PAIRS*NCHUNK*E] fp16
        o = o.reshape(C, PAIRS_PER_CORE, NCHUNK, E).astype(np.float32)
        for p in range(PAIRS_PER_CORE):
            outf[PAIRS_PER_CORE * i + p] = o[:, p].transpose(1, 0, 2).reshape(n, e)
    if run_kwargs:
        kernel16.last_results = res
    return outf.reshape(b, h, n, e)


# Column strides inside shared PSUM banks (8-byte aligned regions)
PW = 72            # per-pair region width in the output bank (>= E1)
SW = 66            # per-(pair,branch) region width in the state bank (>= E1)


def build_kernel_m(repeat=1, loop_k=None):
    """Pair-merged variant: both (b,h) pairs handled per core share single
    PSUM banks for AT, numerator/denominator, and state, so the causal mask,
    the state evacuation, and the reciprocal each run as ONE wide
    vector/scalar op per chunk instead of one per pair. Cuts the DVE/ACT
    instruction count (and their fixed per-op drain cost) roughly in half."""
    nc = bacc.Bacc("TRN2", target_bir_lowering=False, debug=False,
                   num_devices=N_CORES)

    MT = F32  # typed-f32r rejected by walrus codegen (odd-N ISA check)

    def mm(out_ap, lhsT_ap, rhs_ap, **kw):
        if mm_f32r:
            lhsT_ap = lhsT_ap.bitcast(F32R)
            rhs_ap = rhs_ap.bitcast(F32R)
        return nc.tensor.matmul(out_ap, lhsT_ap, rhs_ap, **kw)

    qT = nc.dram_tensor("qT", [D, NROWS], MT, kind="ExternalInput").ap()
    kT = nc.dram_tensor("kT", [D, NROWS], MT, kind="ExternalInput").ap()
    qrT = nc.dram_tensor("qrT", [D, NROWS], MT, kind="ExternalInput").ap()
    krT = nc.dram_tensor("krT", [D, NROWS], MT, kind="ExternalInput").ap()
    kn = nc.dram_tensor("kn", [NROWS, D], MT, kind="ExternalInput").ap()
    krn = nc.dram_tensor("krn", [NROWS, D], MT, kind="ExternalInput").ap()
    v1 = nc.dram_tensor("v1", [NROWS, E1], MT, kind="ExternalInput").ap()
    mask2 = nc.dram_tensor("mask2", [C, 2 * C], F32, kind="ExternalInput").ap()
    out = nc.dram_tensor("out", [NROWS, E], F32, kind="ExternalOutput").ap()

    NP = PAIRS_PER_CORE  # 2

    with tile.TileContext(nc) as tc:
        with (
            tc.tile_pool(name="const", bufs=1) as constp,
            tc.tile_pool(name="slabs", bufs=6) as slabs,
            tc.tile_pool(name="atm", bufs=3) as atmp,
            tc.tile_pool(name="ssb", bufs=4) as ssbp,
            tc.tile_pool(name="dinv", bufs=8) as dinvp,
            tc.tile_pool(name="pat", bufs=3, space="PSUM") as patp,
            tc.tile_pool(name="pout", bufs=3, space="PSUM") as poutp,
            tc.tile_pool(name="pst", bufs=1, space="PSUM") as pstp,
        ):
            mask_t = constp.tile([C, 2 * C], F32, tag="mask")
            nc.sync.dma_start(mask_t[:], mask2[:])

            for rep in range(repeat):
              with (tc.For_i(0, loop_k, 1, hint_engines=(
                        mybir.EngineType.PE, mybir.EngineType.DVE,
                        mybir.EngineType.Activation, mybir.EngineType.SP))
                    if (loop_k is not None and loop_k > 1)
                    else _nullctx()):
                  # one state bank: region (p, br) at cols (2p+br)*SW
                  pSt = pstp.tile([D, 2 * NP * SW], F32, tag="pS",
                                  name=f"pSm_{rep}")

                  slab_t = [None] * NP
                  S_sbuf = [None]     # boxed: current [D, 4*SW] sbuf state

                  pending = None
                  for cc in range(NCHUNK + 1):
                    back = pending
                    pending = None
                    if cc < NCHUNK:
                      c = cc
                      sl = {}
                      for p in range(NP):
                          if c % SLAB == 0:
                              base = p * N + c * C
                              cols = slice(base, base + SLAB * C)
                              st = {}
                              st["qT"] = slabs.tile([D, slen * C], F32, tag="qT", name=f"qTs_{rep}_{p}_{c}")
                              nc.sync.dma_start(st["qT"][:], qT[:, cols])
                              st["kT"] = slabs.tile([D, slen * C], F32, tag="kT", name=f"kTs_{rep}_{p}_{c}")
                              nc.sync.dma_start(st["kT"][:], kT[:, cols])
                              st["qrT"] = slabs.tile([D, slen * C], F32, tag="qrT", name=f"qrTs_{rep}_{p}_{c}")
                              nc.sync.dma_start(st["qrT"][:], qrT[:, cols])
                              st["krT"] = slabs.tile([D, slen * C], F32, tag="krT", name=f"krTs_{rep}_{p}_{c}")
                              nc.sync.dma_start(st["krT"][:], krT[:, cols])
                              st["kn"] = slabs.tile([C, slen, D], F32, tag="kn", name=f"kns_{rep}_{p}_{c}")
                              nc.sync.dma_start(
                                  st["kn"][:],
                                  kn[cols, :].rearrange("(n p) d -> p n d", p=C))
                              st["krn"] = slabs.tile([C, slen, D], F32, tag="krn", name=f"krns_{rep}_{p}_{c}")
                              nc.sync.dma_start(
                                  st["krn"][:],
                                  krn[cols, :].rearrange("(n p) d -> p n d", p=C))
                              st["v1"] = slabs.tile([C, slen, E1], F32, tag="v1", name=f"v1s_{rep}_{p}_{c}")
                              nc.sync.dma_start(
                                  st["v1"][:],
                                  v1[cols, :].rearrange("(n p) e -> p n e", p=C))
                              st["outs"] = slabs.tile([C, SLAB, E], F32, tag="outs", name=f"outs_{rep}_{p}_{c}")
                              slab_t[p] = st

                          st = slab_t[p]
                          j = c - c0
                          sl[p] = dict(
                              st=st, j=j,
                              qcT=st["qT"][:, j * C:(j + 1) * C],
                              kcT=st["kT"][:, j * C:(j + 1) * C],
                              qrcT=st["qrT"][:, j * C:(j + 1) * C],
                              krcT=st["krT"][:, j * C:(j + 1) * C],
                              knc=st["kn"][:, j, :],
                              krnc=st["krn"][:, j, :],
                              vc=st["v1"][:, j, :],
                          )

                      prev_S = S_sbuf[0]

                      # state updates, all four into one bank
                      for p in range(NP):
                          z = sl[p]
                          nc.tensor.matmul(
                              pSt[:, (2 * p) * SW:(2 * p) * SW + E1],
                              z["knc"], z["vc"],
                              start=(c == 0 and p == 0), stop=False,
                              skip_group_check=True)
                          nc.tensor.matmul(
                              pSt[:, (2 * p + 1) * SW:(2 * p + 1) * SW + E1],
                              z["krnc"], z["vc"],
                              start=False,
                              stop=(c == NCHUNK - 1 and p == NP - 1),
                              skip_group_check=True)
                      if c < NCHUNK - 1:
                          s01 = ssbp.tile([D, 2 * NP * SW], F32, tag="ssb")
                          nc.scalar.copy(s01[:], pSt[:])
                          S_sbuf[0] = s01

                      # AT for both pairs into one bank, one mask op
                      patb = patp.tile([C, 2 * C], F32, tag="pat")
                      for p in range(NP):
                          z = sl[p]
                          reg = patb[:, p * C:(p + 1) * C]
                          nc.tensor.matmul(reg, z["kcT"], z["qcT"],
                                           start=True, stop=False,
                                           skip_group_check=True)
                          nc.tensor.matmul(reg, z["krcT"], z["qrcT"],
                                           start=False, stop=True,
                                           skip_group_check=True)
                      atm = atmp.tile([C, 2 * C], F32, tag="atm")
                      nc.vector.tensor_mul(atm[:], patb[:], mask_t[:])

                      pending = dict(atm=atm, sl=sl, c=c, prev_S=prev_S)

                    if back is not None:
                        cb = back["c"]
                        pob = poutp.tile([C, NP * PW], F32, tag="po")
                        for p in range(NP):
                            z = back["sl"][p]
                            reg = pob[:, p * PW:p * PW + E1]
                            only = (cb == 0)
                            nc.tensor.matmul(
                                reg, back["atm"][:, p * C:(p + 1) * C],
                                z["vc"], start=True, stop=only,
                                skip_group_check=True)
                            if cb > 0:
                                pv = back["prev_S"]
                                nc.tensor.matmul(
                                    reg, z["qcT"],
                                    pv[:, (2 * p) * SW:(2 * p) * SW + E1],
                                    start=False, stop=False,
                                    skip_group_check=True)
                                nc.tensor.matmul(
                                    reg, z["qrcT"],
                                    pv[:, (2 * p + 1) * SW:(2 * p + 1) * SW + E1],
                                    start=False, stop=True,
                                    skip_group_check=True)

                        # one reciprocal for both pairs' denominators
                        dinv = dinvp.tile([C, NP], F32, tag="dinv")
                        nc.vector.reciprocal(
                            dinv[:], pob[:, E:NP * PW:PW])
                        for p in range(NP):
                            z = back["sl"][p]
                            nc.scalar.mul(z["st"]["outs"][:, z["j"], :],
                                          pob[:, p * PW:p * PW + E],
                                          dinv[:, p:p + 1])
                            if z["j"] == SLAB - 1:
                                base = p * N + (cb - SLAB + 1) * C
                                rows = slice(base, base + SLAB * C)
                                nc.sync.dma_start(
                                    out[rows, :].rearrange(
                                        "(n p) e -> p n e", p=C),
                                    z["st"]["outs"][:])

    nc.compile()
    return nc



def _prepare_in_maps(q, k, q_rot, k_rot, v, transpose_k=False, merged=False):
    b, h, n, d = q.shape
    e = v.shape[-1]
    nbh = b * h
    qf = np.ascontiguousarray(q.reshape(nbh, n, d).astype(np.float32))
    kf = np.ascontiguousarray(k.reshape(nbh, n, d).astype(np.float32))
    qrf = np.ascontiguousarray(q_rot.reshape(nbh, n, d).astype(np.float32))
    krf = np.ascontiguousarray(k_rot.reshape(nbh, n, d).astype(np.float32))
    vf = np.ascontiguousarray(v.reshape(nbh, n, e).astype(np.float32))
    mask = np.triu(np.ones((C, C), dtype=np.float32))

    in_maps = []
    for i in range(N_CORES):
        sel = [PAIRS_PER_CORE * i + p for p in range(PAIRS_PER_CORE)]
        qT = np.ascontiguousarray(
            np.concatenate([qf[s].T for s in sel], axis=1))
        kT = np.ascontiguousarray(
            np.concatenate([kf[s].T for s in sel], axis=1))
        qrT = np.ascontiguousarray(
            np.concatenate([qrf[s].T for s in sel], axis=1))
        krT = np.ascontiguousarray(
            np.concatenate([krf[s].T for s in sel], axis=1))
        knat = np.ascontiguousarray(np.concatenate([kf[s] for s in sel], axis=0))
        krnat = np.ascontiguousarray(np.concatenate([krf[s] for s in sel], axis=0))
        vcat = np.concatenate([vf[s] for s in sel], axis=0)
        v1 = np.ascontiguousarray(
            np.concatenate([vcat, np.ones((vcat.shape[0], 1), np.float32)],
                           axis=1))
        m = dict(qT=qT, kT=kT, qrT=qrT, krT=krT, v1=v1)
        if merged:
            m["mask2"] = np.ascontiguousarray(np.concatenate([mask, mask], axis=1))
        else:
            m["mask"] = mask
        if transpose_k:
            m["ident"] = np.eye(C, dtype=np.float32)
        else:
            m["kn"] = knat
            m["krn"] = krnat
        in_maps.append(m)
    return in_maps


def kernel_f32(q, k, q_rot, k_rot, v, horizon=128, **run_kwargs):
    q = np.asarray(q)
    k = np.asarray(k)
    q_rot = np.asarray(q_rot)
    k_rot = np.asarray(k_rot)
    v = np.asarray(v)
    b, h, n, d = q.shape
    e = v.shape[-1]
    assert (b * h, n, d, e) == (N_CORES * PAIRS_PER_CORE, N, D, E), \
        "kernel is hardcoded for b*h=16, n=2048, d=128, e=64"

    if "nc" not in _cached:
        _cached["nc"] = build_kernel()
    nc = _cached["nc"]

    in_maps = _prepare_in_maps(q, k, q_rot, k_rot, v)
    res = run_bass_kernel_spmd(nc, in_maps, core_ids=list(range(N_CORES)),
                               **run_kwargs)

    outf = np.empty((b * h, n, e), dtype=np.float32)
    for i in range(N_CORES):
        o = res.results[i]["out"].reshape(PAIRS_PER_CORE, n, e)
        for p in range(PAIRS_PER_CORE):
            outf[PAIRS_PER_CORE * i + p] = o[p]
    if run_kwargs:
        kernel_f32.last_results = res
    return outf.reshape(b, h, n, e)


def kernel(q, k, q_rot, k_rot, v, horizon=128, **run_kwargs):
    return kernel16(q, k, q_rot, k_rot, v, horizon, **run_kwargs)


if __name__ == "__main__":
    rng = np.random.default_rng(0)
    q = rng.random((2, 8, N, D), dtype=np.float32)
    k = rng.random((2, 8, N, D), dtype=np.float32)
    qr = rng.standard_normal((2, 8, N, D), dtype=np.float32)
    kr = rng.standard_normal((2, 8, N, D), dtype=np.float32)
    v = rng.random((2, 8, N, E), dtype=np.float32)
    o = kernel(q, k, qr, kr, v, 128)
    print("ok", o.shape, o.dtype, np.abs(o).mean())



# revision 21
# speedup vs baseline: 2.4183x; 1.5504x over previous
"""Trainium2 Bass kernel for chunked recurrent causal linear attention.

Problem: b=2, h=8, n=2048, d=128, e=64, chunk=128, two branches (plain +
rotary) sharing one denominator.

Math (per (b,h), per chunk c, token t in chunk, with running state
S[d,e], Z[d] per branch):
    AT[s,t]   = k_s . q_t                  (s,t in chunk; masked to s<=t)
    num[t,:]  = sum_s ATm[s,t] v_s + q_t @ S      (both branches summed)
    den[t]    = sum_s ATm[s,t]   + q_t . Z        (both branches summed)
    out[t,:]  = num[t,:] / den[t]
    S += k_chunk^T v_chunk ;  Z += sum_s k_s

Sharding: 16 (b,h) pairs over 8 cores, 2 pairs per core. Host ships
pre-transposed copies of q/k/q_rot/k_rot (so no on-device transposes are
needed) plus natural-layout k/k_rot (stationary operand of the state
update) and v with a ones-column appended (fuses the denominator into
the numerator matmuls).
"""

import contextlib
import sys

_nullctx = contextlib.nullcontext

if "/opt/trn_rl_repo" not in sys.path:
    sys.path.insert(0, "/opt/trn_rl_repo")

import numpy as np

import concourse.bass as bass
import concourse.tile as tile
from concourse import bacc, mybir
from concourse.bass_utils import run_bass_kernel_spmd

F32 = mybir.dt.float32
F32R = mybir.dt.float32r

N_CORES = 8
PAIRS_PER_CORE = 2
N = 2048           # sequence length per (b,h)
D = 128            # qk head dim
E = 64             # v head dim
E1 = E + 1         # v plus ones column
C = 128            # chunk size
NCHUNK = N // C    # 16
SLAB = 4           # chunks per DMA slab
SLAB_BUFS = 6      # slab pool buffers
NROWS = PAIRS_PER_CORE * N  # 4096

_cached = {}


def build_kernel(repeat=1, loop_k=None, dma_only=False, reuse_slab=False,
                 probe_no_at=False, probe_no_state=False, transpose_k=False,
                 pipe=1, host_norm=False, dma_split=False, taper=False,
                 big_bufs=False, load_reorder=False, bank_42=False,
                 stagger=False, probe_pe_only=False, mm_f32r=False,
                 f32r=False, fast_start=False, ilv=True):
    nc = bacc.Bacc("TRN2", target_bir_lowering=False, debug=False,
                   num_devices=N_CORES)

    MT = F32  # typed-f32r rejected by walrus codegen (odd-N ISA check)

    def mm(out_ap, lhsT_ap, rhs_ap, **kw):
        if mm_f32r:
            lhsT_ap = lhsT_ap.bitcast(F32R)
            rhs_ap = rhs_ap.bitcast(F32R)
        return nc.tensor.matmul(out_ap, lhsT_ap, rhs_ap, **kw)

    qT = nc.dram_tensor("qT", [D, NROWS], MT, kind="ExternalInput").ap()
    kT = nc.dram_tensor("kT", [D, NROWS], MT, kind="ExternalInput").ap()
    qrT = nc.dram_tensor("qrT", [D, NROWS], MT, kind="ExternalInput").ap()
    krT = nc.dram_tensor("krT", [D, NROWS], MT, kind="ExternalInput").ap()
    if not transpose_k:
        kn = nc.dram_tensor("kn", [NROWS, D], MT, kind="ExternalInput").ap()
        krn = nc.dram_tensor("krn", [NROWS, D], MT, kind="ExternalInput").ap()
    else:
        ident = nc.dram_tensor("ident", [C, C], F32, kind="ExternalInput").ap()
    v1 = nc.dram_tensor("v1", [NROWS, E1], MT, kind="ExternalInput").ap()
    mask = nc.dram_tensor("mask", [C, C], F32, kind="ExternalInput").ap()
    EO = E1 if host_norm else E
    out = nc.dram_tensor("out", [NROWS, EO], F32, kind="ExternalOutput").ap()

    if taper:
        plans = [{0: 2, 2: 4, 6: 4, 10: 4, 14: 2}] * PAIRS_PER_CORE
    elif stagger:
        plans = [{c0: SLAB for c0 in range(0, NCHUNK, SLAB)},
                 {0: 2, 2: 4, 6: 4, 10: 4, 14: 2}]
    else:
        plans = [{c0: SLAB for c0 in range(0, NCHUNK, SLAB)}] * PAIRS_PER_CORE
    slab_of = []
    for pp in range(PAIRS_PER_CORE):
        m = {}
        for c0, ln in plans[pp].items():
            for c in range(c0, c0 + ln):
                m[c] = (c0, ln)
        slab_of.append(m)

    with tile.TileContext(nc) as tc:
        with (
            tc.tile_pool(name="const", bufs=1) as constp,
            tc.tile_pool(name="slabs", bufs=SLAB_BUFS) as slabs,
            tc.tile_pool(name="atm", bufs=(6 if big_bufs else (4 if pipe == 1 else 6))) as atmp,
            tc.tile_pool(name="ssb", bufs=(12 if big_bufs else 8)) as ssbp,
            tc.tile_pool(name="dinv", bufs=(12 if big_bufs else 8)) as dinvp,
            tc.tile_pool(name="pat", bufs=(2 if (transpose_k or bank_42) else 3),
                         space="PSUM") as patp,
            tc.tile_pool(name="pout", bufs=(2 if transpose_k else (4 if bank_42 else 3)),
                         space="PSUM") as poutp,
            tc.tile_pool(name="pst", bufs=2, space="PSUM") as pstp,
            tc.tile_pool(name="ktr", bufs=2, space="PSUM") as ktrp,
            tc.tile_pool(name="kns", bufs=4) as knsp,
        ):
            mask_t = constp.tile([C, C], F32, tag="mask")
            nc.sync.dma_start(mask_t[:], mask[:])
            if transpose_k:
                ident_t = constp.tile([C, C], F32, tag="ident")
                nc.sync.dma_start(ident_t[:], ident[:])

            for rep in range(repeat):
              with (tc.For_i(0, loop_k, 1, hint_engines=(
                        mybir.EngineType.PE, mybir.EngineType.DVE,
                        mybir.EngineType.Activation, mybir.EngineType.SP))
                    if (loop_k is not None and loop_k > 1)
                    else _nullctx()):
                  # per-pair state accumulator in one PSUM bank:
                  # cols 0:65 -> branch 0 [S|Z], cols 66:131 -> branch 1
                  pS = {}
                  for p in range(PAIRS_PER_CORE):
                      pS[p] = pstp.tile([D, 2 * E1 + 2], F32, tag="pS", name=f"pS_{rep}_{p}")

                  slab_t = [None] * PAIRS_PER_CORE   # per pair: dict of slab tiles
                  S_sbuf = {}                        # (p, br) -> sbuf state tile

                  # Software pipeline, one chunk deep: the "front" stage of
                  # chunk c emits loads, the state update (PE), and AT+mask
                  # (PE then DVE); the "back" stage consumes chunk c-1's
                  # masked AT for the numerator/denominator matmuls. This
                  # gives every cross-engine hop a full stage of slack, so
                  # the PE never head-of-line blocks on DVE/ACT latency.
                  fifo = []
                  for cc in range(NCHUNK + pipe):
                    pending = {}
                    back = {}
                    if cc >= pipe:
                        back = fifo.pop(0)
                    if cc < NCHUNK:
                        fifo.append(pending)
                    if cc < NCHUNK:
                      c = cc
                      for p in range(PAIRS_PER_CORE):
                          c0, slen = slab_of[p][c]
                          if (c == c0) and not (reuse_slab and c > 0):
                              base = p * N + c * C
                              cols = slice(base, base + slen * C)
                              dmae = nc.gpsimd if (dma_split and p == 1) else nc.sync
                              st = {"len": slen}
                              if not load_reorder:
                                  st["qT"] = slabs.tile([D, slen * C], MT, tag="qT", name=f"qTs_{rep}_{p}_{c}")
                                  st["kT"] = slabs.tile([D, slen * C], MT, tag="kT", name=f"kTs_{rep}_{p}_{c}")
                                  if fast_start and c == 0:
                                      # split the very first q/k loads so chunk
                                      # 0's AT matmul starts after 128KB, not
                                      # a full slab (range-level tile deps)
                                      dmae.dma_start(st["qT"][:, 0:C], qT[:, base:base + C])
                                      dmae.dma_start(st["kT"][:, 0:C], kT[:, base:base + C])
                                      dmae.dma_start(st["qT"][:, C:slen * C], qT[:, base + C:base + slen * C])
                                      dmae.dma_start(st["kT"][:, C:slen * C], kT[:, base + C:base + slen * C])
                                  else:
                                      dmae.dma_start(st["qT"][:], qT[:, cols])
                                      dmae.dma_start(st["kT"][:], kT[:, cols])
                                  st["qrT"] = slabs.tile([D, slen * C], MT, tag="qrT", name=f"qrTs_{rep}_{p}_{c}")
                                  dmae.dma_start(st["qrT"][:], qrT[:, cols])
                                  st["krT"] = slabs.tile([D, slen * C], MT, tag="krT", name=f"krTs_{rep}_{p}_{c}")
                                  dmae.dma_start(st["krT"][:], krT[:, cols])
                              # load the state-update inputs (kn/krn/v1)
                              # first: they feed the first PE ops of the chunk
                              if not transpose_k:
                                  st["kn"] = slabs.tile([C, slen, D], MT, tag="kn", name=f"kns_{rep}_{p}_{c}")
                                  dmae.dma_start(
                                      st["kn"][:],
                                      kn[cols, :].rearrange("(n p) d -> p n d", p=C))
                                  st["krn"] = slabs.tile([C, slen, D], MT, tag="krn", name=f"krns_{rep}_{p}_{c}")
                                  dmae.dma_start(
                                      st["krn"][:],
                                      krn[cols, :].rearrange("(n p) d -> p n d", p=C))
                              st["v1"] = slabs.tile([C, slen, E1], MT, tag="v1", name=f"v1s_{rep}_{p}_{c}")
                              dmae.dma_start(
                                  st["v1"][:],
                                  v1[cols, :].rearrange("(n p) e -> p n e", p=C))
                              if load_reorder:
                                  st["kT"] = slabs.tile([D, slen * C], MT, tag="kT", name=f"kTs_{rep}_{p}_{c}")
                                  dmae.dma_start(st["kT"][:], kT[:, cols])
                                  st["qT"] = slabs.tile([D, slen * C], MT, tag="qT", name=f"qTs_{rep}_{p}_{c}")
                                  dmae.dma_start(st["qT"][:], qT[:, cols])
                                  st["qrT"] = slabs.tile([D, slen * C], MT, tag="qrT", name=f"qrTs_{rep}_{p}_{c}")
                                  dmae.dma_start(st["qrT"][:], qrT[:, cols])
                                  st["krT"] = slabs.tile([D, slen * C], MT, tag="krT", name=f"krTs_{rep}_{p}_{c}")
                                  dmae.dma_start(st["krT"][:], krT[:, cols])
                              st["outs"] = slabs.tile([C, slen, EO], F32, tag="outs", name=f"outs_{rep}_{p}_{c}")
                              slab_t[p] = st

                          st = slab_t[p]
                          j = c - c0
                          qcT = st["qT"][:, j * C:(j + 1) * C]
                          kcT = st["kT"][:, j * C:(j + 1) * C]
                          qrcT = st["qrT"][:, j * C:(j + 1) * C]
                          krcT = st["krT"][:, j * C:(j + 1) * C]
                          vc = st["v1"][:, j, :]
                          knc = krnc = None
                          if not transpose_k:
                              knc = st["kn"][:, j, :]
                              krnc = st["krn"][:, j, :]

                          if dma_only:
                              continue

                          if probe_pe_only:
                              # pure matmul throughput probe: same 7 MMs as the
                              # real kernel, but no cross-engine deps at all
                              pat0 = patp.tile([C, C], F32, tag="pat")
                              mm(pat0[:], kcT, qcT, start=True, stop=False)
                              mm(pat0[:], krcT, qrcT, start=False, stop=True)
                              po = poutp.tile([C, E1], F32, tag="po")
                              mm(po[:], mask_t[:], vc, start=True, stop=False)
                              mm(po[:], qcT, mask_t[:, 0:E1], start=False, stop=False, skip_group_check=True)
                              mm(po[:], qrcT, mask_t[:, 0:E1], start=False, stop=True, skip_group_check=True)
                              mm(pS[p][:, 0:E1], knc, vc, start=(c == 0), stop=False, skip_group_check=True)
                              mm(pS[p][:, E1 + 1:2 * E1 + 1], krnc, vc, start=False, stop=(c == NCHUNK - 1), skip_group_check=True)
                              continue

                          prev_S = S_sbuf.get(p)

                          if ilv:
                              # MMs emitted pair-interleaved after this loop
                              pending[p] = dict(qcT=qcT, qrcT=qrcT, kcT=kcT,
                                                krcT=krcT, knc=knc, krnc=krnc,
                                                vc=vc, st=st, j=j, c=c,
                                                prev_S=prev_S, kns0=None,
                                                kns1=None, c0=c0,
                                                slen=st.get("len", SLAB))
                              continue

                          # State update: both branches share one PSUM bank
                          # (start=True on c0/br0 clears it; br1 overwrites its
                          # unwritten columns). Without transpose_k the natural-
                          # layout k arrives by DMA and the update is emitted
                          # here (front stage); with transpose_k the k tiles are
                          # transposed on the PE this stage and the state update
                          # moves to the back stage so the transpose->copy->
                          # matmul chain gets a stage of slack.
                          kns0 = kns1 = None
                          if transpose_k and not probe_no_state:
                              ktp0 = ktrp.tile([C, C], F32, tag="ktr")
                              nc.tensor.transpose(ktp0[:], kcT, ident_t[:])
                              kns0 = knsp.tile([C, C], F32, tag="kns")
                              nc.vector.tensor_copy(kns0[:], ktp0[:])
                              ktp1 = ktrp.tile([C, C], F32, tag="ktr")
                              nc.tensor.transpose(ktp1[:], krcT, ident_t[:])
                              kns1 = knsp.tile([C, C], F32, tag="kns")
                              nc.scalar.copy(kns1[:], ktp1[:])
                          if not transpose_k and not probe_no_state:
                              mm(pS[p][:, 0:E1], knc, vc,
                                               start=(c == 0), stop=False,
                                               skip_group_check=True)
                              mm(pS[p][:, E1 + 1:2 * E1 + 1], krnc, vc,
                                               start=False, stop=(c == NCHUNK - 1),
                                               skip_group_check=True)
                              if c < NCHUNK - 1:
                                  s01 = ssbp.tile([D, 2 * E1 + 2], MT, tag="ssb")
                                  nc.scalar.copy(s01[:], pS[p][:])
                                  S_sbuf[p] = s01

                          # AT = K0 Q0^T + K1 Q1^T (both branches accumulate in
                          # one PSUM bank), then one causal mask (s<=t)
                          if probe_no_at:
                              atm0 = mask_t
                          else:
                              pat0 = patp.tile([C, C], F32, tag="pat")
                              if f32r:
                                  mm(pat0[:], kcT.bitcast(F32R),
                                     qcT.bitcast(F32R), start=True, stop=False)
                                  mm(pat0[:], krcT.bitcast(F32R),
                                     qrcT.bitcast(F32R), start=False, stop=True)
                              else:
                                  mm(pat0[:], kcT, qcT, start=True, stop=False)
                                  mm(pat0[:], krcT, qrcT, start=False, stop=True)
                              atm0 = atmp.tile([C, C], MT, tag="atm")
                              nc.vector.tensor_mul(atm0[:], pat0[:], mask_t[:])

                          pending[p] = dict(atm=atm0, qcT=qcT, qrcT=qrcT,
                                            vc=vc, st=st, j=j, c=c,
                                            prev_S=prev_S, kns0=kns0, kns1=kns1,
                                            c0=c0, slen=st.get("len", SLAB))

                    if ilv and cc < NCHUNK and not dma_only and not probe_pe_only:
                        ps = sorted(pending.keys())
                        # state matmuls, pair-interleaved (consecutive MMs hit
                        # different PSUM banks)
                        for br in range(2):
                            for p in ps:
                                z = pending[p]
                                if br == 0:
                                    mm(pS[p][:, 0:E1], z["knc"], z["vc"],
                                       start=(c == 0), stop=False,
                                       skip_group_check=True)
                                else:
                                    mm(pS[p][:, E1 + 1:2 * E1 + 1], z["krnc"],
                                       z["vc"], start=False,
                                       stop=(c == NCHUNK - 1),
                                       skip_group_check=True)
                        for p in ps:
                            if c < NCHUNK - 1:
                                s01 = ssbp.tile([D, 2 * E1 + 2], MT, tag="ssb",
                                                name=f"s01i_{rep}_{p}_{c}")
                                nc.scalar.copy(s01[:], pS[p][:])
                                S_sbuf[p] = s01
                        pats = {}
                        for p in ps:
                            pats[p] = patp.tile([C, C], F32, tag="pat",
                                                name=f"pati_{rep}_{p}_{c}")
                        for br in range(2):
                            for p in ps:
                                z = pending[p]
                                if br == 0:
                                    mm(pats[p][:], z["kcT"], z["qcT"],
                                       start=True, stop=False,
                                       skip_group_check=True)
                                else:
                                    mm(pats[p][:], z["krcT"], z["qrcT"],
                                       start=False, stop=True,
                                       skip_group_check=True)
                        for p in ps:
                            atm = atmp.tile([C, C], MT, tag="atm",
                                            name=f"atmi_{rep}_{p}_{c}")
                            nc.vector.tensor_mul(atm[:], pats[p][:], mask_t[:])
                            pending[p]["atm"] = atm

                    if ilv:
                        items = sorted(back.items())
                        pos = {}
                        for p, z in items:
                            pos[p] = poutp.tile([C, E1], F32, tag="po",
                                                name=f"poi_{rep}_{p}_{z['c']}")
                        for p, z in items:
                            mm(pos[p][:], z["atm"][:], z["vc"], start=True,
                               stop=(z["c"] == 0 or z["prev_S"] is None),
                               skip_group_check=True)
                        for p, z in items:
                            if z["c"] > 0 and z["prev_S"] is not None:
                                mm(pos[p][:], z["qcT"], z["prev_S"][:, 0:E1],
                                   start=False, stop=False,
                                   skip_group_check=True)
                        for p, z in items:
                            if z["c"] > 0 and z["prev_S"] is not None:
                                mm(pos[p][:], z["qrcT"],
                                   z["prev_S"][:, E1 + 1:2 * E1 + 1],
                                   start=False, stop=True,
                                   skip_group_check=True)
                        for p, z in items:
                            po = pos[p]
                            dinv = dinvp.tile([C, 1], F32, tag="dinv",
                                              name=f"dinvi_{rep}_{p}_{z['c']}")
                            nc.vector.reciprocal(dinv[:], po[:, E:E1])
                            nc.scalar.mul(z["st"]["outs"][:, z["j"], :],
                                          po[:, 0:E], dinv[:])
                            if z["j"] == z["slen"] - 1:
                                base = p * N + z["c0"] * C
                                rows = slice(base, base + z["slen"] * C)
                                nc.sync.dma_start(
                                    out[rows, :].rearrange(
                                        "(n p) e -> p n e", p=C),
                                    z["st"]["outs"][:])
                        back = {}

                    for p, z in back.items():
                        cb = z["c"]
                        # with transpose_k the state update happens here, so
                        # the pre-update state must be captured here as well
                        if transpose_k:
                            z["prev_S"] = S_sbuf.get(p)
                        if transpose_k and z["kns0"] is not None:
                            mm(pS[p][:, 0:E1], z["kns0"][:],
                                             z["vc"], start=(cb == 0),
                                             stop=False, skip_group_check=True)
                            mm(pS[p][:, E1 + 1:2 * E1 + 1],
                                             z["kns1"][:], z["vc"],
                                             start=False,
                                             stop=(cb == NCHUNK - 1),
                                             skip_group_check=True)
                            if cb < NCHUNK - 1:
                                s01 = ssbp.tile([D, 2 * E1 + 2], MT, tag="ssb")
                                nc.scalar.copy(s01[:], pS[p][:])
                                S_sbuf[p] = s01
                        # numerator (cols 0..63) + denominator (col 64)
                        po = poutp.tile([C, E1], F32, tag="po")
                        mm(po[:], z["atm"][:], z["vc"],
                                         start=True,
                                         stop=(cb == 0 or z["prev_S"] is None))
                        if cb > 0 and z["prev_S"] is not None:
                            mm(po[:], z["qcT"],
                                             z["prev_S"][:, 0:E1],
                                             start=False, stop=False,
                                             skip_group_check=True)
                            mm(po[:], z["qrcT"],
                                             z["prev_S"][:, E1 + 1:2 * E1 + 1],
                                             start=False, stop=True,
                                             skip_group_check=True)

                        if host_norm:
                            # ship numerator and denominator; host divides
                            nc.scalar.copy(z["st"]["outs"][:, z["j"], :],
                                           po[:, 0:E1])
                        else:
                            # out[t,:] = num[t,:] / den[t]
                            dinv = dinvp.tile([C, 1], F32, tag="dinv")
                            nc.vector.reciprocal(dinv[:], po[:, E:E1])
                            nc.scalar.mul(z["st"]["outs"][:, z["j"], :],
                                          po[:, 0:E], dinv[:])

                        if z["j"] == z["slen"] - 1:
                            base = p * N + z["c0"] * C
                            rows = slice(base, base + z["slen"] * C)
                            nc.sync.dma_start(
                                out[rows, :].rearrange("(n p) e -> p n e", p=C),
                                z["st"]["outs"][:])

    nc.compile()
    return nc




F16 = mybir.dt.float16
F8 = mybir.dt.float8e4
U8 = mybir.dt.uint8


def build_kernel16(repeat=1, loop_k=None, dma_only=False, probe_pe_only=False,
                   slab=SLAB, slab_bufs=SLAB_BUFS):
    """fp16 variant. All inputs ship as fp16; natural-layout tensors
    (kn/krn/v1) and the output use a chunk-major [C, nchunk*f] DRAM layout so
    every DMA descriptor is a contiguous >=512B run. fp16 matmuls run at 1
    cycle/row on the PE (vs 4 for fp32), accumulation stays f32 in PSUM.
    Host un-permutes the output and upcasts to f32."""
    nc = bacc.Bacc("TRN2", target_bir_lowering=False, debug=False,
                   num_devices=N_CORES)

    NPC = PAIRS_PER_CORE
    qT = nc.dram_tensor("qT", [D, NROWS], F16, kind="ExternalInput").ap()
    kT = nc.dram_tensor("kT", [D, NROWS], F16, kind="ExternalInput").ap()
    qrT = nc.dram_tensor("qrT", [D, NROWS], F16, kind="ExternalInput").ap()
    krT = nc.dram_tensor("krT", [D, NROWS], F16, kind="ExternalInput").ap()
    kn = nc.dram_tensor("kn", [C, NPC * NCHUNK * D], F16, kind="ExternalInput").ap()
    krn = nc.dram_tensor("krn", [C, NPC * NCHUNK * D], F16, kind="ExternalInput").ap()
    v1 = nc.dram_tensor("v1", [C, NPC * NCHUNK * E1], F16, kind="ExternalInput").ap()
    mask = nc.dram_tensor("mask", [C, C], F32, kind="ExternalInput").ap()
    out = nc.dram_tensor("out", [C, NPC * NCHUNK * E], F16, kind="ExternalOutput").ap()

    plans = [{c0: slab for c0 in range(0, NCHUNK, slab)}] * NPC
    slab_of = []
    for pp in range(NPC):
        m = {}
        for c0, ln in plans[pp].items():
            for c in range(c0, c0 + ln):
                m[c] = (c0, ln)
        slab_of.append(m)

    with tile.TileContext(nc) as tc:
        with (
            tc.tile_pool(name="const", bufs=1) as constp,
            tc.tile_pool(name="slabs", bufs=slab_bufs) as slabs,
            tc.tile_pool(name="atm", bufs=4) as atmp,
            tc.tile_pool(name="ssb", bufs=8) as ssbp,
            tc.tile_pool(name="dinv", bufs=8) as dinvp,
            tc.tile_pool(name="pat", bufs=3, space="PSUM") as patp,
            tc.tile_pool(name="pout", bufs=3, space="PSUM") as poutp,
            tc.tile_pool(name="pst", bufs=2, space="PSUM") as pstp,
        ):
            mask_t = constp.tile([C, C], F32, tag="mask")
            nc.sync.dma_start(mask_t[:], mask[:])

            for rep in range(repeat):
              with (tc.For_i(0, loop_k, 1, hint_engines=(
                        mybir.EngineType.PE, mybir.EngineType.DVE,
                        mybir.EngineType.Activation, mybir.EngineType.SP))
                    if (loop_k is not None and loop_k > 1)
                    else _nullctx()):
                  pS = {}
                  for p in range(NPC):
                      pS[p] = pstp.tile([D, 2 * E1 + 2], F32, tag="pS",
                                        name=f"pS16_{rep}_{p}")

                  slab_t = [None] * NPC
                  S_sbuf = {}

                  fifo = []
                  for cc in range(NCHUNK + 1):
                    pending = {}
                    back = {}
                    if cc >= 1:
                        back = fifo.pop(0)
                    if cc < NCHUNK:
                        fifo.append(pending)
                    if cc < NCHUNK:
                      c = cc
                      for p in range(NPC):
                          c0, slen = slab_of[p][c]
                          if c == c0:
                              base = p * N + c * C
                              cols = slice(base, base + slen * C)
                              ncols = slice((p * NCHUNK + c) * D,
                                            (p * NCHUNK + c + slen) * D)
                              vcols = slice((p * NCHUNK + c) * E1,
                                            (p * NCHUNK + c + slen) * E1)
                              st = {"len": slen}
                              st["qT"] = slabs.tile([D, slen * C], F16, tag="qT", name=f"qTs16_{rep}_{p}_{c}")
                              nc.sync.dma_start(st["qT"][:], qT[:, cols])
                              st["kT"] = slabs.tile([D, slen * C], F16, tag="kT", name=f"kTs16_{rep}_{p}_{c}")
                              nc.sync.dma_start(st["kT"][:], kT[:, cols])
                              st["qrT"] = slabs.tile([D, slen * C], F16, tag="qrT", name=f"qrTs16_{rep}_{p}_{c}")
                              nc.sync.dma_start(st["qrT"][:], qrT[:, cols])
                              st["krT"] = slabs.tile([D, slen * C], F16, tag="krT", name=f"krTs16_{rep}_{p}_{c}")
                              nc.sync.dma_start(st["krT"][:], krT[:, cols])
                              st["kn"] = slabs.tile([C, slen * D], F16, tag="kn", name=f"kns16_{rep}_{p}_{c}")
                              nc.sync.dma_start(st["kn"][:], kn[:, ncols])
                              st["krn"] = slabs.tile([C, slen * D], F16, tag="krn", name=f"krns16_{rep}_{p}_{c}")
                              nc.sync.dma_start(st["krn"][:], krn[:, ncols])
                              st["v1"] = slabs.tile([C, slen * E1], F16, tag="v1", name=f"v1s16_{rep}_{p}_{c}")
                              nc.sync.dma_start(st["v1"][:], v1[:, vcols])
                              st["outs"] = slabs.tile([C, slen * E], F16, tag="outs", name=f"outs16_{rep}_{p}_{c}")
                              slab_t[p] = st

                          st = slab_t[p]
                          j = c - c0
                          if dma_only:
                              continue
                          z = dict(
                              qcT=st["qT"][:, j * C:(j + 1) * C],
                              kcT=st["kT"][:, j * C:(j + 1) * C],
                              qrcT=st["qrT"][:, j * C:(j + 1) * C],
                              krcT=st["krT"][:, j * C:(j + 1) * C],
                              knc=st["kn"][:, j * D:(j + 1) * D],
                              krnc=st["krn"][:, j * D:(j + 1) * D],
                              vc=st["v1"][:, j * E1:(j + 1) * E1],
                              st=st, j=j, c=c, c0=c0, slen=slen,
                              prev_S=S_sbuf.get(p))
                          pending[p] = z

                      if probe_pe_only and pending:
                          for p, z in sorted(pending.items()):
                              pat0 = patp.tile([C, C], F32, tag="pat")
                              nc.tensor.matmul(pat0[:], z["kcT"], z["qcT"], start=True, stop=False)
                              nc.tensor.matmul(pat0[:], z["krcT"], z["qrcT"], start=False, stop=True)
                              po = poutp.tile([C, E1], F32, tag="po")
                              nc.tensor.matmul(po[:], z["qcT"], mask_t[:, 0:E1].bitcast(F16)[:, 0:E1], start=True, stop=False, skip_group_check=True)
                              nc.tensor.matmul(po[:], z["qrcT"], mask_t[:, 0:E1].bitcast(F16)[:, 0:E1], start=False, stop=False, skip_group_check=True)
                              nc.tensor.matmul(po[:], z["kcT"], mask_t[:, 0:E1].bitcast(F16)[:, 0:E1], start=False, stop=True, skip_group_check=True)
                              nc.tensor.matmul(pS[p][:, 0:E1], z["knc"], z["vc"], start=(z["c"] == 0), stop=False, skip_group_check=True)
                              nc.tensor.matmul(pS[p][:, E1 + 1:2 * E1 + 1], z["krnc"], z["vc"], start=False, stop=(z["c"] == NCHUNK - 1), skip_group_check=True)
                          continue

                      if pending and not dma_only:
                        ps = sorted(pending.keys())
                        for br in range(2):
                            for p in ps:
                                z = pending[p]
                                if br == 0:
                                    nc.tensor.matmul(pS[p][:, 0:E1], z["knc"],
                                                     z["vc"], start=(c == 0),
                                                     stop=False,
                                                     skip_group_check=True)
                                else:
                                    nc.tensor.matmul(pS[p][:, E1 + 1:2 * E1 + 1],
                                                     z["krnc"], z["vc"],
                                                     start=False,
                                                     stop=(c == NCHUNK - 1),
                                                     skip_group_check=True)
                        for p in ps:
                            if c < NCHUNK - 1:
                                s01 = ssbp.tile([D, 2 * E1 + 2], F16, tag="ssb",
                                                name=f"s01h_{rep}_{p}_{c}")
                                nc.scalar.copy(s01[:], pS[p][:])
                                S_sbuf[p] = s01
                        pats = {}
                        for p in ps:
                            pats[p] = patp.tile([C, C], F32, tag="pat",
                                                name=f"path_{rep}_{p}_{c}")
                        for br in range(2):
                            for p in ps:
                                z = pending[p]
                                if br == 0:
                                    nc.tensor.matmul(pats[p][:], z["kcT"],
                                                     z["qcT"], start=True,
                                                     stop=False,
                                                     skip_group_check=True)
                                else:
                                    nc.tensor.matmul(pats[p][:], z["krcT"],
                                                     z["qrcT"], start=False,
                                                     stop=True,
                                                     skip_group_check=True)
                        for p in ps:
                            atm = atmp.tile([C, C], F16, tag="atm",
                                            name=f"atmh_{rep}_{p}_{c}")
                            nc.vector.tensor_mul(atm[:], pats[p][:], mask_t[:])
                            pending[p]["atm"] = atm

                    if back and not dma_only and not probe_pe_only:
                        items = sorted(back.items())
                        pos = {}
                        for p, z in items:
                            pos[p] = poutp.tile([C, E1], F32, tag="po",
                                                name=f"poh_{rep}_{p}_{z['c']}")
                        for p, z in items:
                            nc.tensor.matmul(pos[p][:], z["atm"][:], z["vc"],
                                             start=True,
                                             stop=(z["c"] == 0 or z["prev_S"] is None),
                                             skip_group_check=True)
                        for p, z in items:
                            if z["c"] > 0 and z["prev_S"] is not None:
                                nc.tensor.matmul(pos[p][:], z["qcT"],
                                                 z["prev_S"][:, 0:E1],
                                                 start=False, stop=False,
                                                 skip_group_check=True)
                        for p, z in items:
                            if z["c"] > 0 and z["prev_S"] is not None:
                                nc.tensor.matmul(pos[p][:], z["qrcT"],
                                                 z["prev_S"][:, E1 + 1:2 * E1 + 1],
                                                 start=False, stop=True,
                                                 skip_group_check=True)
                        for p, z in items:
                            po = pos[p]
                            dinv = dinvp.tile([C, 1], F32, tag="dinv",
                                              name=f"dinvh_{rep}_{p}_{z['c']}")
                            nc.vector.reciprocal(dinv[:], po[:, E:E1])
                            nc.scalar.mul(
                                z["st"]["outs"][:, z["j"] * E:(z["j"] + 1) * E],
                                po[:, 0:E], dinv[:])
                            if z["j"] == z["slen"] - 1:
                                ocols = slice((p * NCHUNK + z["c0"]) * E,
                                              (p * NCHUNK + z["c0"] + z["slen"]) * E)
                                nc.sync.dma_start(out[:, ocols],
                                                  z["st"]["outs"][:])

    nc.compile()
    return nc


def build_kernel16b(repeat=1, loop_k=None, dma_only=False, probe_pe_pure=False,
                    slab=8, slab_bufs=3):
    """fp16 + packed-DMA variant: per (pair, slab) ONE load of the merged
    transposed block [qT|kT|qrT|krT], ONE load of the merged natural block
    [kn|krn|v1], ONE store of the output block. At slab=8 that is 12 DMA
    instructions per iteration (vs 57 in v1), sidestepping the ~625ns/DMA
    HWDGE descriptor-generation serialization that dominated the v1 floor.

    probe_pe_pure: run the full per-chunk matmul bundle on tiles loaded once
    outside the loop — a clean PE-only floor with no DMA dependencies."""
    nc = bacc.Bacc("TRN2", target_bir_lowering=False, debug=False,
                   num_devices=N_CORES)

    NPC = PAIRS_PER_CORE
    NSLAB = NCHUNK // slab
    SC = slab * C
    KVW = slab * (2 * D + E1)       # merged natural-block width per slab
    qk = nc.dram_tensor("qk", [D, NPC * NSLAB * 4 * SC], F16,
                        kind="ExternalInput").ap()
    kv = nc.dram_tensor("kv", [C, NPC * NSLAB * KVW], F16,
                        kind="ExternalInput").ap()
    mask = nc.dram_tensor("mask", [C, C], F32, kind="ExternalInput").ap()
    out = nc.dram_tensor("out", [C, NPC * NCHUNK * E], F16,
                         kind="ExternalOutput").ap()

    with tile.TileContext(nc) as tc:
        with (
            tc.tile_pool(name="const", bufs=1) as constp,
            tc.tile_pool(name="slabs", bufs=slab_bufs) as slabs,
            tc.tile_pool(name="atm", bufs=4) as atmp,
            tc.tile_pool(name="ssb", bufs=8) as ssbp,
            tc.tile_pool(name="dinv", bufs=8) as dinvp,
            tc.tile_pool(name="pat", bufs=3, space="PSUM") as patp,
            tc.tile_pool(name="pout", bufs=3, space="PSUM") as poutp,
            tc.tile_pool(name="pst", bufs=2, space="PSUM") as pstp,
        ):
            mask_t = constp.tile([C, C], F32, tag="mask")
            nc.sync.dma_start(mask_t[:], mask[:])

            pure = {}
            if probe_pe_pure:
                # one fixed tile set, loaded once; the loop's MMs reference it
                pure["qk"] = constp.tile([D, 4 * SC], F16, tag="pqk", name="pqk")
                nc.sync.dma_start(pure["qk"][:], qk[:, 0:4 * SC])
                pure["kv"] = constp.tile([C, KVW], F16, tag="pkv", name="pkv")
                nc.sync.dma_start(pure["kv"][:], kv[:, 0:KVW])
                pure["atm"] = constp.tile([C, C], F16, tag="patm", name="patm")
                nc.vector.tensor_copy(pure["atm"][:], mask_t[:])
                pure["s01"] = constp.tile([D, 2 * E1 + 2], F16, tag="ps01", name="ps01")
                nc.vector.tensor_copy(pure["s01"][:], pure["kv"][:, 0:2 * E1 + 2])

            for rep in range(repeat):
              with (tc.For_i(0, loop_k, 1, hint_engines=(
                        mybir.EngineType.PE, mybir.EngineType.DVE,
                        mybir.EngineType.Activation, mybir.EngineType.SP))
                    if (loop_k is not None and loop_k > 1)
                    else _nullctx()):
                  if probe_pe_pure:
                      # 7-MM bundle x NCHUNK x NPC on fixed tiles
                      pqk, pkv = pure["qk"], pure["kv"]
                      for c in range(NCHUNK):
                        for p in range(NPC):
                          j = c % slab
                          qcT = pqk[:, 0 * SC + j * C:0 * SC + (j + 1) * C]
                          kcT = pqk[:, 1 * SC + j * C:1 * SC + (j + 1) * C]
                          qrcT = pqk[:, 2 * SC + j * C:2 * SC + (j + 1) * C]
                          krcT = pqk[:, 3 * SC + j * C:3 * SC + (j + 1) * C]
                          knc = pkv[:, j * D:(j + 1) * D]
                          krnc = pkv[:, slab * D + j * D:slab * D + (j + 1) * D]
                          vc = pkv[:, 2 * slab * D + j * E1:2 * slab * D + (j + 1) * E1]
                          pS = pstp.tile([D, 2 * E1 + 2], F32, tag="pS")
                          nc.tensor.matmul(pS[:, 0:E1], knc, vc, start=True, stop=False, skip_group_check=True)
                          nc.tensor.matmul(pS[:, E1 + 1:2 * E1 + 1], krnc, vc, start=False, stop=True, skip_group_check=True)
                          pat0 = patp.tile([C, C], F32, tag="pat")
                          nc.tensor.matmul(pat0[:], kcT, qcT, start=True, stop=False)
                          nc.tensor.matmul(pat0[:], krcT, qrcT, start=False, stop=True)
                          po = poutp.tile([C, E1], F32, tag="po")
                          nc.tensor.matmul(po[:], pure["atm"][:, 0:C], vc, start=True, stop=False, skip_group_check=True)
                          nc.tensor.matmul(po[:], qcT, pure["s01"][:, 0:E1], start=False, stop=False, skip_group_check=True)
                          nc.tensor.matmul(po[:], qrcT, pure["s01"][:, E1 + 1:2 * E1 + 1], start=False, stop=True, skip_group_check=True)
                      continue

                  pS = {}
                  for p in range(NPC):
                      pS[p] = pstp.tile([D, 2 * E1 + 2], F32, tag="pS",
                                        name=f"pSb_{rep}_{p}")

                  slab_t = [None] * NPC
                  S_sbuf = {}

                  fifo = []
                  for cc in range(NCHUNK + 1):
                    pending = {}
                    back = {}
                    if cc >= 1:
                        back = fifo.pop(0)
                    if cc < NCHUNK:
                        fifo.append(pending)
                    if cc < NCHUNK:
                      c = cc
                      for p in range(NPC):
                          c0 = (c // slab) * slab
                          si = c // slab
                          if c == c0:
                              qbase = (p * NSLAB + si) * 4 * SC
                              kbase = (p * NSLAB + si) * KVW
                              st = {}
                              st["kv"] = slabs.tile([C, KVW], F16, tag="kv",
                                                    name=f"kvs_{rep}_{p}_{c}")
                              nc.sync.dma_start(st["kv"][:],
                                                kv[:, kbase:kbase + KVW])
                              st["qk"] = slabs.tile([D, 4 * SC], F16, tag="qk",
                                                    name=f"qks_{rep}_{p}_{c}")
                              nc.sync.dma_start(st["qk"][:],
                                                qk[:, qbase:qbase + 4 * SC])
                              st["outs"] = slabs.tile([C, slab * E], F16,
                                                      tag="outs",
                                                      name=f"outsb_{rep}_{p}_{c}")
                              slab_t[p] = st

                          st = slab_t[p]
                          j = c - c0
                          if dma_only:
                              continue
                          z = dict(
                              qcT=st["qk"][:, 0 * SC + j * C:0 * SC + (j + 1) * C],
                              kcT=st["qk"][:, 1 * SC + j * C:1 * SC + (j + 1) * C],
                              qrcT=st["qk"][:, 2 * SC + j * C:2 * SC + (j + 1) * C],
                              krcT=st["qk"][:, 3 * SC + j * C:3 * SC + (j + 1) * C],
                              knc=st["kv"][:, j * D:(j + 1) * D],
                              krnc=st["kv"][:, slab * D + j * D:slab * D + (j + 1) * D],
                              vc=st["kv"][:, 2 * slab * D + j * E1:2 * slab * D + (j + 1) * E1],
                              st=st, j=j, c=c, c0=c0, slen=slab,
                              prev_S=S_sbuf.get(p))
                          pending[p] = z

                      if pending and not dma_only:
                        ps = sorted(pending.keys())
                        for br in range(2):
                            for p in ps:
                                z = pending[p]
                                if br == 0:
                                    nc.tensor.matmul(pS[p][:, 0:E1], z["knc"],
                                                     z["vc"], start=(c == 0),
                                                     stop=False,
                                                     skip_group_check=True)
                                else:
                                    nc.tensor.matmul(pS[p][:, E1 + 1:2 * E1 + 1],
                                                     z["krnc"], z["vc"],
                                                     start=False,
                                                     stop=(c == NCHUNK - 1),
                                                     skip_group_check=True)
                        for p in ps:
                            if c < NCHUNK - 1:
                                s01 = ssbp.tile([D, 2 * E1 + 2], F16, tag="ssb",
                                                name=f"s01b_{rep}_{p}_{c}")
                                nc.scalar.copy(s01[:], pS[p][:])
                                S_sbuf[p] = s01
                        pats = {}
                        for p in ps:
                            pats[p] = patp.tile([C, C], F32, tag="pat",
                                                name=f"patb_{rep}_{p}_{c}")
                        for br in range(2):
                            for p in ps:
                                z = pending[p]
                                if br == 0:
                                    nc.tensor.matmul(pats[p][:], z["kcT"],
                                                     z["qcT"], start=True,
                                                     stop=False,
                                                     skip_group_check=True)
                                else:
                                    nc.tensor.matmul(pats[p][:], z["krcT"],
                                                     z["qrcT"], start=False,
                                                     stop=True,
                                                     skip_group_check=True)
                        for p in ps:
                            atm = atmp.tile([C, C], F16, tag="atm",
                                            name=f"atmb_{rep}_{p}_{c}")
                            nc.vector.tensor_mul(atm[:], pats[p][:], mask_t[:])
                            pending[p]["atm"] = atm

                    if back and not dma_only:
                        items = sorted(back.items())
                        pos = {}
                        for p, z in items:
                            pos[p] = poutp.tile([C, E1], F32, tag="po",
                                                name=f"pob_{rep}_{p}_{z['c']}")
                        for p, z in items:
                            nc.tensor.matmul(pos[p][:], z["atm"][:], z["vc"],
                                             start=True,
                                             stop=(z["c"] == 0 or z["prev_S"] is None),
                                             skip_group_check=True)
                        for p, z in items:
                            if z["c"] > 0 and z["prev_S"] is not None:
                                nc.tensor.matmul(pos[p][:], z["qcT"],
                                                 z["prev_S"][:, 0:E1],
                                                 start=False, stop=False,
                                                 skip_group_check=True)
                        for p, z in items:
                            if z["c"] > 0 and z["prev_S"] is not None:
                                nc.tensor.matmul(pos[p][:], z["qrcT"],
                                                 z["prev_S"][:, E1 + 1:2 * E1 + 1],
                                                 start=False, stop=True,
                                                 skip_group_check=True)
                        for p, z in items:
                            po = pos[p]
                            dinv = dinvp.tile([C, 1], F32, tag="dinv",
                                              name=f"dinvb_{rep}_{p}_{z['c']}")
                            nc.vector.reciprocal(dinv[:], po[:, E:E1])
                            nc.scalar.mul(
                                z["st"]["outs"][:, z["j"] * E:(z["j"] + 1) * E],
                                po[:, 0:E], dinv[:])
                            if z["j"] == z["slen"] - 1:
                                ocols = slice((p * NCHUNK + z["c0"]) * E,
                                              (p * NCHUNK + z["c0"] + z["slen"]) * E)
                                nc.sync.dma_start(out[:, ocols],
                                                  z["st"]["outs"][:])

    nc.compile()
    return nc


def build_kernel16c(repeat=1, loop_k=None, dma_only=False, slab=8,
                    slab_bufs=3, recip_dev=False):
    """v3: fp16 + packed DMA (as 16b) + pair-merged PSUM banks.

    Both (b,h) pairs handled by a core share single PSUM banks for AT, for
    num|den, and for the scan state, so the causal mask, the state
    evacuation, and the num/den evacuation each run as ONE wide DVE/ACT op
    per chunk instead of one per pair. The division happens on the host
    (kernel ships num and den); no reciprocal / scale ops on device.
    """
    nc = bacc.Bacc("TRN2", target_bir_lowering=False, debug=False,
                   num_devices=N_CORES)

    NPC = PAIRS_PER_CORE
    NSLAB = NCHUNK // slab
    SC = slab * C
    KVW = slab * (2 * D + E1)
    OW = 2 * E1                      # per-chunk output cols (both pairs)
    qk = nc.dram_tensor("qk", [D, NPC * NSLAB * 4 * SC], F16,
                        kind="ExternalInput").ap()
    kv = nc.dram_tensor("kv", [C, NPC * NSLAB * KVW], F16,
                        kind="ExternalInput").ap()
    mask2 = nc.dram_tensor("mask2", [C, 2 * C], F32, kind="ExternalInput").ap()
    out = nc.dram_tensor("out", [C, NCHUNK * OW], F16,
                         kind="ExternalOutput").ap()

    with tile.TileContext(nc) as tc:
        with (
            tc.tile_pool(name="const", bufs=1) as constp,
            tc.tile_pool(name="slabs", bufs=slab_bufs) as slabs,
            tc.tile_pool(name="atm", bufs=4) as atmp,
            tc.tile_pool(name="ssb", bufs=4) as ssbp,
            tc.tile_pool(name="pat", bufs=3, space="PSUM") as patp,
            tc.tile_pool(name="pout", bufs=3, space="PSUM") as poutp,
            tc.tile_pool(name="pst", bufs=2, space="PSUM") as pstp,
        ):
            mask_t = constp.tile([C, 2 * C], F32, tag="mask")
            nc.sync.dma_start(mask_t[:], mask2[:])

            fixed = None
            if probe_nodma:
                # one 8-chunk slab-set per pair, loaded once; the loop reuses
                # it for both halves -> compute rhythm with no load DMAs
                fixed = []
                fl = 8
                for p in range(NPC):
                    fqk = constp.tile([D, 4 * fl * C], F16, tag=f"fqk{p}",
                                      name=f"fqk{p}")
                    nc.sync.dma_start(fqk[:], qk[:, 0:4 * fl * C])
                    fkv = constp.tile([C, fl * (2 * D + E1)], F16,
                                      tag=f"fkv{p}", name=f"fkv{p}")
                    nc.sync.dma_start(fkv[:], kv[:, 0:fl * (2 * D + E1)])
                    fixed.append({"qk": fqk, "kv": fkv, "ln": fl})

            for rep in range(repeat):
              with (tc.For_i(0, loop_k, 1, hint_engines=(
                        mybir.EngineType.PE, mybir.EngineType.DVE,
                        mybir.EngineType.Activation, mybir.EngineType.SP))
                    if (loop_k is not None and loop_k > 1)
                    else _nullctx()):
                  pSb = pstp.tile([D, 2 * NPC * SW], F32, tag="pS",
                                  name=f"pSc_{rep}")

                  slab_t = [None] * NPC
                  outs_t = [None]
                  S_sbuf = [None]

                  pend = None
                  for cc in range(NCHUNK + 1):
                    back = pend
                    pend = None
                    if cc < NCHUNK:
                      c = cc
                      si = c // slab
                      c0 = si * slab
                      j = c - c0
                      sl = {}
                      for p in range(NPC):
                          if c == c0:
                              qbase = (p * NSLAB + si) * 4 * SC
                              kbase = (p * NSLAB + si) * KVW
                              st = {}
                              st["kv"] = slabs.tile([C, KVW], F16, tag="kv",
                                                    name=f"kvc_{rep}_{p}_{c}")
                              nc.sync.dma_start(st["kv"][:],
                                                kv[:, kbase:kbase + KVW])
                              st["qk"] = slabs.tile([D, 4 * SC], F16, tag="qk",
                                                    name=f"qkc_{rep}_{p}_{c}")
                              nc.sync.dma_start(st["qk"][:],
                                                qk[:, qbase:qbase + 4 * SC])
                              slab_t[p] = st
                          st = slab_t[p]
                          sl[p] = dict(
                              qcT=st["qk"][:, 0 * SC + j * C:0 * SC + (j + 1) * C],
                              kcT=st["qk"][:, 1 * SC + j * C:1 * SC + (j + 1) * C],
                              qrcT=st["qk"][:, 2 * SC + j * C:2 * SC + (j + 1) * C],
                              krcT=st["qk"][:, 3 * SC + j * C:3 * SC + (j + 1) * C],
                              knc=st["kv"][:, j * D:(j + 1) * D],
                              krnc=st["kv"][:, slab * D + j * D:slab * D + (j + 1) * D],
                              vc=st["kv"][:, 2 * slab * D + j * E1:2 * slab * D + (j + 1) * E1])
                      if c == c0:
                          outs_t[0] = slabs.tile([C, slab * OW], F16, tag="outs",
                                                 name=f"outsc_{rep}_{c}")
                      if dma_only:
                          if c - c0 == slab - 1:
                              ocols = slice(c0 * OW, (c0 + slab) * OW)
                              nc.sync.dma_start(out[:, ocols], outs_t[0][:])
                          continue

                      prev_S = S_sbuf[0]

                      # state updates: 4 regions of one PSUM bank
                      for br in range(2):
                          for p in range(NPC):
                              z = sl[p]
                              reg = pSb[:, (2 * p + br) * SW:(2 * p + br) * SW + E1]
                              nc.tensor.matmul(
                                  reg, z["knc"] if br == 0 else z["krnc"],
                                  z["vc"],
                                  start=(c == 0 and br == 0 and p == 0),
                                  stop=(c == NCHUNK - 1 and br == 1 and p == NPC - 1),
                                  skip_group_check=True)
                      if c < NCHUNK - 1:
                          s01 = ssbp.tile([D, 2 * NPC * SW], F16, tag="ssb",
                                          name=f"s01c_{rep}_{c}")
                          nc.scalar.copy(s01[:], pSb[:])
                          S_sbuf[0] = s01

                      # AT for both pairs into one bank, one mask op
                      patb = patp.tile([C, 2 * C], F32, tag="pat",
                                       name=f"patc_{rep}_{c}")
                      for br in range(2):
                          for p in range(NPC):
                              z = sl[p]
                              reg = patb[:, p * C:(p + 1) * C]
                              nc.tensor.matmul(
                                  reg, z["kcT"] if br == 0 else z["krcT"],
                                  z["qcT"] if br == 0 else z["qrcT"],
                                  start=(br == 0 and p == 0),
                                  stop=(br == 1 and p == NPC - 1),
                                  skip_group_check=True)
                      atm = atmp.tile([C, 2 * C], F16, tag="atm",
                                      name=f"atmc_{rep}_{c}")
                      nc.vector.tensor_mul(atm[:], patb[:], mask_t[:])

                      pend = dict(atm=atm, sl=sl, c=c, c0=c0, j=j,
                                  prev_S=prev_S, outs=outs_t[0])

                    if back is not None and not dma_only:
                        cb = back["c"]
                        pob = poutp.tile([C, NPC * PW], F32, tag="po",
                                         name=f"poc_{rep}_{cb}")
                        for p in range(NPC):
                            z = back["sl"][p]
                            reg = pob[:, p * PW:p * PW + E1]
                            nc.tensor.matmul(
                                reg, back["atm"][:, p * C:(p + 1) * C],
                                z["vc"], start=(p == 0),
                                stop=(cb == 0 and p == NPC - 1),
                                skip_group_check=True)
                        if cb > 0:
                            pv = back["prev_S"]
                            for br in range(2):
                                for p in range(NPC):
                                    z = back["sl"][p]
                                    reg = pob[:, p * PW:p * PW + E1]
                                    nc.tensor.matmul(
                                        reg,
                                        z["qcT"] if br == 0 else z["qrcT"],
                                        pv[:, (2 * p + br) * SW:(2 * p + br) * SW + E1],
                                        start=False,
                                        stop=(br == 1 and p == NPC - 1),
                                        skip_group_check=True)

                        # evacuate num|den for both pairs in one op
                        src = pob[:].rearrange("p (g w) -> p g w", g=NPC)[:, :, 0:E1]
                        dst = back["outs"][:, back["j"] * OW:(back["j"] + 1) * OW]
                        dst = dst.rearrange("p (g w) -> p g w", g=NPC)
                        if cb % 2 == 0:
                            nc.vector.tensor_copy(dst, src)
                        else:
                            nc.scalar.copy(dst, src)
                        if back["j"] == slab - 1:
                            ocols = slice(back["c0"] * OW,
                                          (back["c0"] + slab) * OW)
                            nc.sync.dma_start(out[:, ocols], back["outs"][:])

    nc.compile()
    return nc


def _plan_slabs(plan):
    """plan: list of slab lengths summing to NCHUNK -> per-chunk lookup."""
    slabs, c0 = [], 0
    for ln in plan:
        slabs.append((c0, ln))
        c0 += ln
    assert c0 == NCHUNK
    of = {}
    for si, (c0, ln) in enumerate(slabs):
        for c in range(c0, c0 + ln):
            of[c] = (si, c0, ln)
    return slabs, of


def build_kernel16d(repeat=1, loop_k=None, dma_only=False, slab=8,
                    slab_bufs=3, pipe=2, store_q="gpsimd", plan=None,
                    probe_nodma=False, stag=False, add_eng="vector",
                    kn8=False):
    """v4: 16c + latency-chain fixes.

    - The scan state lives as an SBUF fp16 running sum: each chunk's outer
      product goes to a FRESH PSUM bank (no in-place PSUM accumulation), and
      ACT folds it into the running state (s01 += pS). This breaks the
      PE->ACT->PE anti-dependency cycle that serialized v3 (~1us/chunk).
    - Back stage (num/den matmuls + evacuation) trails by `pipe` chunks so
      every cross-engine hop has >= 1 full stage of slack.
    - Output stores go on the Pool (SWDGE) queue so the SP load queue never
      head-of-line blocks on end-of-iteration stores.
    """
    nc = bacc.Bacc("TRN2", target_bir_lowering=False, debug=False,
                   num_devices=N_CORES)

    NPC = PAIRS_PER_CORE
    if plan is None:
        plan = [slab] * (NCHUNK // slab)
    slabs_l, slab_of = _plan_slabs(plan)
    # per-(pair, slab) base offsets in the packed DRAM tensors
    QKW_of = [4 * ln * C for _, ln in slabs_l]
    # kv block: kn|krn (fp8 bytes when kn8) + v1 (fp16); widths in ELEMENTS
    # of the kv dram dtype (uint8 when kn8, fp16 otherwise)
    KD = D if kn8 else D          # kn col width per chunk in dram elements
    KVW_of = [ln * (2 * D + 2 * E1) if kn8 else ln * (2 * D + E1)
              for _, ln in slabs_l]
    KVT = U8 if kn8 else F16
    qk_tot = sum(QKW_of)
    kv_tot = sum(KVW_of)
    qk_base = [[p * qk_tot + sum(QKW_of[:si]) for si in range(len(slabs_l))]
               for p in range(NPC)]
    kv_base = [[p * kv_tot + sum(KVW_of[:si]) for si in range(len(slabs_l))]
               for p in range(NPC)]
    OW = 2 * E1
    qk = nc.dram_tensor("qk", [D, NPC * qk_tot], F16,
                        kind="ExternalInput").ap()
    kv = nc.dram_tensor("kv", [C, NPC * kv_tot], KVT,
                        kind="ExternalInput").ap()
    mask2 = nc.dram_tensor("mask2", [C, 2 * C], F32, kind="ExternalInput").ap()
    out = nc.dram_tensor("out", [C, NCHUNK * OW], F16,
                         kind="ExternalOutput").ap()

    store_eng = {"gpsimd": nc.gpsimd, "sync": nc.sync}[store_q]

    with tile.TileContext(nc) as tc:
        with (
            tc.tile_pool(name="const", bufs=1) as constp,
            tc.tile_pool(name="slabs", bufs=slab_bufs) as slabs,
            tc.tile_pool(name="atm", bufs=pipe + 2) as atmp,
            tc.tile_pool(name="ssb", bufs=pipe + 3) as ssbp,
            tc.tile_pool(name="pat", bufs=3, space="PSUM") as patp,
            tc.tile_pool(name="pout", bufs=3, space="PSUM") as poutp,
            tc.tile_pool(name="pst", bufs=2, space="PSUM") as pstp,
        ):
            mask_t = constp.tile([C, 2 * C], F32, tag="mask")
            nc.sync.dma_start(mask_t[:], mask2[:])

            fixed = None
            if probe_nodma:
                # one 8-chunk slab-set per pair, loaded once; the loop reuses
                # it for both halves -> compute rhythm with no load DMAs
                fixed = []
                fl = 8
                for p in range(NPC):
                    fqk = constp.tile([D, 4 * fl * C], F16, tag=f"fqk{p}",
                                      name=f"fqk{p}")
                    nc.sync.dma_start(fqk[:], qk[:, 0:4 * fl * C])
                    fkv = constp.tile([C, fl * (2 * D + E1)], F16,
                                      tag=f"fkv{p}", name=f"fkv{p}")
                    nc.sync.dma_start(fkv[:], kv[:, 0:fl * (2 * D + E1)])
                    fixed.append({"qk": fqk, "kv": fkv, "ln": fl})

            for rep in range(repeat):
              with (tc.For_i(0, loop_k, 1, staggered_reset=stag,
                             hint_engines=(
                        mybir.EngineType.PE, mybir.EngineType.DVE,
                        mybir.EngineType.Activation, mybir.EngineType.SP,
                        mybir.EngineType.Pool))
                    if (loop_k is not None and loop_k > 1)
                    else _nullctx()):
                  slab_t = [None] * NPC
                  outs_t = [None]
                  S_sbuf = [None]

                  fifo = []
                  for cc in range(NCHUNK + pipe):
                    back = None
                    if cc >= pipe and fifo:
                        back = fifo.pop(0)
                    if cc < NCHUNK:
                      c = cc
                      si, c0, ln = slab_of[c]
                      j = c - c0
                      sl = {}
                      for p in range(NPC):
                          if probe_nodma:
                              slab_t[p] = fixed[p]
                              c0, ln = (0 if c < 8 else 8), 8
                              j = c - c0
                          elif c == c0:
                              qbase = qk_base[p][si]
                              kbase = kv_base[p][si]
                              st = {"ln": ln}
                              st["kv"] = slabs.tile([C, KVW_of[si]], KVT, tag="kv",
                                                    name=f"kvd_{rep}_{p}_{c}")
                              nc.sync.dma_start(st["kv"][:],
                                                kv[:, kbase:kbase + KVW_of[si]])
                              st["qk"] = slabs.tile([D, QKW_of[si]], F16, tag="qk",
                                                    name=f"qkd_{rep}_{p}_{c}")
                              nc.sync.dma_start(st["qk"][:],
                                                qk[:, qbase:qbase + QKW_of[si]])
                              slab_t[p] = st
                          st = slab_t[p]
                          SC = st["ln"] * C
                          lnp = st["ln"]
                          if kn8:
                              knc = st["kv"][:, j * D:(j + 1) * D].bitcast(F8)
                              krnc = st["kv"][:, lnp * D + j * D:lnp * D + (j + 1) * D].bitcast(F8)
                              vc = st["kv"][:, 2 * lnp * D + j * 2 * E1:2 * lnp * D + (j + 1) * 2 * E1].bitcast(F16)
                          else:
                              knc = st["kv"][:, j * D:(j + 1) * D]
                              krnc = st["kv"][:, lnp * D + j * D:lnp * D + (j + 1) * D]
                              vc = st["kv"][:, 2 * lnp * D + j * E1:2 * lnp * D + (j + 1) * E1]
                          sl[p] = dict(
                              qcT=st["qk"][:, 0 * SC + j * C:0 * SC + (j + 1) * C],
                              kcT=st["qk"][:, 1 * SC + j * C:1 * SC + (j + 1) * C],
                              qrcT=st["qk"][:, 2 * SC + j * C:2 * SC + (j + 1) * C],
                              krcT=st["qk"][:, 3 * SC + j * C:3 * SC + (j + 1) * C],
                              knc=knc, krnc=krnc, vc=vc)
                      if probe_nodma:
                          ln = 8
                      if c == c0:
                          outs_t[0] = slabs.tile([C, ln * OW], F16, tag="outs",
                                                 name=f"outsd_{rep}_{c}")
                      if dma_only:
                          if j == ln - 1:
                              ocols = slice(c0 * OW, (c0 + ln) * OW)
                              store_eng.dma_start(
                                  out[:, ocols],
                                  slab_t[0]["kv"][:, 0:2 * ln * OW].bitcast(F16)
                                  if kn8 else slab_t[0]["kv"][:, 0:ln * OW])
                          continue

                      prev_S = S_sbuf[0]

                      # fresh per-chunk outer product (one PSUM group)
                      pSc = pstp.tile([D, 2 * NPC * SW], F32, tag="pS",
                                      name=f"pSd_{rep}_{c}")
                      for br in range(2):
                          for p in range(NPC):
                              z = sl[p]
                              reg = pSc[:, (2 * p + br) * SW:(2 * p + br) * SW + E1]
                              nc.tensor.matmul(
                                  reg, z["knc"] if br == 0 else z["krnc"],
                                  z["vc"],
                                  start=(br == 0 and p == 0),
                                  stop=(br == 1 and p == NPC - 1),
                                  skip_group_check=True)
                      if c < NCHUNK - 1:
                          s01 = ssbp.tile([D, 2 * NPC * SW], F16, tag="ssb",
                                          name=f"s01d_{rep}_{c}")
                          addeng = getattr(nc, add_eng)
                          if prev_S is None:
                              addeng.tensor_copy(s01[:], pSc[:])
                          else:
                              addeng.tensor_add(s01[:], pSc[:], prev_S[:])
                          S_sbuf[0] = s01

                      patb = patp.tile([C, 2 * C], F32, tag="pat",
                                       name=f"patd_{rep}_{c}")
                      for br in range(2):
                          for p in range(NPC):
                              z = sl[p]
                              reg = patb[:, p * C:(p + 1) * C]
                              nc.tensor.matmul(
                                  reg, z["kcT"] if br == 0 else z["krcT"],
                                  z["qcT"] if br == 0 else z["qrcT"],
                                  start=(br == 0 and p == 0),
                                  stop=(br == 1 and p == NPC - 1),
                                  skip_group_check=True)
                      atm = atmp.tile([C, 2 * C], F16, tag="atm",
                                      name=f"atmd_{rep}_{c}")
                      nc.vector.tensor_mul(atm[:], patb[:], mask_t[:])

                      fifo.append(dict(atm=atm, sl=sl, c=c, c0=c0, j=j,
                                       ln=ln, prev_S=prev_S, outs=outs_t[0]))

                    if back is not None and not dma_only:
                        cb = back["c"]
                        pob = poutp.tile([C, NPC * PW], F32, tag="po",
                                         name=f"pod_{rep}_{cb}")
                        for p in range(NPC):
                            z = back["sl"][p]
                            reg = pob[:, p * PW:p * PW + E1]
                            nc.tensor.matmul(
                                reg, back["atm"][:, p * C:(p + 1) * C],
                                z["vc"], start=(p == 0),
                                stop=(cb == 0 and p == NPC - 1),
                                skip_group_check=True)
                        if cb > 0:
                            pv = back["prev_S"]
                            for br in range(2):
                                for p in range(NPC):
                                    z = back["sl"][p]
                                    reg = pob[:, p * PW:p * PW + E1]
                                    nc.tensor.matmul(
                                        reg,
                                        z["qcT"] if br == 0 else z["qrcT"],
                                        pv[:, (2 * p + br) * SW:(2 * p + br) * SW + E1],
                                        start=False,
                                        stop=(br == 1 and p == NPC - 1),
                                        skip_group_check=True)

                        src = pob[:].rearrange("p (g w) -> p g w", g=NPC)[:, :, 0:E1]
                        dst = back["outs"][:, back["j"] * OW:(back["j"] + 1) * OW]
                        dst = dst.rearrange("p (g w) -> p g w", g=NPC)
                        nc.scalar.copy(dst, src)
                        if back["j"] == back["ln"] - 1:
                            ocols = slice(back["c0"] * OW,
                                          (back["c0"] + back["ln"]) * OW)
                            store_eng.dma_start(out[:, ocols], back["outs"][:])

    nc.compile()
    return nc


# Final tuned configuration (see session notes): fp16 everywhere, fp8-e4m3
# for the state-path k/k_rot, packed per-(pair,slab) DMA blocks with a
# tapered-tail slab plan, SBUF-accumulated scan state, 3-deep back stage,
# stores on the Pool/SWDGE queue.
BEST = dict(plan=[4, 4, 4, 2, 2], slab_bufs=10, pipe=3, kn8=True)


def bench_build(loop_k=None, **over):
    kw = dict(BEST)
    kw.update(over)
    return build_kernel16d(loop_k=loop_k, **kw)


def bench_in_maps(q, k, q_rot, k_rot, v):
    in_maps = _prepare_in_maps16b(q, k, q_rot, k_rot, v, vscale=1.0 / 16.0,
                                  plan=BEST["plan"], kn8=BEST["kn8"])
    for m in in_maps:
        msk = m.pop("mask")
        m["mask2"] = np.ascontiguousarray(np.concatenate([msk, msk], axis=1))
    return in_maps


def kernel16d(q, k, q_rot, k_rot, v, horizon=128, slab=8, **run_kwargs):
    q = np.asarray(q)
    k = np.asarray(k)
    q_rot = np.asarray(q_rot)
    k_rot = np.asarray(k_rot)
    v = np.asarray(v)
    b, h, n, d = q.shape
    e = v.shape[-1]
    assert (b * h, n, d, e) == (N_CORES * PAIRS_PER_CORE, N, D, E)

    if "nc16d" not in _cached:
        _cached["nc16d"] = bench_build()
    nc = _cached["nc16d"]

    in_maps = bench_in_maps(q, k, q_rot, k_rot, v)
    res = run_bass_kernel_spmd(nc, in_maps, core_ids=list(range(N_CORES)),
                               **run_kwargs)

    outf = np.empty((b * h, n, e), dtype=np.float32)
    for i in range(N_CORES):
        o = res.results[i]["out"]
        o = o.reshape(C, NCHUNK, PAIRS_PER_CORE, E1).astype(np.float32)
        for p in range(PAIRS_PER_CORE):
            num = o[:, :, p, 0:E].transpose(1, 0, 2).reshape(n, E)
            den = o[:, :, p, E].transpose(1, 0).reshape(n, 1)
            outf[PAIRS_PER_CORE * i + p] = num / den
    if run_kwargs:
        kernel16d.last_results = res
    return outf.reshape(b, h, n, e)


def kernel16c(q, k, q_rot, k_rot, v, horizon=128, slab=8, **run_kwargs):
    q = np.asarray(q)
    k = np.asarray(k)
    q_rot = np.asarray(q_rot)
    k_rot = np.asarray(k_rot)
    v = np.asarray(v)
    b, h, n, d = q.shape
    e = v.shape[-1]
    assert (b * h, n, d, e) == (N_CORES * PAIRS_PER_CORE, N, D, E)

    key = f"nc16c_{slab}"
    if key not in _cached:
        _cached[key] = build_kernel16c(slab=slab)
    nc = _cached[key]

    # v (and the fused ones column) are pre-scaled by 1/16 so the shipped
    # fp16 num|den never overflow (den reaches ~65536 unscaled); the host
    # division num/den cancels the scale exactly.
    in_maps = _prepare_in_maps16b(q, k, q_rot, k_rot, v, slab=slab,
                                  vscale=1.0 / 16.0)
    for m in in_maps:
        msk = m.pop("mask")
        m["mask2"] = np.ascontiguousarray(np.concatenate([msk, msk], axis=1))
    res = run_bass_kernel_spmd(nc, in_maps, core_ids=list(range(N_CORES)),
                               **run_kwargs)

    outf = np.empty((b * h, n, e), dtype=np.float32)
    for i in range(N_CORES):
        o = res.results[i]["out"]  # [C, NCHUNK * 2 * E1] fp16
        o = o.reshape(C, NCHUNK, PAIRS_PER_CORE, E1).astype(np.float32)
        for p in range(PAIRS_PER_CORE):
            num = o[:, :, p, 0:E].transpose(1, 0, 2).reshape(n, E)
            den = o[:, :, p, E].transpose(1, 0).reshape(n, 1)
            outf[PAIRS_PER_CORE * i + p] = num / den
    if run_kwargs:
        kernel16c.last_results = res
    return outf.reshape(b, h, n, e)


def _prepare_in_maps16b(q, k, q_rot, k_rot, v, slab=8, vscale=1.0, plan=None,
                        kn8=False):
    b, h, n, d = q.shape
    e = v.shape[-1]
    nbh = b * h
    if plan is None:
        plan = [slab] * (NCHUNK // slab)
    qf = q.reshape(nbh, n, d)
    kf = k.reshape(nbh, n, d)
    qrf = q_rot.reshape(nbh, n, d)
    krf = k_rot.reshape(nbh, n, d)
    vf = v.reshape(nbh, n, e)
    mask = np.triu(np.ones((C, C), dtype=np.float32))

    def chunk_major(x, nch):
        f = x.shape[-1]
        return x.reshape(nch, C, f).transpose(1, 0, 2).reshape(C, nch * f)

    in_maps = []
    for i in range(N_CORES):
        sel = [PAIRS_PER_CORE * i + p for p in range(PAIRS_PER_CORE)]
        qkblks, kvblks = [], []
        for s in sel:
            qT, kT, qrT, krT = (x[s].T.astype(np.float16)
                                for x in (qf, kf, qrf, krf))
            v1s = (vscale * np.concatenate(
                [vf[s], np.ones((n, 1), vf.dtype)], axis=1)).astype(np.float16)
            kns = kf[s].astype(np.float16)
            krns = krf[s].astype(np.float16)
            c0 = 0
            for ln in plan:
                cs = slice(c0 * C, (c0 + ln) * C)
                c0 += ln
                qkblks.append(np.concatenate(
                    [qT[:, cs], kT[:, cs], qrT[:, cs], krT[:, cs]], axis=1))
                if kn8:
                    import ml_dtypes
                    kn_b = chunk_major(kns[cs], ln).astype(
                        ml_dtypes.float8_e4m3fn).view(np.uint8)
                    krn_b = chunk_major(krns[cs], ln).astype(
                        ml_dtypes.float8_e4m3fn).view(np.uint8)
                    v1_b = chunk_major(v1s[cs], ln).view(np.uint8)
                    kvblks.append(np.concatenate([kn_b, krn_b, v1_b], axis=1))
                else:
                    kvblks.append(np.concatenate(
                        [chunk_major(kns[cs], ln), chunk_major(krns[cs], ln),
                         chunk_major(v1s[cs], ln)], axis=1))
        in_maps.append(dict(
            qk=np.ascontiguousarray(np.concatenate(qkblks, axis=1)),
            kv=np.ascontiguousarray(np.concatenate(kvblks, axis=1)),
            mask=mask))
    return in_maps


def kernel16b(q, k, q_rot, k_rot, v, horizon=128, slab=8, **run_kwargs):
    q = np.asarray(q)
    k = np.asarray(k)
    q_rot = np.asarray(q_rot)
    k_rot = np.asarray(k_rot)
    v = np.asarray(v)
    b, h, n, d = q.shape
    e = v.shape[-1]
    assert (b * h, n, d, e) == (N_CORES * PAIRS_PER_CORE, N, D, E)

    key = f"nc16b_{slab}"
    if key not in _cached:
        _cached[key] = build_kernel16b(slab=slab)
    nc = _cached[key]

    in_maps = _prepare_in_maps16b(q, k, q_rot, k_rot, v, slab=slab)
    res = run_bass_kernel_spmd(nc, in_maps, core_ids=list(range(N_CORES)),
                               **run_kwargs)

    outf = np.empty((b * h, n, e), dtype=np.float32)
    for i in range(N_CORES):
        o = res.results[i]["out"]  # [C, PAIRS*NCHUNK*E] fp16
        o = o.reshape(C, PAIRS_PER_CORE, NCHUNK, E).astype(np.float32)
        for p in range(PAIRS_PER_CORE):
            outf[PAIRS_PER_CORE * i + p] = o[:, p].transpose(1, 0, 2).reshape(n, e)
    if run_kwargs:
        kernel16b.last_results = res
    return outf.reshape(b, h, n, e)


def _prepare_in_maps16(q, k, q_rot, k_rot, v):
    b, h, n, d = q.shape
    e = v.shape[-1]
    nbh = b * h
    qf = q.reshape(nbh, n, d)
    kf = k.reshape(nbh, n, d)
    qrf = q_rot.reshape(nbh, n, d)
    krf = k_rot.reshape(nbh, n, d)
    vf = v.reshape(nbh, n, e)
    mask = np.triu(np.ones((C, C), dtype=np.float32))

    def chunk_major(x):
        # [n, f] -> [C, NCHUNK * f]: column-major-by-chunk on-chip layout
        f = x.shape[-1]
        return x.reshape(NCHUNK, C, f).transpose(1, 0, 2).reshape(C, NCHUNK * f)

    in_maps = []
    for i in range(N_CORES):
        sel = [PAIRS_PER_CORE * i + p for p in range(PAIRS_PER_CORE)]
        qT = np.concatenate([qf[s].T for s in sel], axis=1).astype(np.float16)
        kT = np.concatenate([kf[s].T for s in sel], axis=1).astype(np.float16)
        qrT = np.concatenate([qrf[s].T for s in sel], axis=1).astype(np.float16)
        krT = np.concatenate([krf[s].T for s in sel], axis=1).astype(np.float16)
        kn = np.concatenate([chunk_major(kf[s]) for s in sel], axis=1).astype(np.float16)
        krn = np.concatenate([chunk_major(krf[s]) for s in sel], axis=1).astype(np.float16)
        v1 = np.concatenate(
            [chunk_major(np.concatenate(
                [vf[s], np.ones((n, 1), vf.dtype)], axis=1)) for s in sel],
            axis=1).astype(np.float16)
        in_maps.append(dict(qT=np.ascontiguousarray(qT),
                            kT=np.ascontiguousarray(kT),
                            qrT=np.ascontiguousarray(qrT),
                            krT=np.ascontiguousarray(krT),
                            kn=np.ascontiguousarray(kn),
                            krn=np.ascontiguousarray(krn),
                            v1=np.ascontiguousarray(v1),
                            mask=mask))
    return in_maps


def kernel16(q, k, q_rot, k_rot, v, horizon=128, **run_kwargs):
    q = np.asarray(q)
    k = np.asarray(k)
    q_rot = np.asarray(q_rot)
    k_rot = np.asarray(k_rot)
    v = np.asarray(v)
    b, h, n, d = q.shape
    e = v.shape[-1]
    assert (b * h, n, d, e) == (N_CORES * PAIRS_PER_CORE, N, D, E)

    if "nc16" not in _cached:
        _cached["nc16"] = build_kernel16()
    nc = _cached["nc16"]

    in_maps = _prepare_in_maps16(q, k, q_rot, k_rot, v)
    res = run_bass_kernel_spmd(nc, in_maps, core_ids=list(range(N_CORES)),
                               **run_kwargs)

    outf = np.empty((b * h, n, e), dtype=np.float32)
    for i in range(N_CORES):
        o = res.results[i]["out"]  # [C, PAIRS*NCHUNK*E] fp16
        o = o.reshape(C, PAIRS_PER_CORE, NCHUNK, E).astype(np.float32)
        for p in range(PAIRS_PER_CORE):
            outf[PAIRS_PER_CORE * i + p] = o[:, p].transpose(1, 0, 2).reshape(n, e)
    if run_kwargs:
        kernel16.last_results = res
    return outf.reshape(b, h, n, e)


# Column strides inside shared PSUM banks (8-byte aligned regions)
PW = 72            # per-pair region width in the output bank (>= E1)
SW = 66            # per-(pair,branch) region width in the state bank (>= E1)


def build_kernel_m(repeat=1, loop_k=None):
    """Pair-merged variant: both (b,h) pairs handled per core share single
    PSUM banks for AT, numerator/denominator, and state, so the causal mask,
    the state evacuation, and the reciprocal each run as ONE wide
    vector/scalar op per chunk instead of one per pair. Cuts the DVE/ACT
    instruction count (and their fixed per-op drain cost) roughly in half."""
    nc = bacc.Bacc("TRN2", target_bir_lowering=False, debug=False,
                   num_devices=N_CORES)

    MT = F32  # typed-f32r rejected by walrus codegen (odd-N ISA check)

    def mm(out_ap, lhsT_ap, rhs_ap, **kw):
        if mm_f32r:
            lhsT_ap = lhsT_ap.bitcast(F32R)
            rhs_ap = rhs_ap.bitcast(F32R)
        return nc.tensor.matmul(out_ap, lhsT_ap, rhs_ap, **kw)

    qT = nc.dram_tensor("qT", [D, NROWS], MT, kind="ExternalInput").ap()
    kT = nc.dram_tensor("kT", [D, NROWS], MT, kind="ExternalInput").ap()
    qrT = nc.dram_tensor("qrT", [D, NROWS], MT, kind="ExternalInput").ap()
    krT = nc.dram_tensor("krT", [D, NROWS], MT, kind="ExternalInput").ap()
    kn = nc.dram_tensor("kn", [NROWS, D], MT, kind="ExternalInput").ap()
    krn = nc.dram_tensor("krn", [NROWS, D], MT, kind="ExternalInput").ap()
    v1 = nc.dram_tensor("v1", [NROWS, E1], MT, kind="ExternalInput").ap()
    mask2 = nc.dram_tensor("mask2", [C, 2 * C], F32, kind="ExternalInput").ap()
    out = nc.dram_tensor("out", [NROWS, E], F32, kind="ExternalOutput").ap()

    NP = PAIRS_PER_CORE  # 2

    with tile.TileContext(nc) as tc:
        with (
            tc.tile_pool(name="const", bufs=1) as constp,
            tc.tile_pool(name="slabs", bufs=6) as slabs,
            tc.tile_pool(name="atm", bufs=3) as atmp,
            tc.tile_pool(name="ssb", bufs=4) as ssbp,
            tc.tile_pool(name="dinv", bufs=8) as dinvp,
            tc.tile_pool(name="pat", bufs=3, space="PSUM") as patp,
            tc.tile_pool(name="pout", bufs=3, space="PSUM") as poutp,
            tc.tile_pool(name="pst", bufs=1, space="PSUM") as pstp,
        ):
            mask_t = constp.tile([C, 2 * C], F32, tag="mask")
            nc.sync.dma_start(mask_t[:], mask2[:])

            fixed = None
            if probe_nodma:
                # one 8-chunk slab-set per pair, loaded once; the loop reuses
                # it for both halves -> compute rhythm with no load DMAs
                fixed = []
                fl = 8
                for p in range(NPC):
                    fqk = constp.tile([D, 4 * fl * C], F16, tag=f"fqk{p}",
                                      name=f"fqk{p}")
                    nc.sync.dma_start(fqk[:], qk[:, 0:4 * fl * C])
                    fkv = constp.tile([C, fl * (2 * D + E1)], F16,
                                      tag=f"fkv{p}", name=f"fkv{p}")
                    nc.sync.dma_start(fkv[:], kv[:, 0:fl * (2 * D + E1)])
                    fixed.append({"qk": fqk, "kv": fkv, "ln": fl})

            for rep in range(repeat):
              with (tc.For_i(0, loop_k, 1, hint_engines=(
                        mybir.EngineType.PE, mybir.EngineType.DVE,
                        mybir.EngineType.Activation, mybir.EngineType.SP))
                    if (loop_k is not None and loop_k > 1)
                    else _nullctx()):
                  # one state bank: region (p, br) at cols (2p+br)*SW
                  pSt = pstp.tile([D, 2 * NP * SW], F32, tag="pS",
                                  name=f"pSm_{rep}")

                  slab_t = [None] * NP
                  S_sbuf = [None]     # boxed: current [D, 4*SW] sbuf state

                  pending = None
                  for cc in range(NCHUNK + 1):
                    back = pending
                    pending = None
                    if cc < NCHUNK:
                      c = cc
                      sl = {}
                      for p in range(NP):
                          if c % SLAB == 0:
                              base = p * N + c * C
                              cols = slice(base, base + SLAB * C)
                              st = {}
                              st["qT"] = slabs.tile([D, slen * C], F32, tag="qT", name=f"qTs_{rep}_{p}_{c}")
                              nc.sync.dma_start(st["qT"][:], qT[:, cols])
                              st["kT"] = slabs.tile([D, slen * C], F32, tag="kT", name=f"kTs_{rep}_{p}_{c}")
                              nc.sync.dma_start(st["kT"][:], kT[:, cols])
                              st["qrT"] = slabs.tile([D, slen * C], F32, tag="qrT", name=f"qrTs_{rep}_{p}_{c}")
                              nc.sync.dma_start(st["qrT"][:], qrT[:, cols])
                              st["krT"] = slabs.tile([D, slen * C], F32, tag="krT", name=f"krTs_{rep}_{p}_{c}")
                              nc.sync.dma_start(st["krT"][:], krT[:, cols])
                              st["kn"] = slabs.tile([C, slen, D], F32, tag="kn", name=f"kns_{rep}_{p}_{c}")
                              nc.sync.dma_start(
                                  st["kn"][:],
                                  kn[cols, :].rearrange("(n p) d -> p n d", p=C))
                              st["krn"] = slabs.tile([C, slen, D], F32, tag="krn", name=f"krns_{rep}_{p}_{c}")
                              nc.sync.dma_start(
                                  st["krn"][:],
                                  krn[cols, :].rearrange("(n p) d -> p n d", p=C))
                              st["v1"] = slabs.tile([C, slen, E1], F32, tag="v1", name=f"v1s_{rep}_{p}_{c}")
                              nc.sync.dma_start(
                                  st["v1"][:],
                                  v1[cols, :].rearrange("(n p) e -> p n e", p=C))
                              st["outs"] = slabs.tile([C, SLAB, E], F32, tag="outs", name=f"outs_{rep}_{p}_{c}")
                              slab_t[p] = st

                          st = slab_t[p]
                          j = c - c0
                          sl[p] = dict(
                              st=st, j=j,
                              qcT=st["qT"][:, j * C:(j + 1) * C],
                              kcT=st["kT"][:, j * C:(j + 1) * C],
                              qrcT=st["qrT"][:, j * C:(j + 1) * C],
                              krcT=st["krT"][:, j * C:(j + 1) * C],
                              knc=st["kn"][:, j, :],
                              krnc=st["krn"][:, j, :],
                              vc=st["v1"][:, j, :],
                          )

                      prev_S = S_sbuf[0]

                      # state updates, all four into one bank
                      for p in range(NP):
                          z = sl[p]
                          nc.tensor.matmul(
                              pSt[:, (2 * p) * SW:(2 * p) * SW + E1],
                              z["knc"], z["vc"],
                              start=(c == 0 and p == 0), stop=False,
                              skip_group_check=True)
                          nc.tensor.matmul(
                              pSt[:, (2 * p + 1) * SW:(2 * p + 1) * SW + E1],
                              z["krnc"], z["vc"],
                              start=False,
                              stop=(c == NCHUNK - 1 and p == NP - 1),
                              skip_group_check=True)
                      if c < NCHUNK - 1:
                          s01 = ssbp.tile([D, 2 * NP * SW], F32, tag="ssb")
                          nc.scalar.copy(s01[:], pSt[:])
                          S_sbuf[0] = s01

                      # AT for both pairs into one bank, one mask op
                      patb = patp.tile([C, 2 * C], F32, tag="pat")
                      for p in range(NP):
                          z = sl[p]
                          reg = patb[:, p * C:(p + 1) * C]
                          nc.tensor.matmul(reg, z["kcT"], z["qcT"],
                                           start=True, stop=False,
                                           skip_group_check=True)
                          nc.tensor.matmul(reg, z["krcT"], z["qrcT"],
                                           start=False, stop=True,
                                           skip_group_check=True)
                      atm = atmp.tile([C, 2 * C], F32, tag="atm")
                      nc.vector.tensor_mul(atm[:], patb[:], mask_t[:])

                      pending = dict(atm=atm, sl=sl, c=c, prev_S=prev_S)

                    if back is not None:
                        cb = back["c"]
                        pob = poutp.tile([C, NP * PW], F32, tag="po")
                        for p in range(NP):
                            z = back["sl"][p]
                            reg = pob[:, p * PW:p * PW + E1]
                            only = (cb == 0)
                            nc.tensor.matmul(
                                reg, back["atm"][:, p * C:(p + 1) * C],
                                z["vc"], start=True, stop=only,
                                skip_group_check=True)
                            if cb > 0:
                                pv = back["prev_S"]
                                nc.tensor.matmul(
                                    reg, z["qcT"],
                                    pv[:, (2 * p) * SW:(2 * p) * SW + E1],
                                    start=False, stop=False,
                                    skip_group_check=True)
                                nc.tensor.matmul(
                                    reg, z["qrcT"],
                                    pv[:, (2 * p + 1) * SW:(2 * p + 1) * SW + E1],
                                    start=False, stop=True,
                                    skip_group_check=True)

                        # one reciprocal for both pairs' denominators
                        dinv = dinvp.tile([C, NP], F32, tag="dinv")
                        nc.vector.reciprocal(
                            dinv[:], pob[:, E:NP * PW:PW])
                        for p in range(NP):
                            z = back["sl"][p]
                            nc.scalar.mul(z["st"]["outs"][:, z["j"], :],
                                          pob[:, p * PW:p * PW + E],
                                          dinv[:, p:p + 1])
                            if z["j"] == SLAB - 1:
                                base = p * N + (cb - SLAB + 1) * C
                                rows = slice(base, base + SLAB * C)
                                nc.sync.dma_start(
                                    out[rows, :].rearrange(
                                        "(n p) e -> p n e", p=C),
                                    z["st"]["outs"][:])

    nc.compile()
    return nc



def _prepare_in_maps(q, k, q_rot, k_rot, v, transpose_k=False, merged=False):
    b, h, n, d = q.shape
    e = v.shape[-1]
    nbh = b * h
    qf = np.ascontiguousarray(q.reshape(nbh, n, d).astype(np.float32))
    kf = np.ascontiguousarray(k.reshape(nbh, n, d).astype(np.float32))
    qrf = np.ascontiguousarray(q_rot.reshape(nbh, n, d).astype(np.float32))
    krf = np.ascontiguousarray(k_rot.reshape(nbh, n, d).astype(np.float32))
    vf = np.ascontiguousarray(v.reshape(nbh, n, e).astype(np.float32))
    mask = np.triu(np.ones((C, C), dtype=np.float32))

    in_maps = []
    for i in range(N_CORES):
        sel = [PAIRS_PER_CORE * i + p for p in range(PAIRS_PER_CORE)]
        qT = np.ascontiguousarray(
            np.concatenate([qf[s].T for s in sel], axis=1))
        kT = np.ascontiguousarray(
            np.concatenate([kf[s].T for s in sel], axis=1))
        qrT = np.ascontiguousarray(
            np.concatenate([qrf[s].T for s in sel], axis=1))
        krT = np.ascontiguousarray(
            np.concatenate([krf[s].T for s in sel], axis=1))
        knat = np.ascontiguousarray(np.concatenate([kf[s] for s in sel], axis=0))
        krnat = np.ascontiguousarray(np.concatenate([krf[s] for s in sel], axis=0))
        vcat = np.concatenate([vf[s] for s in sel], axis=0)
        v1 = np.ascontiguousarray(
            np.concatenate([vcat, np.ones((vcat.shape[0], 1), np.float32)],
                           axis=1))
        m = dict(qT=qT, kT=kT, qrT=qrT, krT=krT, v1=v1)
        if merged:
            m["mask2"] = np.ascontiguousarray(np.concatenate([mask, mask], axis=1))
        else:
            m["mask"] = mask
        if transpose_k:
            m["ident"] = np.eye(C, dtype=np.float32)
        else:
            m["kn"] = knat
            m["krn"] = krnat
        in_maps.append(m)
    return in_maps


def kernel_f32(q, k, q_rot, k_rot, v, horizon=128, **run_kwargs):
    q = np.asarray(q)
    k = np.asarray(k)
    q_rot = np.asarray(q_rot)
    k_rot = np.asarray(k_rot)
    v = np.asarray(v)
    b, h, n, d = q.shape
    e = v.shape[-1]
    assert (b * h, n, d, e) == (N_CORES * PAIRS_PER_CORE, N, D, E), \
        "kernel is hardcoded for b*h=16, n=2048, d=128, e=64"

    if "nc" not in _cached:
        _cached["nc"] = build_kernel()
    nc = _cached["nc"]

    in_maps = _prepare_in_maps(q, k, q_rot, k_rot, v)
    res = run_bass_kernel_spmd(nc, in_maps, core_ids=list(range(N_CORES)),
                               **run_kwargs)

    outf = np.empty((b * h, n, e), dtype=np.float32)
    for i in range(N_CORES):
        o = res.results[i]["out"].reshape(PAIRS_PER_CORE, n, e)
        for p in range(PAIRS_PER_CORE):
            outf[PAIRS_PER_CORE * i + p] = o[p]
    if run_kwargs:
        kernel_f32.last_results = res
    return outf.reshape(b, h, n, e)


def kernel(q, k, q_rot, k_rot, v, horizon=128, **run_kwargs):
    return kernel16d(q, k, q_rot, k_rot, v, horizon, **run_kwargs)


if __name__ == "__main__":
    rng = np.random.default_rng(0)
    q = rng.random((2, 8, N, D), dtype=np.float32)
    k = rng.random((2, 8, N, D), dtype=np.float32)
    qr = rng.standard_normal((2, 8, N, D), dtype=np.float32)
    kr = rng.standard_normal((2, 8, N, D), dtype=np.float32)
    v = rng.random((2, 8, N, E), dtype=np.float32)
    o = kernel(q, k, qr, kr, v, 128)
    print("ok", o.shape, o.dtype, np.abs(o).mean())



# revision 23
# speedup vs baseline: 2.8649x; 1.1847x over previous
"""Trainium2 Bass kernel for chunked recurrent causal linear attention.

Problem: b=2, h=8, n=2048, d=128, e=64, chunk=128, two branches (plain +
rotary) sharing one denominator.

Math (per (b,h), per chunk c, token t in chunk, with running state
S[d,e], Z[d] per branch):
    AT[s,t]   = k_s . q_t                  (s,t in chunk; masked to s<=t)
    num[t,:]  = sum_s ATm[s,t] v_s + q_t @ S      (both branches summed)
    den[t]    = sum_s ATm[s,t]   + q_t . Z        (both branches summed)
    out[t,:]  = num[t,:] / den[t]
    S += k_chunk^T v_chunk ;  Z += sum_s k_s

Sharding: 16 (b,h) pairs over 8 cores, 2 pairs per core. Host ships
pre-transposed copies of q/k/q_rot/k_rot (so no on-device transposes are
needed) plus natural-layout k/k_rot (stationary operand of the state
update) and v with a ones-column appended (fuses the denominator into
the numerator matmuls).
"""

import contextlib
import sys

_nullctx = contextlib.nullcontext

if "/opt/trn_rl_repo" not in sys.path:
    sys.path.insert(0, "/opt/trn_rl_repo")

import numpy as np

import concourse.bass as bass
import concourse.tile as tile
from concourse import bacc, mybir
from concourse.bass_utils import run_bass_kernel_spmd

F32 = mybir.dt.float32
F32R = mybir.dt.float32r

N_CORES = 8
PAIRS_PER_CORE = 2
N = 2048           # sequence length per (b,h)
D = 128            # qk head dim
E = 64             # v head dim
E1 = E + 1         # v plus ones column
C = 128            # chunk size
NCHUNK = N // C    # 16
SLAB = 4           # chunks per DMA slab
SLAB_BUFS = 6      # slab pool buffers
NROWS = PAIRS_PER_CORE * N  # 4096

_cached = {}


def build_kernel(repeat=1, loop_k=None, dma_only=False, reuse_slab=False,
                 probe_no_at=False, probe_no_state=False, transpose_k=False,
                 pipe=1, host_norm=False, dma_split=False, taper=False,
                 big_bufs=False, load_reorder=False, bank_42=False,
                 stagger=False, probe_pe_only=False, mm_f32r=False,
                 f32r=False, fast_start=False, ilv=True):
    nc = bacc.Bacc("TRN2", target_bir_lowering=False, debug=False,
                   num_devices=N_CORES)

    MT = F32  # typed-f32r rejected by walrus codegen (odd-N ISA check)

    def mm(out_ap, lhsT_ap, rhs_ap, **kw):
        if mm_f32r:
            lhsT_ap = lhsT_ap.bitcast(F32R)
            rhs_ap = rhs_ap.bitcast(F32R)
        return nc.tensor.matmul(out_ap, lhsT_ap, rhs_ap, **kw)

    qT = nc.dram_tensor("qT", [D, NROWS], MT, kind="ExternalInput").ap()
    kT = nc.dram_tensor("kT", [D, NROWS], MT, kind="ExternalInput").ap()
    qrT = nc.dram_tensor("qrT", [D, NROWS], MT, kind="ExternalInput").ap()
    krT = nc.dram_tensor("krT", [D, NROWS], MT, kind="ExternalInput").ap()
    if not transpose_k:
        kn = nc.dram_tensor("kn", [NROWS, D], MT, kind="ExternalInput").ap()
        krn = nc.dram_tensor("krn", [NROWS, D], MT, kind="ExternalInput").ap()
    else:
        ident = nc.dram_tensor("ident", [C, C], F32, kind="ExternalInput").ap()
    v1 = nc.dram_tensor("v1", [NROWS, E1], MT, kind="ExternalInput").ap()
    mask = nc.dram_tensor("mask", [C, C], F32, kind="ExternalInput").ap()
    EO = E1 if host_norm else E
    out = nc.dram_tensor("out", [NROWS, EO], F32, kind="ExternalOutput").ap()

    if taper:
        plans = [{0: 2, 2: 4, 6: 4, 10: 4, 14: 2}] * PAIRS_PER_CORE
    elif stagger:
        plans = [{c0: SLAB for c0 in range(0, NCHUNK, SLAB)},
                 {0: 2, 2: 4, 6: 4, 10: 4, 14: 2}]
    else:
        plans = [{c0: SLAB for c0 in range(0, NCHUNK, SLAB)}] * PAIRS_PER_CORE
    slab_of = []
    for pp in range(PAIRS_PER_CORE):
        m = {}
        for c0, ln in plans[pp].items():
            for c in range(c0, c0 + ln):
                m[c] = (c0, ln)
        slab_of.append(m)

    with tile.TileContext(nc) as tc:
        with (
            tc.tile_pool(name="const", bufs=1) as constp,
            tc.tile_pool(name="slabs", bufs=SLAB_BUFS) as slabs,
            tc.tile_pool(name="atm", bufs=(6 if big_bufs else (4 if pipe == 1 else 6))) as atmp,
            tc.tile_pool(name="ssb", bufs=(12 if big_bufs else 8)) as ssbp,
            tc.tile_pool(name="dinv", bufs=(12 if big_bufs else 8)) as dinvp,
            tc.tile_pool(name="pat", bufs=(2 if (transpose_k or bank_42) else 3),
                         space="PSUM") as patp,
            tc.tile_pool(name="pout", bufs=(2 if transpose_k else (4 if bank_42 else 3)),
                         space="PSUM") as poutp,
            tc.tile_pool(name="pst", bufs=2, space="PSUM") as pstp,
            tc.tile_pool(name="ktr", bufs=2, space="PSUM") as ktrp,
            tc.tile_pool(name="kns", bufs=4) as knsp,
        ):
            mask_t = constp.tile([C, C], F32, tag="mask")
            nc.sync.dma_start(mask_t[:], mask[:])
            if transpose_k:
                ident_t = constp.tile([C, C], F32, tag="ident")
                nc.sync.dma_start(ident_t[:], ident[:])

            for rep in range(repeat):
              with (tc.For_i(0, loop_k, 1, hint_engines=(
                        mybir.EngineType.PE, mybir.EngineType.DVE,
                        mybir.EngineType.Activation, mybir.EngineType.SP))
                    if (loop_k is not None and loop_k > 1)
                    else _nullctx()):
                  # per-pair state accumulator in one PSUM bank:
                  # cols 0:65 -> branch 0 [S|Z], cols 66:131 -> branch 1
                  pS = {}
                  for p in range(PAIRS_PER_CORE):
                      pS[p] = pstp.tile([D, 2 * E1 + 2], F32, tag="pS", name=f"pS_{rep}_{p}")

                  slab_t = [None] * PAIRS_PER_CORE   # per pair: dict of slab tiles
                  S_sbuf = {}                        # (p, br) -> sbuf state tile

                  # Software pipeline, one chunk deep: the "front" stage of
                  # chunk c emits loads, the state update (PE), and AT+mask
                  # (PE then DVE); the "back" stage consumes chunk c-1's
                  # masked AT for the numerator/denominator matmuls. This
                  # gives every cross-engine hop a full stage of slack, so
                  # the PE never head-of-line blocks on DVE/ACT latency.
                  fifo = []
                  for cc in range(NCHUNK + pipe):
                    pending = {}
                    back = {}
                    if cc >= pipe:
                        back = fifo.pop(0)
                    if cc < NCHUNK:
                        fifo.append(pending)
                    if cc < NCHUNK:
                      c = cc
                      for p in range(PAIRS_PER_CORE):
                          c0, slen = slab_of[p][c]
                          if (c == c0) and not (reuse_slab and c > 0):
                              base = p * N + c * C
                              cols = slice(base, base + slen * C)
                              dmae = nc.gpsimd if (dma_split and p == 1) else nc.sync
                              st = {"len": slen}
                              if not load_reorder:
                                  st["qT"] = slabs.tile([D, slen * C], MT, tag="qT", name=f"qTs_{rep}_{p}_{c}")
                                  st["kT"] = slabs.tile([D, slen * C], MT, tag="kT", name=f"kTs_{rep}_{p}_{c}")
                                  if fast_start and c == 0:
                                      # split the very first q/k loads so chunk
                                      # 0's AT matmul starts after 128KB, not
                                      # a full slab (range-level tile deps)
                                      dmae.dma_start(st["qT"][:, 0:C], qT[:, base:base + C])
                                      dmae.dma_start(st["kT"][:, 0:C], kT[:, base:base + C])
                                      dmae.dma_start(st["qT"][:, C:slen * C], qT[:, base + C:base + slen * C])
                                      dmae.dma_start(st["kT"][:, C:slen * C], kT[:, base + C:base + slen * C])
                                  else:
                                      dmae.dma_start(st["qT"][:], qT[:, cols])
                                      dmae.dma_start(st["kT"][:], kT[:, cols])
                                  st["qrT"] = slabs.tile([D, slen * C], MT, tag="qrT", name=f"qrTs_{rep}_{p}_{c}")
                                  dmae.dma_start(st["qrT"][:], qrT[:, cols])
                                  st["krT"] = slabs.tile([D, slen * C], MT, tag="krT", name=f"krTs_{rep}_{p}_{c}")
                                  dmae.dma_start(st["krT"][:], krT[:, cols])
                              # load the state-update inputs (kn/krn/v1)
                              # first: they feed the first PE ops of the chunk
                              if not transpose_k:
                                  st["kn"] = slabs.tile([C, slen, D], MT, tag="kn", name=f"kns_{rep}_{p}_{c}")
                                  dmae.dma_start(
                                      st["kn"][:],
                                      kn[cols, :].rearrange("(n p) d -> p n d", p=C))
                                  st["krn"] = slabs.tile([C, slen, D], MT, tag="krn", name=f"krns_{rep}_{p}_{c}")
                                  dmae.dma_start(
                                      st["krn"][:],
                                      krn[cols, :].rearrange("(n p) d -> p n d", p=C))
                              st["v1"] = slabs.tile([C, slen, E1], MT, tag="v1", name=f"v1s_{rep}_{p}_{c}")
                              dmae.dma_start(
                                  st["v1"][:],
                                  v1[cols, :].rearrange("(n p) e -> p n e", p=C))
                              if load_reorder:
                                  st["kT"] = slabs.tile([D, slen * C], MT, tag="kT", name=f"kTs_{rep}_{p}_{c}")
                                  dmae.dma_start(st["kT"][:], kT[:, cols])
                                  st["qT"] = slabs.tile([D, slen * C], MT, tag="qT", name=f"qTs_{rep}_{p}_{c}")
                                  dmae.dma_start(st["qT"][:], qT[:, cols])
                                  st["qrT"] = slabs.tile([D, slen * C], MT, tag="qrT", name=f"qrTs_{rep}_{p}_{c}")
                                  dmae.dma_start(st["qrT"][:], qrT[:, cols])
                                  st["krT"] = slabs.tile([D, slen * C], MT, tag="krT", name=f"krTs_{rep}_{p}_{c}")
                                  dmae.dma_start(st["krT"][:], krT[:, cols])
                              st["outs"] = slabs.tile([C, slen, EO], F32, tag="outs", name=f"outs_{rep}_{p}_{c}")
                              slab_t[p] = st

                          st = slab_t[p]
                          j = c - c0
                          qcT = st["qT"][:, j * C:(j + 1) * C]
                          kcT = st["kT"][:, j * C:(j + 1) * C]
                          qrcT = st["qrT"][:, j * C:(j + 1) * C]
                          krcT = st["krT"][:, j * C:(j + 1) * C]
                          vc = st["v1"][:, j, :]
                          knc = krnc = None
                          if not transpose_k:
                              knc = st["kn"][:, j, :]
                              krnc = st["krn"][:, j, :]

                          if dma_only:
                              continue

                          if probe_pe_only:
                              # pure matmul throughput probe: same 7 MMs as the
                              # real kernel, but no cross-engine deps at all
                              pat0 = patp.tile([C, C], F32, tag="pat")
                              mm(pat0[:], kcT, qcT, start=True, stop=False)
                              mm(pat0[:], krcT, qrcT, start=False, stop=True)
                              po = poutp.tile([C, E1], F32, tag="po")
                              mm(po[:], mask_t[:], vc, start=True, stop=False)
                              mm(po[:], qcT, mask_t[:, 0:E1], start=False, stop=False, skip_group_check=True)
                              mm(po[:], qrcT, mask_t[:, 0:E1], start=False, stop=True, skip_group_check=True)
                              mm(pS[p][:, 0:E1], knc, vc, start=(c == 0), stop=False, skip_group_check=True)
                              mm(pS[p][:, E1 + 1:2 * E1 + 1], krnc, vc, start=False, stop=(c == NCHUNK - 1), skip_group_check=True)
                              continue

                          prev_S = S_sbuf.get(p)

                          if ilv:
                              # MMs emitted pair-interleaved after this loop
                              pending[p] = dict(qcT=qcT, qrcT=qrcT, kcT=kcT,
                                                krcT=krcT, knc=knc, krnc=krnc,
                                                vc=vc, st=st, j=j, c=c,
                                                prev_S=prev_S, kns0=None,
                                                kns1=None, c0=c0,
                                                slen=st.get("len", SLAB))
                              continue

                          # State update: both branches share one PSUM bank
                          # (start=True on c0/br0 clears it; br1 overwrites its
                          # unwritten columns). Without transpose_k the natural-
                          # layout k arrives by DMA and the update is emitted
                          # here (front stage); with transpose_k the k tiles are
                          # transposed on the PE this stage and the state update
                          # moves to the back stage so the transpose->copy->
                          # matmul chain gets a stage of slack.
                          kns0 = kns1 = None
                          if transpose_k and not probe_no_state:
                              ktp0 = ktrp.tile([C, C], F32, tag="ktr")
                              nc.tensor.transpose(ktp0[:], kcT, ident_t[:])
                              kns0 = knsp.tile([C, C], F32, tag="kns")
                              nc.vector.tensor_copy(kns0[:], ktp0[:])
                              ktp1 = ktrp.tile([C, C], F32, tag="ktr")
                              nc.tensor.transpose(ktp1[:], krcT, ident_t[:])
                              kns1 = knsp.tile([C, C], F32, tag="kns")
                              nc.scalar.copy(kns1[:], ktp1[:])
                          if not transpose_k and not probe_no_state:
                              mm(pS[p][:, 0:E1], knc, vc,
                                               start=(c == 0), stop=False,
                                               skip_group_check=True)
                              mm(pS[p][:, E1 + 1:2 * E1 + 1], krnc, vc,
                                               start=False, stop=(c == NCHUNK - 1),
                                               skip_group_check=True)
                              if c < NCHUNK - 1:
                                  s01 = ssbp.tile([D, 2 * E1 + 2], MT, tag="ssb")
                                  nc.scalar.copy(s01[:], pS[p][:])
                                  S_sbuf[p] = s01

                          # AT = K0 Q0^T + K1 Q1^T (both branches accumulate in
                          # one PSUM bank), then one causal mask (s<=t)
                          if probe_no_at:
                              atm0 = mask_t
                          else:
                              pat0 = patp.tile([C, C], F32, tag="pat")
                              if f32r:
                                  mm(pat0[:], kcT.bitcast(F32R),
                                     qcT.bitcast(F32R), start=True, stop=False)
                                  mm(pat0[:], krcT.bitcast(F32R),
                                     qrcT.bitcast(F32R), start=False, stop=True)
                              else:
                                  mm(pat0[:], kcT, qcT, start=True, stop=False)
                                  mm(pat0[:], krcT, qrcT, start=False, stop=True)
                              atm0 = atmp.tile([C, C], MT, tag="atm")
                              nc.vector.tensor_mul(atm0[:], pat0[:], mask_t[:])

                          pending[p] = dict(atm=atm0, qcT=qcT, qrcT=qrcT,
                                            vc=vc, st=st, j=j, c=c,
                                            prev_S=prev_S, kns0=kns0, kns1=kns1,
                                            c0=c0, slen=st.get("len", SLAB))

                    if ilv and cc < NCHUNK and not dma_only and not probe_pe_only:
                        ps = sorted(pending.keys())
                        # state matmuls, pair-interleaved (consecutive MMs hit
                        # different PSUM banks)
                        for br in range(2):
                            for p in ps:
                                z = pending[p]
                                if br == 0:
                                    mm(pS[p][:, 0:E1], z["knc"], z["vc"],
                                       start=(c == 0), stop=False,
                                       skip_group_check=True)
                                else:
                                    mm(pS[p][:, E1 + 1:2 * E1 + 1], z["krnc"],
                                       z["vc"], start=False,
                                       stop=(c == NCHUNK - 1),
                                       skip_group_check=True)
                        for p in ps:
                            if c < NCHUNK - 1:
                                s01 = ssbp.tile([D, 2 * E1 + 2], MT, tag="ssb",
                                                name=f"s01i_{rep}_{p}_{c}")
                                nc.scalar.copy(s01[:], pS[p][:])
                                S_sbuf[p] = s01
                        pats = {}
                        for p in ps:
                            pats[p] = patp.tile([C, C], F32, tag="pat",
                                                name=f"pati_{rep}_{p}_{c}")
                        for br in range(2):
                            for p in ps:
                                z = pending[p]
                                if br == 0:
                                    mm(pats[p][:], z["kcT"], z["qcT"],
                                       start=True, stop=False,
                                       skip_group_check=True)
                                else:
                                    mm(pats[p][:], z["krcT"], z["qrcT"],
                                       start=False, stop=True,
                                       skip_group_check=True)
                        for p in ps:
                            atm = atmp.tile([C, C], MT, tag="atm",
                                            name=f"atmi_{rep}_{p}_{c}")
                            nc.vector.tensor_mul(atm[:], pats[p][:], mask_t[:])
                            pending[p]["atm"] = atm

                    if ilv:
                        items = sorted(back.items())
                        pos = {}
                        for p, z in items:
                            pos[p] = poutp.tile([C, E1], F32, tag="po",
                                                name=f"poi_{rep}_{p}_{z['c']}")
                        for p, z in items:
                            mm(pos[p][:], z["atm"][:], z["vc"], start=True,
                               stop=(z["c"] == 0 or z["prev_S"] is None),
                               skip_group_check=True)
                        for p, z in items:
                            if z["c"] > 0 and z["prev_S"] is not None:
                                mm(pos[p][:], z["qcT"], z["prev_S"][:, 0:E1],
                                   start=False, stop=False,
                                   skip_group_check=True)
                        for p, z in items:
                            if z["c"] > 0 and z["prev_S"] is not None:
                                mm(pos[p][:], z["qrcT"],
                                   z["prev_S"][:, E1 + 1:2 * E1 + 1],
                                   start=False, stop=True,
                                   skip_group_check=True)
                        for p, z in items:
                            po = pos[p]
                            dinv = dinvp.tile([C, 1], F32, tag="dinv",
                                              name=f"dinvi_{rep}_{p}_{z['c']}")
                            nc.vector.reciprocal(dinv[:], po[:, E:E1])
                            nc.scalar.mul(z["st"]["outs"][:, z["j"], :],
                                          po[:, 0:E], dinv[:])
                            if z["j"] == z["slen"] - 1:
                                base = p * N + z["c0"] * C
                                rows = slice(base, base + z["slen"] * C)
                                nc.sync.dma_start(
                                    out[rows, :].rearrange(
                                        "(n p) e -> p n e", p=C),
                                    z["st"]["outs"][:])
                        back = {}

                    for p, z in back.items():
                        cb = z["c"]
                        # with transpose_k the state update happens here, so
                        # the pre-update state must be captured here as well
                        if transpose_k:
                            z["prev_S"] = S_sbuf.get(p)
                        if transpose_k and z["kns0"] is not None:
                            mm(pS[p][:, 0:E1], z["kns0"][:],
                                             z["vc"], start=(cb == 0),
                                             stop=False, skip_group_check=True)
                            mm(pS[p][:, E1 + 1:2 * E1 + 1],
                                             z["kns1"][:], z["vc"],
                                             start=False,
                                             stop=(cb == NCHUNK - 1),
                                             skip_group_check=True)
                            if cb < NCHUNK - 1:
                                s01 = ssbp.tile([D, 2 * E1 + 2], MT, tag="ssb")
                                nc.scalar.copy(s01[:], pS[p][:])
                                S_sbuf[p] = s01
                        # numerator (cols 0..63) + denominator (col 64)
                        po = poutp.tile([C, E1], F32, tag="po")
                        mm(po[:], z["atm"][:], z["vc"],
                                         start=True,
                                         stop=(cb == 0 or z["prev_S"] is None))
                        if cb > 0 and z["prev_S"] is not None:
                            mm(po[:], z["qcT"],
                                             z["prev_S"][:, 0:E1],
                                             start=False, stop=False,
                                             skip_group_check=True)
                            mm(po[:], z["qrcT"],
                                             z["prev_S"][:, E1 + 1:2 * E1 + 1],
                                             start=False, stop=True,
                                             skip_group_check=True)

                        if host_norm:
                            # ship numerator and denominator; host divides
                            nc.scalar.copy(z["st"]["outs"][:, z["j"], :],
                                           po[:, 0:E1])
                        else:
                            # out[t,:] = num[t,:] / den[t]
                            dinv = dinvp.tile([C, 1], F32, tag="dinv")
                            nc.vector.reciprocal(dinv[:], po[:, E:E1])
                            nc.scalar.mul(z["st"]["outs"][:, z["j"], :],
                                          po[:, 0:E], dinv[:])

                        if z["j"] == z["slen"] - 1:
                            base = p * N + z["c0"] * C
                            rows = slice(base, base + z["slen"] * C)
                            nc.sync.dma_start(
                                out[rows, :].rearrange("(n p) e -> p n e", p=C),
                                z["st"]["outs"][:])

    nc.compile()
    return nc




F16 = mybir.dt.float16
F8 = mybir.dt.float8e4
U8 = mybir.dt.uint8


def build_kernel16(repeat=1, loop_k=None, dma_only=False, probe_pe_only=False,
                   slab=SLAB, slab_bufs=SLAB_BUFS):
    """fp16 variant. All inputs ship as fp16; natural-layout tensors
    (kn/krn/v1) and the output use a chunk-major [C, nchunk*f] DRAM layout so
    every DMA descriptor is a contiguous >=512B run. fp16 matmuls run at 1
    cycle/row on the PE (vs 4 for fp32), accumulation stays f32 in PSUM.
    Host un-permutes the output and upcasts to f32."""
    nc = bacc.Bacc("TRN2", target_bir_lowering=False, debug=False,
                   num_devices=N_CORES)

    NPC = PAIRS_PER_CORE
    qT = nc.dram_tensor("qT", [D, NROWS], F16, kind="ExternalInput").ap()
    kT = nc.dram_tensor("kT", [D, NROWS], F16, kind="ExternalInput").ap()
    qrT = nc.dram_tensor("qrT", [D, NROWS], F16, kind="ExternalInput").ap()
    krT = nc.dram_tensor("krT", [D, NROWS], F16, kind="ExternalInput").ap()
    kn = nc.dram_tensor("kn", [C, NPC * NCHUNK * D], F16, kind="ExternalInput").ap()
    krn = nc.dram_tensor("krn", [C, NPC * NCHUNK * D], F16, kind="ExternalInput").ap()
    v1 = nc.dram_tensor("v1", [C, NPC * NCHUNK * E1], F16, kind="ExternalInput").ap()
    mask = nc.dram_tensor("mask", [C, C], F32, kind="ExternalInput").ap()
    out = nc.dram_tensor("out", [C, NPC * NCHUNK * E], F16, kind="ExternalOutput").ap()

    plans = [{c0: slab for c0 in range(0, NCHUNK, slab)}] * NPC
    slab_of = []
    for pp in range(NPC):
        m = {}
        for c0, ln in plans[pp].items():
            for c in range(c0, c0 + ln):
                m[c] = (c0, ln)
        slab_of.append(m)

    with tile.TileContext(nc) as tc:
        with (
            tc.tile_pool(name="const", bufs=1) as constp,
            tc.tile_pool(name="slabs", bufs=slab_bufs) as slabs,
            tc.tile_pool(name="atm", bufs=4) as atmp,
            tc.tile_pool(name="ssb", bufs=8) as ssbp,
            tc.tile_pool(name="dinv", bufs=8) as dinvp,
            tc.tile_pool(name="pat", bufs=3, space="PSUM") as patp,
            tc.tile_pool(name="pout", bufs=3, space="PSUM") as poutp,
            tc.tile_pool(name="pst", bufs=(1 if pp else 2), space="PSUM") as pstp,
        ):
            mask_t = constp.tile([C, C], F32, tag="mask")
            nc.sync.dma_start(mask_t[:], mask[:])

            for rep in range(repeat):
              with (tc.For_i(0, loop_k, 1, hint_engines=(
                        mybir.EngineType.PE, mybir.EngineType.DVE,
                        mybir.EngineType.Activation, mybir.EngineType.SP))
                    if (loop_k is not None and loop_k > 1)
                    else _nullctx()):
                  pS = {}
                  for p in range(NPC):
                      pS[p] = pstp.tile([D, 2 * E1 + 2], F32, tag="pS",
                                        name=f"pS16_{rep}_{p}")

                  slab_t = [None] * NPC
                  S_sbuf = {}

                  fifo = []
                  for cc in range(NCHUNK + 1):
                    pending = {}
                    back = {}
                    if cc >= 1:
                        back = fifo.pop(0)
                    if cc < NCHUNK:
                        fifo.append(pending)
                    if cc < NCHUNK:
                      c = cc
                      for p in range(NPC):
                          c0, slen = slab_of[p][c]
                          if c == c0:
                              base = p * N + c * C
                              cols = slice(base, base + slen * C)
                              ncols = slice((p * NCHUNK + c) * D,
                                            (p * NCHUNK + c + slen) * D)
                              vcols = slice((p * NCHUNK + c) * E1,
                                            (p * NCHUNK + c + slen) * E1)
                              st = {"len": slen}
                              st["qT"] = slabs.tile([D, slen * C], F16, tag="qT", name=f"qTs16_{rep}_{p}_{c}")
                              nc.sync.dma_start(st["qT"][:], qT[:, cols])
                              st["kT"] = slabs.tile([D, slen * C], F16, tag="kT", name=f"kTs16_{rep}_{p}_{c}")
                              nc.sync.dma_start(st["kT"][:], kT[:, cols])
                              st["qrT"] = slabs.tile([D, slen * C], F16, tag="qrT", name=f"qrTs16_{rep}_{p}_{c}")
                              nc.sync.dma_start(st["qrT"][:], qrT[:, cols])
                              st["krT"] = slabs.tile([D, slen * C], F16, tag="krT", name=f"krTs16_{rep}_{p}_{c}")
                              nc.sync.dma_start(st["krT"][:], krT[:, cols])
                              st["kn"] = slabs.tile([C, slen * D], F16, tag="kn", name=f"kns16_{rep}_{p}_{c}")
                              nc.sync.dma_start(st["kn"][:], kn[:, ncols])
                              st["krn"] = slabs.tile([C, slen * D], F16, tag="krn", name=f"krns16_{rep}_{p}_{c}")
                              nc.sync.dma_start(st["krn"][:], krn[:, ncols])
                              st["v1"] = slabs.tile([C, slen * E1], F16, tag="v1", name=f"v1s16_{rep}_{p}_{c}")
                              nc.sync.dma_start(st["v1"][:], v1[:, vcols])
                              st["outs"] = slabs.tile([C, slen * E], F16, tag="outs", name=f"outs16_{rep}_{p}_{c}")
                              slab_t[p] = st

                          st = slab_t[p]
                          j = c - c0
                          if dma_only:
                              continue
                          z = dict(
                              qcT=st["qT"][:, j * C:(j + 1) * C],
                              kcT=st["kT"][:, j * C:(j + 1) * C],
                              qrcT=st["qrT"][:, j * C:(j + 1) * C],
                              krcT=st["krT"][:, j * C:(j + 1) * C],
                              knc=st["kn"][:, j * D:(j + 1) * D],
                              krnc=st["krn"][:, j * D:(j + 1) * D],
                              vc=st["v1"][:, j * E1:(j + 1) * E1],
                              st=st, j=j, c=c, c0=c0, slen=slen,
                              prev_S=S_sbuf.get(p))
                          pending[p] = z

                      if probe_pe_only and pending:
                          for p, z in sorted(pending.items()):
                              pat0 = patp.tile([C, C], F32, tag="pat")
                              nc.tensor.matmul(pat0[:], z["kcT"], z["qcT"], start=True, stop=False)
                              nc.tensor.matmul(pat0[:], z["krcT"], z["qrcT"], start=False, stop=True)
                              po = poutp.tile([C, E1], F32, tag="po")
                              nc.tensor.matmul(po[:], z["qcT"], mask_t[:, 0:E1].bitcast(F16)[:, 0:E1], start=True, stop=False, skip_group_check=True)
                              nc.tensor.matmul(po[:], z["qrcT"], mask_t[:, 0:E1].bitcast(F16)[:, 0:E1], start=False, stop=False, skip_group_check=True)
                              nc.tensor.matmul(po[:], z["kcT"], mask_t[:, 0:E1].bitcast(F16)[:, 0:E1], start=False, stop=True, skip_group_check=True)
                              nc.tensor.matmul(pS[p][:, 0:E1], z["knc"], z["vc"], start=(z["c"] == 0), stop=False, skip_group_check=True)
                              nc.tensor.matmul(pS[p][:, E1 + 1:2 * E1 + 1], z["krnc"], z["vc"], start=False, stop=(z["c"] == NCHUNK - 1), skip_group_check=True)
                          continue

                      if pending and not dma_only:
                        ps = sorted(pending.keys())
                        for br in range(2):
                            for p in ps:
                                z = pending[p]
                                if br == 0:
                                    nc.tensor.matmul(pS[p][:, 0:E1], z["knc"],
                                                     z["vc"], start=(c == 0),
                                                     stop=False,
                                                     skip_group_check=True)
                                else:
                                    nc.tensor.matmul(pS[p][:, E1 + 1:2 * E1 + 1],
                                                     z["krnc"], z["vc"],
                                                     start=False,
                                                     stop=(c == NCHUNK - 1),
                                                     skip_group_check=True)
                        for p in ps:
                            if c < NCHUNK - 1:
                                s01 = ssbp.tile([D, 2 * E1 + 2], F16, tag="ssb",
                                                name=f"s01h_{rep}_{p}_{c}")
                                nc.scalar.copy(s01[:], pS[p][:])
                                S_sbuf[p] = s01
                        pats = {}
                        for p in ps:
                            pats[p] = patp.tile([C, C], F32, tag="pat",
                                                name=f"path_{rep}_{p}_{c}")
                        for br in range(2):
                            for p in ps:
                                z = pending[p]
                                if br == 0:
                                    nc.tensor.matmul(pats[p][:], z["kcT"],
                                                     z["qcT"], start=True,
                                                     stop=False,
                                                     skip_group_check=True)
                                else:
                                    nc.tensor.matmul(pats[p][:], z["krcT"],
                                                     z["qrcT"], start=False,
                                                     stop=True,
                                                     skip_group_check=True)
                        for p in ps:
                            atm = atmp.tile([C, C], F16, tag="atm",
                                            name=f"atmh_{rep}_{p}_{c}")
                            nc.vector.tensor_mul(atm[:], pats[p][:], mask_t[:])
                            pending[p]["atm"] = atm

                    if back and not dma_only and not probe_pe_only:
                        items = sorted(back.items())
                        pos = {}
                        for p, z in items:
                            pos[p] = poutp.tile([C, E1], F32, tag="po",
                                                name=f"poh_{rep}_{p}_{z['c']}")
                        for p, z in items:
                            nc.tensor.matmul(pos[p][:], z["atm"][:], z["vc"],
                                             start=True,
                                             stop=(z["c"] == 0 or z["prev_S"] is None),
                                             skip_group_check=True)
                        for p, z in items:
                            if z["c"] > 0 and z["prev_S"] is not None:
                                nc.tensor.matmul(pos[p][:], z["qcT"],
                                                 z["prev_S"][:, 0:E1],
                                                 start=False, stop=False,
                                                 skip_group_check=True)
                        for p, z in items:
                            if z["c"] > 0 and z["prev_S"] is not None:
                                nc.tensor.matmul(pos[p][:], z["qrcT"],
                                                 z["prev_S"][:, E1 + 1:2 * E1 + 1],
                                                 start=False, stop=True,
                                                 skip_group_check=True)
                        for p, z in items:
                            po = pos[p]
                            dinv = dinvp.tile([C, 1], F32, tag="dinv",
                                              name=f"dinvh_{rep}_{p}_{z['c']}")
                            nc.vector.reciprocal(dinv[:], po[:, E:E1])
                            nc.scalar.mul(
                                z["st"]["outs"][:, z["j"] * E:(z["j"] + 1) * E],
                                po[:, 0:E], dinv[:])
                            if z["j"] == z["slen"] - 1:
                                ocols = slice((p * NCHUNK + z["c0"]) * E,
                                              (p * NCHUNK + z["c0"] + z["slen"]) * E)
                                nc.sync.dma_start(out[:, ocols],
                                                  z["st"]["outs"][:])

    nc.compile()
    return nc


def build_kernel16b(repeat=1, loop_k=None, dma_only=False, probe_pe_pure=False,
                    slab=8, slab_bufs=3):
    """fp16 + packed-DMA variant: per (pair, slab) ONE load of the merged
    transposed block [qT|kT|qrT|krT], ONE load of the merged natural block
    [kn|krn|v1], ONE store of the output block. At slab=8 that is 12 DMA
    instructions per iteration (vs 57 in v1), sidestepping the ~625ns/DMA
    HWDGE descriptor-generation serialization that dominated the v1 floor.

    probe_pe_pure: run the full per-chunk matmul bundle on tiles loaded once
    outside the loop — a clean PE-only floor with no DMA dependencies."""
    nc = bacc.Bacc("TRN2", target_bir_lowering=False, debug=False,
                   num_devices=N_CORES)

    NPC = PAIRS_PER_CORE
    NSLAB = NCHUNK // slab
    SC = slab * C
    KVW = slab * (2 * D + E1)       # merged natural-block width per slab
    qk = nc.dram_tensor("qk", [D, NPC * NSLAB * 4 * SC], F16,
                        kind="ExternalInput").ap()
    kv = nc.dram_tensor("kv", [C, NPC * NSLAB * KVW], F16,
                        kind="ExternalInput").ap()
    mask = nc.dram_tensor("mask", [C, C], F32, kind="ExternalInput").ap()
    out = nc.dram_tensor("out", [C, NPC * NCHUNK * E], F16,
                         kind="ExternalOutput").ap()

    with tile.TileContext(nc) as tc:
        with (
            tc.tile_pool(name="const", bufs=1) as constp,
            tc.tile_pool(name="slabs", bufs=slab_bufs) as slabs,
            tc.tile_pool(name="atm", bufs=4) as atmp,
            tc.tile_pool(name="ssb", bufs=8) as ssbp,
            tc.tile_pool(name="dinv", bufs=8) as dinvp,
            tc.tile_pool(name="pat", bufs=3, space="PSUM") as patp,
            tc.tile_pool(name="pout", bufs=3, space="PSUM") as poutp,
            tc.tile_pool(name="pst", bufs=(1 if pp else 2), space="PSUM") as pstp,
        ):
            mask_t = constp.tile([C, C], F32, tag="mask")
            nc.sync.dma_start(mask_t[:], mask[:])

            pure = {}
            if probe_pe_pure:
                # one fixed tile set, loaded once; the loop's MMs reference it
                pure["qk"] = constp.tile([D, 4 * SC], F16, tag="pqk", name="pqk")
                nc.sync.dma_start(pure["qk"][:], qk[:, 0:4 * SC])
                pure["kv"] = constp.tile([C, KVW], F16, tag="pkv", name="pkv")
                nc.sync.dma_start(pure["kv"][:], kv[:, 0:KVW])
                pure["atm"] = constp.tile([C, C], F16, tag="patm", name="patm")
                nc.vector.tensor_copy(pure["atm"][:], mask_t[:])
                pure["s01"] = constp.tile([D, 2 * E1 + 2], F16, tag="ps01", name="ps01")
                nc.vector.tensor_copy(pure["s01"][:], pure["kv"][:, 0:2 * E1 + 2])

            for rep in range(repeat):
              with (tc.For_i(0, loop_k, 1, hint_engines=(
                        mybir.EngineType.PE, mybir.EngineType.DVE,
                        mybir.EngineType.Activation, mybir.EngineType.SP))
                    if (loop_k is not None and loop_k > 1)
                    else _nullctx()):
                  if probe_pe_pure:
                      # 7-MM bundle x NCHUNK x NPC on fixed tiles
                      pqk, pkv = pure["qk"], pure["kv"]
                      for c in range(NCHUNK):
                        for p in range(NPC):
                          j = c % slab
                          qcT = pqk[:, 0 * SC + j * C:0 * SC + (j + 1) * C]
                          kcT = pqk[:, 1 * SC + j * C:1 * SC + (j + 1) * C]
                          qrcT = pqk[:, 2 * SC + j * C:2 * SC + (j + 1) * C]
                          krcT = pqk[:, 3 * SC + j * C:3 * SC + (j + 1) * C]
                          knc = pkv[:, j * D:(j + 1) * D]
                          krnc = pkv[:, slab * D + j * D:slab * D + (j + 1) * D]
                          vc = pkv[:, 2 * slab * D + j * E1:2 * slab * D + (j + 1) * E1]
                          pS = pstp.tile([D, 2 * E1 + 2], F32, tag="pS")
                          nc.tensor.matmul(pS[:, 0:E1], knc, vc, start=True, stop=False, skip_group_check=True)
                          nc.tensor.matmul(pS[:, E1 + 1:2 * E1 + 1], krnc, vc, start=False, stop=True, skip_group_check=True)
                          pat0 = patp.tile([C, C], F32, tag="pat")
                          nc.tensor.matmul(pat0[:], kcT, qcT, start=True, stop=False)
                          nc.tensor.matmul(pat0[:], krcT, qrcT, start=False, stop=True)
                          po = poutp.tile([C, E1], F32, tag="po")
                          nc.tensor.matmul(po[:], pure["atm"][:, 0:C], vc, start=True, stop=False, skip_group_check=True)
                          nc.tensor.matmul(po[:], qcT, pure["s01"][:, 0:E1], start=False, stop=False, skip_group_check=True)
                          nc.tensor.matmul(po[:], qrcT, pure["s01"][:, E1 + 1:2 * E1 + 1], start=False, stop=True, skip_group_check=True)
                      continue

                  pS = {}
                  for p in range(NPC):
                      pS[p] = pstp.tile([D, 2 * E1 + 2], F32, tag="pS",
                                        name=f"pSb_{rep}_{p}")

                  slab_t = [None] * NPC
                  S_sbuf = {}

                  fifo = []
                  for cc in range(NCHUNK + 1):
                    pending = {}
                    back = {}
                    if cc >= 1:
                        back = fifo.pop(0)
                    if cc < NCHUNK:
                        fifo.append(pending)
                    if cc < NCHUNK:
                      c = cc
                      for p in range(NPC):
                          c0 = (c // slab) * slab
                          si = c // slab
                          if c == c0:
                              qbase = (p * NSLAB + si) * 4 * SC
                              kbase = (p * NSLAB + si) * KVW
                              st = {}
                              st["kv"] = slabs.tile([C, KVW], F16, tag="kv",
                                                    name=f"kvs_{rep}_{p}_{c}")
                              nc.sync.dma_start(st["kv"][:],
                                                kv[:, kbase:kbase + KVW])
                              st["qk"] = slabs.tile([D, 4 * SC], F16, tag="qk",
                                                    name=f"qks_{rep}_{p}_{c}")
                              nc.sync.dma_start(st["qk"][:],
                                                qk[:, qbase:qbase + 4 * SC])
                              st["outs"] = slabs.tile([C, slab * E], F16,
                                                      tag="outs",
                                                      name=f"outsb_{rep}_{p}_{c}")
                              slab_t[p] = st

                          st = slab_t[p]
                          j = c - c0
                          if dma_only:
                              continue
                          z = dict(
                              qcT=st["qk"][:, 0 * SC + j * C:0 * SC + (j + 1) * C],
                              kcT=st["qk"][:, 1 * SC + j * C:1 * SC + (j + 1) * C],
                              qrcT=st["qk"][:, 2 * SC + j * C:2 * SC + (j + 1) * C],
                              krcT=st["qk"][:, 3 * SC + j * C:3 * SC + (j + 1) * C],
                              knc=st["kv"][:, j * D:(j + 1) * D],
                              krnc=st["kv"][:, slab * D + j * D:slab * D + (j + 1) * D],
                              vc=st["kv"][:, 2 * slab * D + j * E1:2 * slab * D + (j + 1) * E1],
                              st=st, j=j, c=c, c0=c0, slen=slab,
                              prev_S=S_sbuf.get(p))
                          pending[p] = z

                      if pending and not dma_only:
                        ps = sorted(pending.keys())
                        for br in range(2):
                            for p in ps:
                                z = pending[p]
                                if br == 0:
                                    nc.tensor.matmul(pS[p][:, 0:E1], z["knc"],
                                                     z["vc"], start=(c == 0),
                                                     stop=False,
                                                     skip_group_check=True)
                                else:
                                    nc.tensor.matmul(pS[p][:, E1 + 1:2 * E1 + 1],
                                                     z["krnc"], z["vc"],
                                                     start=False,
                                                     stop=(c == NCHUNK - 1),
                                                     skip_group_check=True)
                        for p in ps:
                            if c < NCHUNK - 1:
                                s01 = ssbp.tile([D, 2 * E1 + 2], F16, tag="ssb",
                                                name=f"s01b_{rep}_{p}_{c}")
                                nc.scalar.copy(s01[:], pS[p][:])
                                S_sbuf[p] = s01
                        pats = {}
                        for p in ps:
                            pats[p] = patp.tile([C, C], F32, tag="pat",
                                                name=f"patb_{rep}_{p}_{c}")
                        for br in range(2):
                            for p in ps:
                                z = pending[p]
                                if br == 0:
                                    nc.tensor.matmul(pats[p][:], z["kcT"],
                                                     z["qcT"], start=True,
                                                     stop=False,
                                                     skip_group_check=True)
                                else:
                                    nc.tensor.matmul(pats[p][:], z["krcT"],
                                                     z["qrcT"], start=False,
                                                     stop=True,
                                                     skip_group_check=True)
                        for p in ps:
                            atm = atmp.tile([C, C], F16, tag="atm",
                                            name=f"atmb_{rep}_{p}_{c}")
                            nc.vector.tensor_mul(atm[:], pats[p][:], mask_t[:])
                            pending[p]["atm"] = atm

                    if back and not dma_only:
                        items = sorted(back.items())
                        pos = {}
                        for p, z in items:
                            pos[p] = poutp.tile([C, E1], F32, tag="po",
                                                name=f"pob_{rep}_{p}_{z['c']}")
                        for p, z in items:
                            nc.tensor.matmul(pos[p][:], z["atm"][:], z["vc"],
                                             start=True,
                                             stop=(z["c"] == 0 or z["prev_S"] is None),
                                             skip_group_check=True)
                        for p, z in items:
                            if z["c"] > 0 and z["prev_S"] is not None:
                                nc.tensor.matmul(pos[p][:], z["qcT"],
                                                 z["prev_S"][:, 0:E1],
                                                 start=False, stop=False,
                                                 skip_group_check=True)
                        for p, z in items:
                            if z["c"] > 0 and z["prev_S"] is not None:
                                nc.tensor.matmul(pos[p][:], z["qrcT"],
                                                 z["prev_S"][:, E1 + 1:2 * E1 + 1],
                                                 start=False, stop=True,
                                                 skip_group_check=True)
                        for p, z in items:
                            po = pos[p]
                            dinv = dinvp.tile([C, 1], F32, tag="dinv",
                                              name=f"dinvb_{rep}_{p}_{z['c']}")
                            nc.vector.reciprocal(dinv[:], po[:, E:E1])
                            nc.scalar.mul(
                                z["st"]["outs"][:, z["j"] * E:(z["j"] + 1) * E],
                                po[:, 0:E], dinv[:])
                            if z["j"] == z["slen"] - 1:
                                ocols = slice((p * NCHUNK + z["c0"]) * E,
                                              (p * NCHUNK + z["c0"] + z["slen"]) * E)
                                nc.sync.dma_start(out[:, ocols],
                                                  z["st"]["outs"][:])

    nc.compile()
    return nc


def build_kernel16c(repeat=1, loop_k=None, dma_only=False, slab=8,
                    slab_bufs=3, recip_dev=False):
    """v3: fp16 + packed DMA (as 16b) + pair-merged PSUM banks.

    Both (b,h) pairs handled by a core share single PSUM banks for AT, for
    num|den, and for the scan state, so the causal mask, the state
    evacuation, and the num/den evacuation each run as ONE wide DVE/ACT op
    per chunk instead of one per pair. The division happens on the host
    (kernel ships num and den); no reciprocal / scale ops on device.
    """
    nc = bacc.Bacc("TRN2", target_bir_lowering=False, debug=False,
                   num_devices=N_CORES)

    NPC = PAIRS_PER_CORE
    NSLAB = NCHUNK // slab
    SC = slab * C
    KVW = slab * (2 * D + E1)
    OW = 2 * E1                      # per-chunk output cols (both pairs)
    qk = nc.dram_tensor("qk", [D, NPC * NSLAB * 4 * SC], F16,
                        kind="ExternalInput").ap()
    kv = nc.dram_tensor("kv", [C, NPC * NSLAB * KVW], F16,
                        kind="ExternalInput").ap()
    mask2 = nc.dram_tensor("mask2", [C, 2 * C], F32, kind="ExternalInput").ap()
    out = nc.dram_tensor("out", [C, NCHUNK * OW], F16,
                         kind="ExternalOutput").ap()

    with tile.TileContext(nc) as tc:
        with (
            tc.tile_pool(name="const", bufs=1) as constp,
            tc.tile_pool(name="slabs", bufs=slab_bufs) as slabs,
            tc.tile_pool(name="atm", bufs=4) as atmp,
            tc.tile_pool(name="ssb", bufs=4) as ssbp,
            tc.tile_pool(name="pat", bufs=3, space="PSUM") as patp,
            tc.tile_pool(name="pout", bufs=3, space="PSUM") as poutp,
            tc.tile_pool(name="pst", bufs=(1 if pp else 2), space="PSUM") as pstp,
        ):
            mask_t = constp.tile([C, 2 * C], F32, tag="mask")
            nc.sync.dma_start(mask_t[:], mask2[:])

            fixed = None
            if probe_nodma:
                # one 8-chunk slab-set per pair, loaded once; the loop reuses
                # it for both halves -> compute rhythm with no load DMAs
                fixed = []
                fl = 8
                for p in range(NPC):
                    fqk = constp.tile([D, 4 * fl * C], F16, tag=f"fqk{p}",
                                      name=f"fqk{p}")
                    nc.sync.dma_start(fqk[:], qk[:, 0:4 * fl * C])
                    fkv = constp.tile([C, fl * (2 * D + E1)], F16,
                                      tag=f"fkv{p}", name=f"fkv{p}")
                    nc.sync.dma_start(fkv[:], kv[:, 0:fl * (2 * D + E1)])
                    fixed.append({"qk": fqk, "kv": fkv, "ln": fl})

            for rep in range(repeat):
              with (tc.For_i(0, loop_k, 1, hint_engines=(
                        mybir.EngineType.PE, mybir.EngineType.DVE,
                        mybir.EngineType.Activation, mybir.EngineType.SP))
                    if (loop_k is not None and loop_k > 1)
                    else _nullctx()):
                  pSb = pstp.tile([D, 2 * NPC * SW], F32, tag="pS",
                                  name=f"pSc_{rep}")

                  slab_t = [None] * NPC
                  outs_t = [None]
                  S_sbuf = [None]

                  pend = None
                  for cc in range(NCHUNK + 1):
                    back = pend
                    pend = None
                    if cc < NCHUNK:
                      c = cc
                      si = c // slab
                      c0 = si * slab
                      j = c - c0
                      sl = {}
                      for p in range(NPC):
                          if c == c0:
                              qbase = (p * NSLAB + si) * 4 * SC
                              kbase = (p * NSLAB + si) * KVW
                              st = {}
                              st["kv"] = slabs.tile([C, KVW], F16, tag="kv",
                                                    name=f"kvc_{rep}_{p}_{c}")
                              nc.sync.dma_start(st["kv"][:],
                                                kv[:, kbase:kbase + KVW])
                              st["qk"] = slabs.tile([D, 4 * SC], F16, tag="qk",
                                                    name=f"qkc_{rep}_{p}_{c}")
                              nc.sync.dma_start(st["qk"][:],
                                                qk[:, qbase:qbase + 4 * SC])
                              slab_t[p] = st
                          st = slab_t[p]
                          sl[p] = dict(
                              qcT=st["qk"][:, 0 * SC + j * C:0 * SC + (j + 1) * C],
                              kcT=st["qk"][:, 1 * SC + j * C:1 * SC + (j + 1) * C],
                              qrcT=st["qk"][:, 2 * SC + j * C:2 * SC + (j + 1) * C],
                              krcT=st["qk"][:, 3 * SC + j * C:3 * SC + (j + 1) * C],
                              knc=st["kv"][:, j * D:(j + 1) * D],
                              krnc=st["kv"][:, slab * D + j * D:slab * D + (j + 1) * D],
                              vc=st["kv"][:, 2 * slab * D + j * E1:2 * slab * D + (j + 1) * E1])
                      if c == c0:
                          outs_t[0] = slabs.tile([C, slab * OW], F16, tag="outs",
                                                 name=f"outsc_{rep}_{c}")
                      if dma_only:
                          if c - c0 == slab - 1:
                              ocols = slice(c0 * OW, (c0 + slab) * OW)
                              nc.sync.dma_start(out[:, ocols], outs_t[0][:])
                          continue

                      prev_S = S_sbuf[0]

                      # state updates: 4 regions of one PSUM bank
                      for br in range(2):
                          for p in range(NPC):
                              z = sl[p]
                              reg = pSb[:, (2 * p + br) * SW:(2 * p + br) * SW + E1]
                              nc.tensor.matmul(
                                  reg, z["knc"] if br == 0 else z["krnc"],
                                  z["vc"],
                                  start=(c == 0 and br == 0 and p == 0),
                                  stop=(c == NCHUNK - 1 and br == 1 and p == NPC - 1),
                                  skip_group_check=True)
                      if c < NCHUNK - 1:
                          s01 = ssbp.tile([D, 2 * NPC * SW], F16, tag="ssb",
                                          name=f"s01c_{rep}_{c}")
                          nc.scalar.copy(s01[:], pSb[:])
                          S_sbuf[0] = s01

                      # AT for both pairs into one bank, one mask op
                      patb = patp.tile([C, 2 * C], F32, tag="pat",
                                       name=f"patc_{rep}_{c}")
                      for br in range(2):
                          for p in range(NPC):
                              z = sl[p]
                              reg = patb[:, p * C:(p + 1) * C]
                              nc.tensor.matmul(
                                  reg, z["kcT"] if br == 0 else z["krcT"],
                                  z["qcT"] if br == 0 else z["qrcT"],
                                  start=(br == 0 and p == 0),
                                  stop=(br == 1 and p == NPC - 1),
                                  skip_group_check=True)
                      atm = atmp.tile([C, 2 * C], F16, tag="atm",
                                      name=f"atmc_{rep}_{c}")
                      nc.vector.tensor_mul(atm[:], patb[:], mask_t[:])

                      pend = dict(atm=atm, sl=sl, c=c, c0=c0, j=j,
                                  prev_S=prev_S, outs=outs_t[0])

                    if back is not None and not dma_only:
                        cb = back["c"]
                        pob = poutp.tile([C, NPC * PW], F32, tag="po",
                                         name=f"poc_{rep}_{cb}")
                        for p in range(NPC):
                            z = back["sl"][p]
                            reg = pob[:, p * PW:p * PW + E1]
                            nc.tensor.matmul(
                                reg, back["atm"][:, p * C:(p + 1) * C],
                                z["vc"], start=(p == 0),
                                stop=(cb == 0 and p == NPC - 1),
                                skip_group_check=True)
                        if cb > 0:
                            pv = back["prev_S"]
                            if pp:
                                terms = [t for t in pv if t is not None]
                            else:
                                terms = [pv]
                            for ti, term in enumerate(terms):
                                last_t = (ti == len(terms) - 1)
                                for br in range(2):
                                    for p in range(NPC):
                                        z = back["sl"][p]
                                        reg = pob[:, p * PW:p * PW + E1]
                                        nc.tensor.matmul(
                                            reg,
                                            z["qcT"] if br == 0 else z["qrcT"],
                                            term[:, (2 * p + br) * SW:(2 * p + br) * SW + E1],
                                            start=False,
                                            stop=(last_t and br == 1 and p == NPC - 1),
                                            skip_group_check=True)

                        # evacuate num|den for both pairs in one op
                        src = pob[:].rearrange("p (g w) -> p g w", g=NPC)[:, :, 0:E1]
                        dst = back["outs"][:, back["j"] * OW:(back["j"] + 1) * OW]
                        dst = dst.rearrange("p (g w) -> p g w", g=NPC)
                        if cb % 2 == 0:
                            nc.vector.tensor_copy(dst, src)
                        else:
                            nc.scalar.copy(dst, src)
                        if back["j"] == slab - 1:
                            ocols = slice(back["c0"] * OW,
                                          (back["c0"] + slab) * OW)
                            nc.sync.dma_start(out[:, ocols], back["outs"][:])

    nc.compile()
    return nc


def _plan_slabs(plan):
    """plan: list of slab lengths summing to NCHUNK -> per-chunk lookup."""
    slabs, c0 = [], 0
    for ln in plan:
        slabs.append((c0, ln))
        c0 += ln
    assert c0 == NCHUNK
    of = {}
    for si, (c0, ln) in enumerate(slabs):
        for c in range(c0, c0 + ln):
            of[c] = (si, c0, ln)
    return slabs, of


def build_kernel16d(repeat=1, loop_k=None, dma_only=False, slab=8,
                    slab_bufs=3, pipe=2, store_q="gpsimd", plan=None,
                    probe_nodma=False, stag=False, add_eng="vector",
                    kn8=False, pp=False):
    """v4: 16c + latency-chain fixes.

    - The scan state lives as an SBUF fp16 running sum: each chunk's outer
      product goes to a FRESH PSUM bank (no in-place PSUM accumulation), and
      ACT folds it into the running state (s01 += pS). This breaks the
      PE->ACT->PE anti-dependency cycle that serialized v3 (~1us/chunk).
    - Back stage (num/den matmuls + evacuation) trails by `pipe` chunks so
      every cross-engine hop has >= 1 full stage of slack.
    - Output stores go on the Pool (SWDGE) queue so the SP load queue never
      head-of-line blocks on end-of-iteration stores.
    """
    nc = bacc.Bacc("TRN2", target_bir_lowering=False, debug=False,
                   num_devices=N_CORES)

    NPC = PAIRS_PER_CORE
    if plan is None:
        plan = [slab] * (NCHUNK // slab)
    slabs_l, slab_of = _plan_slabs(plan)
    # per-(pair, slab) base offsets in the packed DRAM tensors
    QKW_of = [4 * ln * C for _, ln in slabs_l]
    # kv block: kn|krn (fp8 bytes when kn8) + v1 (fp16); widths in ELEMENTS
    # of the kv dram dtype (uint8 when kn8, fp16 otherwise)
    KD = D if kn8 else D          # kn col width per chunk in dram elements
    KVW_of = [ln * (2 * D + 2 * E1) if kn8 else ln * (2 * D + E1)
              for _, ln in slabs_l]
    KVT = U8 if kn8 else F16
    qk_tot = sum(QKW_of)
    kv_tot = sum(KVW_of)
    qk_base = [[p * qk_tot + sum(QKW_of[:si]) for si in range(len(slabs_l))]
               for p in range(NPC)]
    kv_base = [[p * kv_tot + sum(KVW_of[:si]) for si in range(len(slabs_l))]
               for p in range(NPC)]
    OW = 2 * E1
    qk = nc.dram_tensor("qk", [D, NPC * qk_tot], F16,
                        kind="ExternalInput").ap()
    kv = nc.dram_tensor("kv", [C, NPC * kv_tot], KVT,
                        kind="ExternalInput").ap()
    mask2 = nc.dram_tensor("mask2", [C, 2 * C], F32, kind="ExternalInput").ap()
    out = nc.dram_tensor("out", [C, NCHUNK * OW], F16,
                         kind="ExternalOutput").ap()

    store_eng = {"gpsimd": nc.gpsimd, "sync": nc.sync}[store_q]

    with tile.TileContext(nc) as tc:
        with (
            tc.tile_pool(name="const", bufs=1) as constp,
            tc.tile_pool(name="slabs", bufs=slab_bufs) as slabs,
            tc.tile_pool(name="atm", bufs=pipe + 2) as atmp,
            tc.tile_pool(name="ssb", bufs=pipe + 3) as ssbp,
            tc.tile_pool(name="pat", bufs=3, space="PSUM") as patp,
            tc.tile_pool(name="pout", bufs=3, space="PSUM") as poutp,
            tc.tile_pool(name="pst", bufs=(1 if pp else 2), space="PSUM") as pstp,
        ):
            mask_t = constp.tile([C, 2 * C], F32, tag="mask")
            nc.sync.dma_start(mask_t[:], mask2[:])

            fixed = None
            if probe_nodma:
                # one 8-chunk slab-set per pair, loaded once; the loop reuses
                # it for both halves -> compute rhythm with no load DMAs
                fixed = []
                fl = 8
                for p in range(NPC):
                    fqk = constp.tile([D, 4 * fl * C], F16, tag=f"fqk{p}",
                                      name=f"fqk{p}")
                    nc.sync.dma_start(fqk[:], qk[:, 0:4 * fl * C])
                    fkv = constp.tile([C, fl * (2 * D + E1)], F16,
                                      tag=f"fkv{p}", name=f"fkv{p}")
                    nc.sync.dma_start(fkv[:], kv[:, 0:fl * (2 * D + E1)])
                    fixed.append({"qk": fqk, "kv": fkv, "ln": fl})

            for rep in range(repeat):
              with (tc.For_i(0, loop_k, 1, staggered_reset=stag,
                             hint_engines=(
                        mybir.EngineType.PE, mybir.EngineType.DVE,
                        mybir.EngineType.Activation, mybir.EngineType.SP,
                        mybir.EngineType.Pool))
                    if (loop_k is not None and loop_k > 1)
                    else _nullctx()):
                  slab_t = [None] * NPC
                  outs_t = [None]
                  S_sbuf = [None]
                  pS_pp = [None, None]    # ping-pong PSUM state banks
                  S_snap = [None, None]   # latest SBUF snapshot per bank
                  if pp:
                      pS_pp[0] = pstp.tile([D, 2 * NPC * SW], F32, tag="pSX",
                                           name=f"pSX_{rep}")
                      pS_pp[1] = pstp.tile([D, 2 * NPC * SW], F32, tag="pSY",
                                           name=f"pSY_{rep}")

                  fifo = []
                  for cc in range(NCHUNK + pipe):
                    back = None
                    if cc >= pipe and fifo:
                        back = fifo.pop(0)
                    if cc < NCHUNK:
                      c = cc
                      si, c0, ln = slab_of[c]
                      j = c - c0
                      sl = {}
                      for p in range(NPC):
                          if probe_nodma:
                              slab_t[p] = fixed[p]
                              c0, ln = (0 if c < 8 else 8), 8
                              j = c - c0
                          elif c == c0:
                              qbase = qk_base[p][si]
                              kbase = kv_base[p][si]
                              st = {"ln": ln}
                              st["kv"] = slabs.tile([C, KVW_of[si]], KVT, tag="kv",
                                                    name=f"kvd_{rep}_{p}_{c}")
                              nc.sync.dma_start(st["kv"][:],
                                                kv[:, kbase:kbase + KVW_of[si]])
                              st["qk"] = slabs.tile([D, QKW_of[si]], F16, tag="qk",
                                                    name=f"qkd_{rep}_{p}_{c}")
                              nc.sync.dma_start(st["qk"][:],
                                                qk[:, qbase:qbase + QKW_of[si]])
                              slab_t[p] = st
                          st = slab_t[p]
                          SC = st["ln"] * C
                          lnp = st["ln"]
                          if kn8:
                              knc = st["kv"][:, j * D:(j + 1) * D].bitcast(F8)
                              krnc = st["kv"][:, lnp * D + j * D:lnp * D + (j + 1) * D].bitcast(F8)
                              vc = st["kv"][:, 2 * lnp * D + j * 2 * E1:2 * lnp * D + (j + 1) * 2 * E1].bitcast(F16)
                          else:
                              knc = st["kv"][:, j * D:(j + 1) * D]
                              krnc = st["kv"][:, lnp * D + j * D:lnp * D + (j + 1) * D]
                              vc = st["kv"][:, 2 * lnp * D + j * E1:2 * lnp * D + (j + 1) * E1]
                          sl[p] = dict(
                              qcT=st["qk"][:, 0 * SC + j * C:0 * SC + (j + 1) * C],
                              kcT=st["qk"][:, 1 * SC + j * C:1 * SC + (j + 1) * C],
                              qrcT=st["qk"][:, 2 * SC + j * C:2 * SC + (j + 1) * C],
                              krcT=st["qk"][:, 3 * SC + j * C:3 * SC + (j + 1) * C],
                              knc=knc, krnc=krnc, vc=vc)
                      if probe_nodma:
                          ln = 8
                      if c == c0:
                          outs_t[0] = slabs.tile([C, ln * OW], F16, tag="outs",
                                                 name=f"outsd_{rep}_{c}")
                      if dma_only:
                          if j == ln - 1:
                              ocols = slice(c0 * OW, (c0 + ln) * OW)
                              store_eng.dma_start(
                                  out[:, ocols],
                                  slab_t[0]["kv"][:, 0:2 * ln * OW].bitcast(F16)
                                  if kn8 else slab_t[0]["kv"][:, 0:ln * OW])
                          continue

                      if pp:
                          # ping-pong: bank c%2 accumulates in place; the
                          # snapshot is a PLAIN copy (ACT-capable) and stays
                          # valid for two chunks
                          B = c % 2
                          bank = pS_pp[B]
                          prev_S = (None if c == 0 else
                                    (S_snap[1 - B], S_snap[B] if c >= 2 else None))
                          for br in range(2):
                              for p in range(NPC):
                                  z = sl[p]
                                  reg = bank[:, (2 * p + br) * SW:(2 * p + br) * SW + E1]
                                  nc.tensor.matmul(
                                      reg, z["knc"] if br == 0 else z["krnc"],
                                      z["vc"],
                                      start=(c < 2 and br == 0 and p == 0),
                                      stop=(c >= NCHUNK - 2 and br == 1 and p == NPC - 1),
                                      skip_group_check=True)
                          if c < NCHUNK - 1:
                              s01 = ssbp.tile([D, 2 * NPC * SW], F16, tag="ssb",
                                              name=f"s01d_{rep}_{c}")
                              if c % 2 == 0:
                                  nc.scalar.copy(s01[:], bank[:])
                              else:
                                  nc.vector.tensor_copy(s01[:], bank[:])
                              S_snap[B] = s01
                      else:
                          prev_S = S_sbuf[0]

                          # fresh per-chunk outer product (one PSUM group)
                          pSc = pstp.tile([D, 2 * NPC * SW], F32, tag="pS",
                                          name=f"pSd_{rep}_{c}")
                          for br in range(2):
                              for p in range(NPC):
                                  z = sl[p]
                                  reg = pSc[:, (2 * p + br) * SW:(2 * p + br) * SW + E1]
                                  nc.tensor.matmul(
                                      reg, z["knc"] if br == 0 else z["krnc"],
                                      z["vc"],
                                      start=(br == 0 and p == 0),
                                      stop=(br == 1 and p == NPC - 1),
                                      skip_group_check=True)
                          if c < NCHUNK - 1:
                              s01 = ssbp.tile([D, 2 * NPC * SW], F16, tag="ssb",
                                              name=f"s01d_{rep}_{c}")
                              addeng = getattr(nc, add_eng)
                              if prev_S is None:
                                  addeng.tensor_copy(s01[:], pSc[:])
                              else:
                                  addeng.tensor_add(s01[:], pSc[:], prev_S[:])
                              S_sbuf[0] = s01

                      patb = patp.tile([C, 2 * C], F32, tag="pat",
                                       name=f"patd_{rep}_{c}")
                      for br in range(2):
                          for p in range(NPC):
                              z = sl[p]
                              reg = patb[:, p * C:(p + 1) * C]
                              nc.tensor.matmul(
                                  reg, z["kcT"] if br == 0 else z["krcT"],
                                  z["qcT"] if br == 0 else z["qrcT"],
                                  start=(br == 0 and p == 0),
                                  stop=(br == 1 and p == NPC - 1),
                                  skip_group_check=True)
                      atm = atmp.tile([C, 2 * C], F16, tag="atm",
                                      name=f"atmd_{rep}_{c}")
                      nc.vector.tensor_mul(atm[:], patb[:], mask_t[:])

                      fifo.append(dict(atm=atm, sl=sl, c=c, c0=c0, j=j,
                                       ln=ln, prev_S=prev_S, outs=outs_t[0]))

                    if back is not None and not dma_only:
                        cb = back["c"]
                        pob = poutp.tile([C, NPC * PW], F32, tag="po",
                                         name=f"pod_{rep}_{cb}")
                        for p in range(NPC):
                            z = back["sl"][p]
                            reg = pob[:, p * PW:p * PW + E1]
                            nc.tensor.matmul(
                                reg, back["atm"][:, p * C:(p + 1) * C],
                                z["vc"], start=(p == 0),
                                stop=(cb == 0 and p == NPC - 1),
                                skip_group_check=True)
                        if cb > 0:
                            pv = back["prev_S"]
                            if pp:
                                terms = [t for t in pv if t is not None]
                            else:
                                terms = [pv]
                            for ti, term in enumerate(terms):
                                last_t = (ti == len(terms) - 1)
                                for br in range(2):
                                    for p in range(NPC):
                                        z = back["sl"][p]
                                        reg = pob[:, p * PW:p * PW + E1]
                                        nc.tensor.matmul(
                                            reg,
                                            z["qcT"] if br == 0 else z["qrcT"],
                                            term[:, (2 * p + br) * SW:(2 * p + br) * SW + E1],
                                            start=False,
                                            stop=(last_t and br == 1 and p == NPC - 1),
                                            skip_group_check=True)

                        src = pob[:].rearrange("p (g w) -> p g w", g=NPC)[:, :, 0:E1]
                        dst = back["outs"][:, back["j"] * OW:(back["j"] + 1) * OW]
                        dst = dst.rearrange("p (g w) -> p g w", g=NPC)
                        if pp and cb % 2 == 0:
                            nc.vector.tensor_copy(dst, src)
                        else:
                            nc.scalar.copy(dst, src)
                        if back["j"] == back["ln"] - 1:
                            ocols = slice(back["c0"] * OW,
                                          (back["c0"] + back["ln"]) * OW)
                            store_eng.dma_start(out[:, ocols], back["outs"][:])

    nc.compile()
    return nc


# Final tuned configuration (see session notes): fp16 everywhere, fp8-e4m3
# for the state-path k/k_rot, packed per-(pair,slab) DMA blocks with a
# tapered-tail slab plan, SBUF-accumulated scan state, 3-deep back stage,
# stores on the Pool/SWDGE queue.
BEST = dict(plan=[4, 4, 4, 2, 2], slab_bufs=10, pipe=2, kn8=True, pp=True)


def bench_build(loop_k=None, **over):
    kw = dict(BEST)
    kw.update(over)
    return build_kernel16d(loop_k=loop_k, **kw)


def bench_in_maps(q, k, q_rot, k_rot, v):
    in_maps = _prepare_in_maps16b(q, k, q_rot, k_rot, v, vscale=1.0 / 16.0,
                                  plan=BEST["plan"], kn8=BEST["kn8"])
    for m in in_maps:
        msk = m.pop("mask")
        m["mask2"] = np.ascontiguousarray(np.concatenate([msk, msk], axis=1))
    return in_maps


def kernel16d(q, k, q_rot, k_rot, v, horizon=128, slab=8, **run_kwargs):
    q = np.asarray(q)
    k = np.asarray(k)
    q_rot = np.asarray(q_rot)
    k_rot = np.asarray(k_rot)
    v = np.asarray(v)
    b, h, n, d = q.shape
    e = v.shape[-1]
    assert (b * h, n, d, e) == (N_CORES * PAIRS_PER_CORE, N, D, E)

    if "nc16d" not in _cached:
        _cached["nc16d"] = bench_build()
    nc = _cached["nc16d"]

    in_maps = bench_in_maps(q, k, q_rot, k_rot, v)
    res = run_bass_kernel_spmd(nc, in_maps, core_ids=list(range(N_CORES)),
                               **run_kwargs)

    outf = np.empty((b * h, n, e), dtype=np.float32)
    for i in range(N_CORES):
        o = res.results[i]["out"]
        o = o.reshape(C, NCHUNK, PAIRS_PER_CORE, E1).astype(np.float32)
        for p in range(PAIRS_PER_CORE):
            num = o[:, :, p, 0:E].transpose(1, 0, 2).reshape(n, E)
            den = o[:, :, p, E].transpose(1, 0).reshape(n, 1)
            outf[PAIRS_PER_CORE * i + p] = num / den
    if run_kwargs:
        kernel16d.last_results = res
    return outf.reshape(b, h, n, e)


def kernel16c(q, k, q_rot, k_rot, v, horizon=128, slab=8, **run_kwargs):
    q = np.asarray(q)
    k = np.asarray(k)
    q_rot = np.asarray(q_rot)
    k_rot = np.asarray(k_rot)
    v = np.asarray(v)
    b, h, n, d = q.shape
    e = v.shape[-1]
    assert (b * h, n, d, e) == (N_CORES * PAIRS_PER_CORE, N, D, E)

    key = f"nc16c_{slab}"
    if key not in _cached:
        _cached[key] = build_kernel16c(slab=slab)
    nc = _cached[key]

    # v (and the fused ones column) are pre-scaled by 1/16 so the shipped
    # fp16 num|den never overflow (den reaches ~65536 unscaled); the host
    # division num/den cancels the scale exactly.
    in_maps = _prepare_in_maps16b(q, k, q_rot, k_rot, v, slab=slab,
                                  vscale=1.0 / 16.0)
    for m in in_maps:
        msk = m.pop("mask")
        m["mask2"] = np.ascontiguousarray(np.concatenate([msk, msk], axis=1))
    res = run_bass_kernel_spmd(nc, in_maps, core_ids=list(range(N_CORES)),
                               **run_kwargs)

    outf = np.empty((b * h, n, e), dtype=np.float32)
    for i in range(N_CORES):
        o = res.results[i]["out"]  # [C, NCHUNK * 2 * E1] fp16
        o = o.reshape(C, NCHUNK, PAIRS_PER_CORE, E1).astype(np.float32)
        for p in range(PAIRS_PER_CORE):
            num = o[:, :, p, 0:E].transpose(1, 0, 2).reshape(n, E)
            den = o[:, :, p, E].transpose(1, 0).reshape(n, 1)
            outf[PAIRS_PER_CORE * i + p] = num / den
    if run_kwargs:
        kernel16c.last_results = res
    return outf.reshape(b, h, n, e)


def _prepare_in_maps16b(q, k, q_rot, k_rot, v, slab=8, vscale=1.0, plan=None,
                        kn8=False):
    b, h, n, d = q.shape
    e = v.shape[-1]
    nbh = b * h
    if plan is None:
        plan = [slab] * (NCHUNK // slab)
    qf = q.reshape(nbh, n, d)
    kf = k.reshape(nbh, n, d)
    qrf = q_rot.reshape(nbh, n, d)
    krf = k_rot.reshape(nbh, n, d)
    vf = v.reshape(nbh, n, e)
    mask = np.triu(np.ones((C, C), dtype=np.float32))

    def chunk_major(x, nch):
        f = x.shape[-1]
        return x.reshape(nch, C, f).transpose(1, 0, 2).reshape(C, nch * f)

    in_maps = []
    for i in range(N_CORES):
        sel = [PAIRS_PER_CORE * i + p for p in range(PAIRS_PER_CORE)]
        qkblks, kvblks = [], []
        for s in sel:
            qT, kT, qrT, krT = (x[s].T.astype(np.float16)
                                for x in (qf, kf, qrf, krf))
            v1s = (vscale * np.concatenate(
                [vf[s], np.ones((n, 1), vf.dtype)], axis=1)).astype(np.float16)
            kns = kf[s].astype(np.float16)
            krns = krf[s].astype(np.float16)
            c0 = 0
            for ln in plan:
                cs = slice(c0 * C, (c0 + ln) * C)
                c0 += ln
                qkblks.append(np.concatenate(
                    [qT[:, cs], kT[:, cs], qrT[:, cs], krT[:, cs]], axis=1))
                if kn8:
                    import ml_dtypes
                    kn_b = chunk_major(kns[cs], ln).astype(
                        ml_dtypes.float8_e4m3fn).view(np.uint8)
                    krn_b = chunk_major(krns[cs], ln).astype(
                        ml_dtypes.float8_e4m3fn).view(np.uint8)
                    v1_b = chunk_major(v1s[cs], ln).view(np.uint8)
                    kvblks.append(np.concatenate([kn_b, krn_b, v1_b], axis=1))
                else:
                    kvblks.append(np.concatenate(
                        [chunk_major(kns[cs], ln), chunk_major(krns[cs], ln),
                         chunk_major(v1s[cs], ln)], axis=1))
        in_maps.append(dict(
            qk=np.ascontiguousarray(np.concatenate(qkblks, axis=1)),
            kv=np.ascontiguousarray(np.concatenate(kvblks, axis=1)),
            mask=mask))
    return in_maps


def kernel16b(q, k, q_rot, k_rot, v, horizon=128, slab=8, **run_kwargs):
    q = np.asarray(q)
    k = np.asarray(k)
    q_rot = np.asarray(q_rot)
    k_rot = np.asarray(k_rot)
    v = np.asarray(v)
    b, h, n, d = q.shape
    e = v.shape[-1]
    assert (b * h, n, d, e) == (N_CORES * PAIRS_PER_CORE, N, D, E)

    key = f"nc16b_{slab}"
    if key not in _cached:
        _cached[key] = build_kernel16b(slab=slab)
    nc = _cached[key]

    in_maps = _prepare_in_maps16b(q, k, q_rot, k_rot, v, slab=slab)
    res = run_bass_kernel_spmd(nc, in_maps, core_ids=list(range(N_CORES)),
                               **run_kwargs)

    outf = np.empty((b * h, n, e), dtype=np.float32)
    for i in range(N_CORES):
        o = res.results[i]["out"]  # [C, PAIRS*NCHUNK*E] fp16
        o = o.reshape(C, PAIRS_PER_CORE, NCHUNK, E).astype(np.float32)
        for p in range(PAIRS_PER_CORE):
            outf[PAIRS_PER_CORE * i + p] = o[:, p].transpose(1, 0, 2).reshape(n, e)
    if run_kwargs:
        kernel16b.last_results = res
    return outf.reshape(b, h, n, e)


def _prepare_in_maps16(q, k, q_rot, k_rot, v):
    b, h, n, d = q.shape
    e = v.shape[-1]
    nbh = b * h
    qf = q.reshape(nbh, n, d)
    kf = k.reshape(nbh, n, d)
    qrf = q_rot.reshape(nbh, n, d)
    krf = k_rot.reshape(nbh, n, d)
    vf = v.reshape(nbh, n, e)
    mask = np.triu(np.ones((C, C), dtype=np.float32))

    def chunk_major(x):
        # [n, f] -> [C, NCHUNK * f]: column-major-by-chunk on-chip layout
        f = x.shape[-1]
        return x.reshape(NCHUNK, C, f).transpose(1, 0, 2).reshape(C, NCHUNK * f)

    in_maps = []
    for i in range(N_CORES):
        sel = [PAIRS_PER_CORE * i + p for p in range(PAIRS_PER_CORE)]
        qT = np.concatenate([qf[s].T for s in sel], axis=1).astype(np.float16)
        kT = np.concatenate([kf[s].T for s in sel], axis=1).astype(np.float16)
        qrT = np.concatenate([qrf[s].T for s in sel], axis=1).astype(np.float16)
        krT = np.concatenate([krf[s].T for s in sel], axis=1).astype(np.float16)
        kn = np.concatenate([chunk_major(kf[s]) for s in sel], axis=1).astype(np.float16)
        krn = np.concatenate([chunk_major(krf[s]) for s in sel], axis=1).astype(np.float16)
        v1 = np.concatenate(
            [chunk_major(np.concatenate(
                [vf[s], np.ones((n, 1), vf.dtype)], axis=1)) for s in sel],
            axis=1).astype(np.float16)
        in_maps.append(dict(qT=np.ascontiguousarray(qT),
                            kT=np.ascontiguousarray(kT),
                            qrT=np.ascontiguousarray(qrT),
                            krT=np.ascontiguousarray(krT),
                            kn=np.ascontiguousarray(kn),
                            krn=np.ascontiguousarray(krn),
                            v1=np.ascontiguousarray(v1),
                            mask=mask))
    return in_maps


def kernel16(q, k, q_rot, k_rot, v, horizon=128, **run_kwargs):
    q = np.asarray(q)
    k = np.asarray(k)
    q_rot = np.asarray(q_rot)
    k_rot = np.asarray(k_rot)
    v = np.asarray(v)
    b, h, n, d = q.shape
    e = v.shape[-1]
    assert (b * h, n, d, e) == (N_CORES * PAIRS_PER_CORE, N, D, E)

    if "nc16" not in _cached:
        _cached["nc16"] = build_kernel16()
    nc = _cached["nc16"]

    in_maps = _prepare_in_maps16(q, k, q_rot, k_rot, v)
    res = run_bass_kernel_spmd(nc, in_maps, core_ids=list(range(N_CORES)),
                               **run_kwargs)

    outf = np.empty((b * h, n, e), dtype=np.float32)
    for i in range(N_CORES):
        o = res.results[i]["out"]  # [C, PAIRS*NCHUNK*E] fp16
        o = o.reshape(C, PAIRS_PER_CORE, NCHUNK, E).astype(np.float32)
        for p in range(PAIRS_PER_CORE):
            outf[PAIRS_PER_CORE * i + p] = o[:, p].transpose(1, 0, 2).reshape(n, e)
    if run_kwargs:
        kernel16.last_results = res
    return outf.reshape(b, h, n, e)


# Column strides inside shared PSUM banks (8-byte aligned regions)
PW = 72            # per-pair region width in the output bank (>= E1)
SW = 66            # per-(pair,branch) region width in the state bank (>= E1)


def build_kernel_m(repeat=1, loop_k=None):
    """Pair-merged variant: both (b,h) pairs handled per core share single
    PSUM banks for AT, numerator/denominator, and state, so the causal mask,
    the state evacuation, and the reciprocal each run as ONE wide
    vector/scalar op per chunk instead of one per pair. Cuts the DVE/ACT
    instruction count (and their fixed per-op drain cost) roughly in half."""
    nc = bacc.Bacc("TRN2", target_bir_lowering=False, debug=False,
                   num_devices=N_CORES)

    MT = F32  # typed-f32r rejected by walrus codegen (odd-N ISA check)

    def mm(out_ap, lhsT_ap, rhs_ap, **kw):
        if mm_f32r:
            lhsT_ap = lhsT_ap.bitcast(F32R)
            rhs_ap = rhs_ap.bitcast(F32R)
        return nc.tensor.matmul(out_ap, lhsT_ap, rhs_ap, **kw)

    qT = nc.dram_tensor("qT", [D, NROWS], MT, kind="ExternalInput").ap()
    kT = nc.dram_tensor("kT", [D, NROWS], MT, kind="ExternalInput").ap()
    qrT = nc.dram_tensor("qrT", [D, NROWS], MT, kind="ExternalInput").ap()
    krT = nc.dram_tensor("krT", [D, NROWS], MT, kind="ExternalInput").ap()
    kn = nc.dram_tensor("kn", [NROWS, D], MT, kind="ExternalInput").ap()
    krn = nc.dram_tensor("krn", [NROWS, D], MT, kind="ExternalInput").ap()
    v1 = nc.dram_tensor("v1", [NROWS, E1], MT, kind="ExternalInput").ap()
    mask2 = nc.dram_tensor("mask2", [C, 2 * C], F32, kind="ExternalInput").ap()
    out = nc.dram_tensor("out", [NROWS, E], F32, kind="ExternalOutput").ap()

    NP = PAIRS_PER_CORE  # 2

    with tile.TileContext(nc) as tc:
        with (
            tc.tile_pool(name="const", bufs=1) as constp,
            tc.tile_pool(name="slabs", bufs=6) as slabs,
            tc.tile_pool(name="atm", bufs=3) as atmp,
            tc.tile_pool(name="ssb", bufs=4) as ssbp,
            tc.tile_pool(name="dinv", bufs=8) as dinvp,
            tc.tile_pool(name="pat", bufs=3, space="PSUM") as patp,
            tc.tile_pool(name="pout", bufs=3, space="PSUM") as poutp,
            tc.tile_pool(name="pst", bufs=1, space="PSUM") as pstp,
        ):
            mask_t = constp.tile([C, 2 * C], F32, tag="mask")
            nc.sync.dma_start(mask_t[:], mask2[:])

            fixed = None
            if probe_nodma:
                # one 8-chunk slab-set per pair, loaded once; the loop reuses
                # it for both halves -> compute rhythm with no load DMAs
                fixed = []
                fl = 8
                for p in range(NPC):
                    fqk = constp.tile([D, 4 * fl * C], F16, tag=f"fqk{p}",
                                      name=f"fqk{p}")
                    nc.sync.dma_start(fqk[:], qk[:, 0:4 * fl * C])
                    fkv = constp.tile([C, fl * (2 * D + E1)], F16,
                                      tag=f"fkv{p}", name=f"fkv{p}")
                    nc.sync.dma_start(fkv[:], kv[:, 0:fl * (2 * D + E1)])
                    fixed.append({"qk": fqk, "kv": fkv, "ln": fl})

            for rep in range(repeat):
              with (tc.For_i(0, loop_k, 1, hint_engines=(
                        mybir.EngineType.PE, mybir.EngineType.DVE,
                        mybir.EngineType.Activation, mybir.EngineType.SP))
                    if (loop_k is not None and loop_k > 1)
                    else _nullctx()):
                  # one state bank: region (p, br) at cols (2p+br)*SW
                  pSt = pstp.tile([D, 2 * NP * SW], F32, tag="pS",
                                  name=f"pSm_{rep}")

                  slab_t = [None] * NP
                  S_sbuf = [None]     # boxed: current [D, 4*SW] sbuf state

                  pending = None
                  for cc in range(NCHUNK + 1):
                    back = pending
                    pending = None
                    if cc < NCHUNK:
                      c = cc
                      sl = {}
                      for p in range(NP):
                          if c % SLAB == 0:
                              base = p * N + c * C
                              cols = slice(base, base + SLAB * C)
                              st = {}
                              st["qT"] = slabs.tile([D, slen * C], F32, tag="qT", name=f"qTs_{rep}_{p}_{c}")
                              nc.sync.dma_start(st["qT"][:], qT[:, cols])
                              st["kT"] = slabs.tile([D, slen * C], F32, tag="kT", name=f"kTs_{rep}_{p}_{c}")
                              nc.sync.dma_start(st["kT"][:], kT[:, cols])
                              st["qrT"] = slabs.tile([D, slen * C], F32, tag="qrT", name=f"qrTs_{rep}_{p}_{c}")
                              nc.sync.dma_start(st["qrT"][:], qrT[:, cols])
                              st["krT"] = slabs.tile([D, slen * C], F32, tag="krT", name=f"krTs_{rep}_{p}_{c}")
                              nc.sync.dma_start(st["krT"][:], krT[:, cols])
                              st["kn"] = slabs.tile([C, slen, D], F32, tag="kn", name=f"kns_{rep}_{p}_{c}")
                              nc.sync.dma_start(
                                  st["kn"][:],
                                  kn[cols, :].rearrange("(n p) d -> p n d", p=C))
                              st["krn"] = slabs.tile([C, slen, D], F32, tag="krn", name=f"krns_{rep}_{p}_{c}")
                              nc.sync.dma_start(
                                  st["krn"][:],
                                  krn[cols, :].rearrange("(n p) d -> p n d", p=C))
                              st["v1"] = slabs.tile([C, slen, E1], F32, tag="v1", name=f"v1s_{rep}_{p}_{c}")
                              nc.sync.dma_start(
                                  st["v1"][:],
                                  v1[cols, :].rearrange("(n p) e -> p n e", p=C))
                              st["outs"] = slabs.tile([C, SLAB, E], F32, tag="outs", name=f"outs_{rep}_{p}_{c}")
                              slab_t[p] = st

                          st = slab_t[p]
                          j = c - c0
                          sl[p] = dict(
                              st=st, j=j,
                              qcT=st["qT"][:, j * C:(j + 1) * C],
                              kcT=st["kT"][:, j * C:(j + 1) * C],
                              qrcT=st["qrT"][:, j * C:(j + 1) * C],
                              krcT=st["krT"][:, j * C:(j + 1) * C],
                              knc=st["kn"][:, j, :],
                              krnc=st["krn"][:, j, :],
                              vc=st["v1"][:, j, :],
                          )

                      prev_S = S_sbuf[0]

                      # state updates, all four into one bank
                      for p in range(NP):
                          z = sl[p]
                          nc.tensor.matmul(
                              pSt[:, (2 * p) * SW:(2 * p) * SW + E1],
                              z["knc"], z["vc"],
                              start=(c == 0 and p == 0), stop=False,
                              skip_group_check=True)
                          nc.tensor.matmul(
                              pSt[:, (2 * p + 1) * SW:(2 * p + 1) * SW + E1],
                              z["krnc"], z["vc"],
                              start=False,
                              stop=(c == NCHUNK - 1 and p == NP - 1),
                              skip_group_check=True)
                      if c < NCHUNK - 1:
                          s01 = ssbp.tile([D, 2 * NP * SW], F32, tag="ssb")
                          nc.scalar.copy(s01[:], pSt[:])
                          S_sbuf[0] = s01

                      # AT for both pairs into one bank, one mask op
                      patb = patp.tile([C, 2 * C], F32, tag="pat")
                      for p in range(NP):
                          z = sl[p]
                          reg = patb[:, p * C:(p + 1) * C]
                          nc.tensor.matmul(reg, z["kcT"], z["qcT"],
                                           start=True, stop=False,
                                           skip_group_check=True)
                          nc.tensor.matmul(reg, z["krcT"], z["qrcT"],
                                           start=False, stop=True,
                                           skip_group_check=True)
                      atm = atmp.tile([C, 2 * C], F32, tag="atm")
                      nc.vector.tensor_mul(atm[:], patb[:], mask_t[:])

                      pending = dict(atm=atm, sl=sl, c=c, prev_S=prev_S)

                    if back is not None:
                        cb = back["c"]
                        pob = poutp.tile([C, NP * PW], F32, tag="po")
                        for p in range(NP):
                            z = back["sl"][p]
                            reg = pob[:, p * PW:p * PW + E1]
                            only = (cb == 0)
                            nc.tensor.matmul(
                                reg, back["atm"][:, p * C:(p + 1) * C],
                                z["vc"], start=True, stop=only,
                                skip_group_check=True)
                            if cb > 0:
                                pv = back["prev_S"]
                                nc.tensor.matmul(
                                    reg, z["qcT"],
                                    pv[:, (2 * p) * SW:(2 * p) * SW + E1],
                                    start=False, stop=False,
                                    skip_group_check=True)
                                nc.tensor.matmul(
                                    reg, z["qrcT"],
                                    pv[:, (2 * p + 1) * SW:(2 * p + 1) * SW + E1],
                                    start=False, stop=True,
                                    skip_group_check=True)

                        # one reciprocal for both pairs' denominators
                        dinv = dinvp.tile([C, NP], F32, tag="dinv")
                        nc.vector.reciprocal(
                            dinv[:], pob[:, E:NP * PW:PW])
                        for p in range(NP):
                            z = back["sl"][p]
                            nc.scalar.mul(z["st"]["outs"][:, z["j"], :],
                                          pob[:, p * PW:p * PW + E],
                                          dinv[:, p:p + 1])
                            if z["j"] == SLAB - 1:
                                base = p * N + (cb - SLAB + 1) * C
                                rows = slice(base, base + SLAB * C)
                                nc.sync.dma_start(
                                    out[rows, :].rearrange(
                                        "(n p) e -> p n e", p=C),
                                    z["st"]["outs"][:])

    nc.compile()
    return nc



def _prepare_in_maps(q, k, q_rot, k_rot, v, transpose_k=False, merged=False):
    b, h, n, d = q.shape
    e = v.shape[-1]
    nbh = b * h
    qf = np.ascontiguousarray(q.reshape(nbh, n, d).astype(np.float32))
    kf = np.ascontiguousarray(k.reshape(nbh, n, d).astype(np.float32))
    qrf = np.ascontiguousarray(q_rot.reshape(nbh, n, d).astype(np.float32))
    krf = np.ascontiguousarray(k_rot.reshape(nbh, n, d).astype(np.float32))
    vf = np.ascontiguousarray(v.reshape(nbh, n, e).astype(np.float32))
    mask = np.triu(np.ones((C, C), dtype=np.float32))

    in_maps = []
    for i in range(N_CORES):
        sel = [PAIRS_PER_CORE * i + p for p in range(PAIRS_PER_CORE)]
        qT = np.ascontiguousarray(
            np.concatenate([qf[s].T for s in sel], axis=1))
        kT = np.ascontiguousarray(
            np.concatenate([kf[s].T for s in sel], axis=1))
        qrT = np.ascontiguousarray(
            np.concatenate([qrf[s].T for s in sel], axis=1))
        krT = np.ascontiguousarray(
            np.concatenate([krf[s].T for s in sel], axis=1))
        knat = np.ascontiguousarray(np.concatenate([kf[s] for s in sel], axis=0))
        krnat = np.ascontiguousarray(np.concatenate([krf[s] for s in sel], axis=0))
        vcat = np.concatenate([vf[s] for s in sel], axis=0)
        v1 = np.ascontiguousarray(
            np.concatenate([vcat, np.ones((vcat.shape[0], 1), np.float32)],
                           axis=1))
        m = dict(qT=qT, kT=kT, qrT=qrT, krT=krT, v1=v1)
        if merged:
            m["mask2"] = np.ascontiguousarray(np.concatenate([mask, mask], axis=1))
        else:
            m["mask"] = mask
        if transpose_k:
            m["ident"] = np.eye(C, dtype=np.float32)
        else:
            m["kn"] = knat
            m["krn"] = krnat
        in_maps.append(m)
    return in_maps


def kernel_f32(q, k, q_rot, k_rot, v, horizon=128, **run_kwargs):
    q = np.asarray(q)
    k = np.asarray(k)
    q_rot = np.asarray(q_rot)
    k_rot = np.asarray(k_rot)
    v = np.asarray(v)
    b, h, n, d = q.shape
    e = v.shape[-1]
    assert (b * h, n, d, e) == (N_CORES * PAIRS_PER_CORE, N, D, E), \
        "kernel is hardcoded for b*h=16, n=2048, d=128, e=64"

    if "nc" not in _cached:
        _cached["nc"] = build_kernel()
    nc = _cached["nc"]

    in_maps = _prepare_in_maps(q, k, q_rot, k_rot, v)
    res = run_bass_kernel_spmd(nc, in_maps, core_ids=list(range(N_CORES)),
                               **run_kwargs)

    outf = np.empty((b * h, n, e), dtype=np.float32)
    for i in range(N_CORES):
        o = res.results[i]["out"].reshape(PAIRS_PER_CORE, n, e)
        for p in range(PAIRS_PER_CORE):
            outf[PAIRS_PER_CORE * i + p] = o[p]
    if run_kwargs:
        kernel_f32.last_results = res
    return outf.reshape(b, h, n, e)


def kernel(q, k, q_rot, k_rot, v, horizon=128, **run_kwargs):
    return kernel16d(q, k, q_rot, k_rot, v, horizon, **run_kwargs)


if __name__ == "__main__":
    rng = np.random.default_rng(0)
    q = rng.random((2, 8, N, D), dtype=np.float32)
    k = rng.random((2, 8, N, D), dtype=np.float32)
    qr = rng.standard_normal((2, 8, N, D), dtype=np.float32)
    kr = rng.standard_normal((2, 8, N, D), dtype=np.float32)
    v = rng.random((2, 8, N, E), dtype=np.float32)
    o = kernel(q, k, qr, kr, v, 128)
    print("ok", o.shape, o.dtype, np.abs(o).mean())



# revision 24
# speedup vs baseline: 2.9170x; 1.0182x over previous
"""Trainium2 Bass kernel for chunked recurrent causal linear attention.

Problem: b=2, h=8, n=2048, d=128, e=64, chunk=128, two branches (plain +
rotary) sharing one denominator.

Math (per (b,h), per chunk c, token t in chunk, with running state
S[d,e], Z[d] per branch):
    AT[s,t]   = k_s . q_t                  (s,t in chunk; masked to s<=t)
    num[t,:]  = sum_s ATm[s,t] v_s + q_t @ S      (both branches summed)
    den[t]    = sum_s ATm[s,t]   + q_t . Z        (both branches summed)
    out[t,:]  = num[t,:] / den[t]
    S += k_chunk^T v_chunk ;  Z += sum_s k_s

Sharding: 16 (b,h) pairs over 8 cores, 2 pairs per core. Host ships
pre-transposed copies of q/k/q_rot/k_rot (so no on-device transposes are
needed) plus natural-layout k/k_rot (stationary operand of the state
update) and v with a ones-column appended (fuses the denominator into
the numerator matmuls).
"""

import contextlib
import sys

_nullctx = contextlib.nullcontext

if "/opt/trn_rl_repo" not in sys.path:
    sys.path.insert(0, "/opt/trn_rl_repo")

import numpy as np

import concourse.bass as bass
import concourse.tile as tile
from concourse import bacc, mybir
from concourse.bass_utils import run_bass_kernel_spmd

F32 = mybir.dt.float32
F32R = mybir.dt.float32r

N_CORES = 8
PAIRS_PER_CORE = 2
N = 2048           # sequence length per (b,h)
D = 128            # qk head dim
E = 64             # v head dim
E1 = E + 1         # v plus ones column
C = 128            # chunk size
NCHUNK = N // C    # 16
SLAB = 4           # chunks per DMA slab
SLAB_BUFS = 6      # slab pool buffers
NROWS = PAIRS_PER_CORE * N  # 4096

_cached = {}


def build_kernel(repeat=1, loop_k=None, dma_only=False, reuse_slab=False,
                 probe_no_at=False, probe_no_state=False, transpose_k=False,
                 pipe=1, host_norm=False, dma_split=False, taper=False,
                 big_bufs=False, load_reorder=False, bank_42=False,
                 stagger=False, probe_pe_only=False, mm_f32r=False,
                 f32r=False, fast_start=False, ilv=True):
    nc = bacc.Bacc("TRN2", target_bir_lowering=False, debug=False,
                   num_devices=N_CORES)

    MT = F32  # typed-f32r rejected by walrus codegen (odd-N ISA check)

    def mm(out_ap, lhsT_ap, rhs_ap, **kw):
        if mm_f32r:
            lhsT_ap = lhsT_ap.bitcast(F32R)
            rhs_ap = rhs_ap.bitcast(F32R)
        return nc.tensor.matmul(out_ap, lhsT_ap, rhs_ap, **kw)

    qT = nc.dram_tensor("qT", [D, NROWS], MT, kind="ExternalInput").ap()
    kT = nc.dram_tensor("kT", [D, NROWS], MT, kind="ExternalInput").ap()
    qrT = nc.dram_tensor("qrT", [D, NROWS], MT, kind="ExternalInput").ap()
    krT = nc.dram_tensor("krT", [D, NROWS], MT, kind="ExternalInput").ap()
    if not transpose_k:
        kn = nc.dram_tensor("kn", [NROWS, D], MT, kind="ExternalInput").ap()
        krn = nc.dram_tensor("krn", [NROWS, D], MT, kind="ExternalInput").ap()
    else:
        ident = nc.dram_tensor("ident", [C, C], F32, kind="ExternalInput").ap()
    v1 = nc.dram_tensor("v1", [NROWS, E1], MT, kind="ExternalInput").ap()
    mask = nc.dram_tensor("mask", [C, C], F32, kind="ExternalInput").ap()
    EO = E1 if host_norm else E
    out = nc.dram_tensor("out", [NROWS, EO], F32, kind="ExternalOutput").ap()

    if taper:
        plans = [{0: 2, 2: 4, 6: 4, 10: 4, 14: 2}] * PAIRS_PER_CORE
    elif stagger:
        plans = [{c0: SLAB for c0 in range(0, NCHUNK, SLAB)},
                 {0: 2, 2: 4, 6: 4, 10: 4, 14: 2}]
    else:
        plans = [{c0: SLAB for c0 in range(0, NCHUNK, SLAB)}] * PAIRS_PER_CORE
    slab_of = []
    for pp in range(PAIRS_PER_CORE):
        m = {}
        for c0, ln in plans[pp].items():
            for c in range(c0, c0 + ln):
                m[c] = (c0, ln)
        slab_of.append(m)

    with tile.TileContext(nc) as tc:
        with (
            tc.tile_pool(name="const", bufs=1) as constp,
            tc.tile_pool(name="slabs", bufs=SLAB_BUFS) as slabs,
            tc.tile_pool(name="atm", bufs=(6 if big_bufs else (4 if pipe == 1 else 6))) as atmp,
            tc.tile_pool(name="ssb", bufs=(12 if big_bufs else 8)) as ssbp,
            tc.tile_pool(name="dinv", bufs=(12 if big_bufs else 8)) as dinvp,
            tc.tile_pool(name="pat", bufs=(2 if (transpose_k or bank_42) else 3),
                         space="PSUM") as patp,
            tc.tile_pool(name="pout", bufs=(2 if transpose_k else (4 if bank_42 else 3)),
                         space="PSUM") as poutp,
            tc.tile_pool(name="pst", bufs=2, space="PSUM") as pstp,
            tc.tile_pool(name="ktr", bufs=2, space="PSUM") as ktrp,
            tc.tile_pool(name="kns", bufs=4) as knsp,
        ):
            mask_t = constp.tile([C, C], F32, tag="mask")
            nc.sync.dma_start(mask_t[:], mask[:])
            if transpose_k:
                ident_t = constp.tile([C, C], F32, tag="ident")
                nc.sync.dma_start(ident_t[:], ident[:])

            for rep in range(repeat):
              with (tc.For_i(0, loop_k, 1, hint_engines=(
                        mybir.EngineType.PE, mybir.EngineType.DVE,
                        mybir.EngineType.Activation, mybir.EngineType.SP))
                    if (loop_k is not None and loop_k > 1)
                    else _nullctx()):
                  # per-pair state accumulator in one PSUM bank:
                  # cols 0:65 -> branch 0 [S|Z], cols 66:131 -> branch 1
                  pS = {}
                  for p in range(PAIRS_PER_CORE):
                      pS[p] = pstp.tile([D, 2 * E1 + 2], F32, tag="pS", name=f"pS_{rep}_{p}")

                  slab_t = [None] * PAIRS_PER_CORE   # per pair: dict of slab tiles
                  S_sbuf = {}                        # (p, br) -> sbuf state tile

                  # Software pipeline, one chunk deep: the "front" stage of
                  # chunk c emits loads, the state update (PE), and AT+mask
                  # (PE then DVE); the "back" stage consumes chunk c-1's
                  # masked AT for the numerator/denominator matmuls. This
                  # gives every cross-engine hop a full stage of slack, so
                  # the PE never head-of-line blocks on DVE/ACT latency.
                  fifo = []
                  for cc in range(NCHUNK + pipe):
                    pending = {}
                    back = {}
                    if cc >= pipe:
                        back = fifo.pop(0)
                    if cc < NCHUNK:
                        fifo.append(pending)
                    if cc < NCHUNK:
                      c = cc
                      for p in range(PAIRS_PER_CORE):
                          c0, slen = slab_of[p][c]
                          if (c == c0) and not (reuse_slab and c > 0):
                              base = p * N + c * C
                              cols = slice(base, base + slen * C)
                              dmae = nc.gpsimd if (dma_split and p == 1) else nc.sync
                              st = {"len": slen}
                              if not load_reorder:
                                  st["qT"] = slabs.tile([D, slen * C], MT, tag="qT", name=f"qTs_{rep}_{p}_{c}")
                                  st["kT"] = slabs.tile([D, slen * C], MT, tag="kT", name=f"kTs_{rep}_{p}_{c}")
                                  if fast_start and c == 0:
                                      # split the very first q/k loads so chunk
                                      # 0's AT matmul starts after 128KB, not
                                      # a full slab (range-level tile deps)
                                      dmae.dma_start(st["qT"][:, 0:C], qT[:, base:base + C])
                                      dmae.dma_start(st["kT"][:, 0:C], kT[:, base:base + C])
                                      dmae.dma_start(st["qT"][:, C:slen * C], qT[:, base + C:base + slen * C])
                                      dmae.dma_start(st["kT"][:, C:slen * C], kT[:, base + C:base + slen * C])
                                  else:
                                      dmae.dma_start(st["qT"][:], qT[:, cols])
                                      dmae.dma_start(st["kT"][:], kT[:, cols])
                                  st["qrT"] = slabs.tile([D, slen * C], MT, tag="qrT", name=f"qrTs_{rep}_{p}_{c}")
                                  dmae.dma_start(st["qrT"][:], qrT[:, cols])
                                  st["krT"] = slabs.tile([D, slen * C], MT, tag="krT", name=f"krTs_{rep}_{p}_{c}")
                                  dmae.dma_start(st["krT"][:], krT[:, cols])
                              # load the state-update inputs (kn/krn/v1)
                              # first: they feed the first PE ops of the chunk
                              if not transpose_k:
                                  st["kn"] = slabs.tile([C, slen, D], MT, tag="kn", name=f"kns_{rep}_{p}_{c}")
                                  dmae.dma_start(
                                      st["kn"][:],
                                      kn[cols, :].rearrange("(n p) d -> p n d", p=C))
                                  st["krn"] = slabs.tile([C, slen, D], MT, tag="krn", name=f"krns_{rep}_{p}_{c}")
                                  dmae.dma_start(
                                      st["krn"][:],
                                      krn[cols, :].rearrange("(n p) d -> p n d", p=C))
                              st["v1"] = slabs.tile([C, slen, E1], MT, tag="v1", name=f"v1s_{rep}_{p}_{c}")
                              dmae.dma_start(
                                  st["v1"][:],
                                  v1[cols, :].rearrange("(n p) e -> p n e", p=C))
                              if load_reorder:
                                  st["kT"] = slabs.tile([D, slen * C], MT, tag="kT", name=f"kTs_{rep}_{p}_{c}")
                                  dmae.dma_start(st["kT"][:], kT[:, cols])
                                  st["qT"] = slabs.tile([D, slen * C], MT, tag="qT", name=f"qTs_{rep}_{p}_{c}")
                                  dmae.dma_start(st["qT"][:], qT[:, cols])
                                  st["qrT"] = slabs.tile([D, slen * C], MT, tag="qrT", name=f"qrTs_{rep}_{p}_{c}")
                                  dmae.dma_start(st["qrT"][:], qrT[:, cols])
                                  st["krT"] = slabs.tile([D, slen * C], MT, tag="krT", name=f"krTs_{rep}_{p}_{c}")
                                  dmae.dma_start(st["krT"][:], krT[:, cols])
                              st["outs"] = slabs.tile([C, slen, EO], F32, tag="outs", name=f"outs_{rep}_{p}_{c}")
                              slab_t[p] = st

                          st = slab_t[p]
                          j = c - c0
                          qcT = st["qT"][:, j * C:(j + 1) * C]
                          kcT = st["kT"][:, j * C:(j + 1) * C]
                          qrcT = st["qrT"][:, j * C:(j + 1) * C]
                          krcT = st["krT"][:, j * C:(j + 1) * C]
                          vc = st["v1"][:, j, :]
                          knc = krnc = None
                          if not transpose_k:
                              knc = st["kn"][:, j, :]
                              krnc = st["krn"][:, j, :]

                          if dma_only:
                              continue

                          if probe_pe_only:
                              # pure matmul throughput probe: same 7 MMs as the
                              # real kernel, but no cross-engine deps at all
                              pat0 = patp.tile([C, C], F32, tag="pat")
                              mm(pat0[:], kcT, qcT, start=True, stop=False)
                              mm(pat0[:], krcT, qrcT, start=False, stop=True)
                              po = poutp.tile([C, E1], F32, tag="po")
                              mm(po[:], mask_t[:], vc, start=True, stop=False)
                              mm(po[:], qcT, mask_t[:, 0:E1], start=False, stop=False, skip_group_check=True)
                              mm(po[:], qrcT, mask_t[:, 0:E1], start=False, stop=True, skip_group_check=True)
                              mm(pS[p][:, 0:E1], knc, vc, start=(c == 0), stop=False, skip_group_check=True)
                              mm(pS[p][:, E1 + 1:2 * E1 + 1], krnc, vc, start=False, stop=(c == NCHUNK - 1), skip_group_check=True)
                              continue

                          prev_S = S_sbuf.get(p)

                          if ilv:
                              # MMs emitted pair-interleaved after this loop
                              pending[p] = dict(qcT=qcT, qrcT=qrcT, kcT=kcT,
                                                krcT=krcT, knc=knc, krnc=krnc,
                                                vc=vc, st=st, j=j, c=c,
                                                prev_S=prev_S, kns0=None,
                                                kns1=None, c0=c0,
                                                slen=st.get("len", SLAB))
                              continue

                          # State update: both branches share one PSUM bank
                          # (start=True on c0/br0 clears it; br1 overwrites its
                          # unwritten columns). Without transpose_k the natural-
                          # layout k arrives by DMA and the update is emitted
                          # here (front stage); with transpose_k the k tiles are
                          # transposed on the PE this stage and the state update
                          # moves to the back stage so the transpose->copy->
                          # matmul chain gets a stage of slack.
                          kns0 = kns1 = None
                          if transpose_k and not probe_no_state:
                              ktp0 = ktrp.tile([C, C], F32, tag="ktr")
                              nc.tensor.transpose(ktp0[:], kcT, ident_t[:])
                              kns0 = knsp.tile([C, C], F32, tag="kns")
                              nc.vector.tensor_copy(kns0[:], ktp0[:])
                              ktp1 = ktrp.tile([C, C], F32, tag="ktr")
                              nc.tensor.transpose(ktp1[:], krcT, ident_t[:])
                              kns1 = knsp.tile([C, C], F32, tag="kns")
                              nc.scalar.copy(kns1[:], ktp1[:])
                          if not transpose_k and not probe_no_state:
                              mm(pS[p][:, 0:E1], knc, vc,
                                               start=(c == 0), stop=False,
                                               skip_group_check=True)
                              mm(pS[p][:, E1 + 1:2 * E1 + 1], krnc, vc,
                                               start=False, stop=(c == NCHUNK - 1),
                                               skip_group_check=True)
                              if c < NCHUNK - 1:
                                  s01 = ssbp.tile([D, 2 * E1 + 2], MT, tag="ssb")
                                  nc.scalar.copy(s01[:], pS[p][:])
                                  S_sbuf[p] = s01

                          # AT = K0 Q0^T + K1 Q1^T (both branches accumulate in
                          # one PSUM bank), then one causal mask (s<=t)
                          if probe_no_at:
                              atm0 = mask_t
                          else:
                              pat0 = patp.tile([C, C], F32, tag="pat")
                              if f32r:
                                  mm(pat0[:], kcT.bitcast(F32R),
                                     qcT.bitcast(F32R), start=True, stop=False)
                                  mm(pat0[:], krcT.bitcast(F32R),
                                     qrcT.bitcast(F32R), start=False, stop=True)
                              else:
                                  mm(pat0[:], kcT, qcT, start=True, stop=False)
                                  mm(pat0[:], krcT, qrcT, start=False, stop=True)
                              atm0 = atmp.tile([C, C], MT, tag="atm")
                              nc.vector.tensor_mul(atm0[:], pat0[:], mask_t[:])

                          pending[p] = dict(atm=atm0, qcT=qcT, qrcT=qrcT,
                                            vc=vc, st=st, j=j, c=c,
                                            prev_S=prev_S, kns0=kns0, kns1=kns1,
                                            c0=c0, slen=st.get("len", SLAB))

                    if ilv and cc < NCHUNK and not dma_only and not probe_pe_only:
                        ps = sorted(pending.keys())
                        # state matmuls, pair-interleaved (consecutive MMs hit
                        # different PSUM banks)
                        for br in range(2):
                            for p in ps:
                                z = pending[p]
                                if br == 0:
                                    mm(pS[p][:, 0:E1], z["knc"], z["vc"],
                                       start=(c == 0), stop=False,
                                       skip_group_check=True)
                                else:
                                    mm(pS[p][:, E1 + 1:2 * E1 + 1], z["krnc"],
                                       z["vc"], start=False,
                                       stop=(c == NCHUNK - 1),
                                       skip_group_check=True)
                        for p in ps:
                            if c < NCHUNK - 1:
                                s01 = ssbp.tile([D, 2 * E1 + 2], MT, tag="ssb",
                                                name=f"s01i_{rep}_{p}_{c}")
                                nc.scalar.copy(s01[:], pS[p][:])
                                S_sbuf[p] = s01
                        pats = {}
                        for p in ps:
                            pats[p] = patp.tile([C, C], F32, tag="pat",
                                                name=f"pati_{rep}_{p}_{c}")
                        for br in range(2):
                            for p in ps:
                                z = pending[p]
                                if br == 0:
                                    mm(pats[p][:], z["kcT"], z["qcT"],
                                       start=True, stop=False,
                                       skip_group_check=True)
                                else:
                                    mm(pats[p][:], z["krcT"], z["qrcT"],
                                       start=False, stop=True,
                                       skip_group_check=True)
                        for p in ps:
                            atm = atmp.tile([C, C], MT, tag="atm",
                                            name=f"atmi_{rep}_{p}_{c}")
                            nc.vector.tensor_mul(atm[:], pats[p][:], mask_t[:])
                            pending[p]["atm"] = atm

                    if ilv:
                        items = sorted(back.items())
                        pos = {}
                        for p, z in items:
                            pos[p] = poutp.tile([C, E1], F32, tag="po",
                                                name=f"poi_{rep}_{p}_{z['c']}")
                        for p, z in items:
                            mm(pos[p][:], z["atm"][:], z["vc"], start=True,
                               stop=(z["c"] == 0 or z["prev_S"] is None),
                               skip_group_check=True)
                        for p, z in items:
                            if z["c"] > 0 and z["prev_S"] is not None:
                                mm(pos[p][:], z["qcT"], z["prev_S"][:, 0:E1],
                                   start=False, stop=False,
                                   skip_group_check=True)
                        for p, z in items:
                            if z["c"] > 0 and z["prev_S"] is not None:
                                mm(pos[p][:], z["qrcT"],
                                   z["prev_S"][:, E1 + 1:2 * E1 + 1],
                                   start=False, stop=True,
                                   skip_group_check=True)
                        for p, z in items:
                            po = pos[p]
                            dinv = dinvp.tile([C, 1], F32, tag="dinv",
                                              name=f"dinvi_{rep}_{p}_{z['c']}")
                            nc.vector.reciprocal(dinv[:], po[:, E:E1])
                            nc.scalar.mul(z["st"]["outs"][:, z["j"], :],
                                          po[:, 0:E], dinv[:])
                            if z["j"] == z["slen"] - 1:
                                base = p * N + z["c0"] * C
                                rows = slice(base, base + z["slen"] * C)
                                nc.sync.dma_start(
                                    out[rows, :].rearrange(
                                        "(n p) e -> p n e", p=C),
                                    z["st"]["outs"][:])
                        back = {}

                    for p, z in back.items():
                        cb = z["c"]
                        # with transpose_k the state update happens here, so
                        # the pre-update state must be captured here as well
                        if transpose_k:
                            z["prev_S"] = S_sbuf.get(p)
                        if transpose_k and z["kns0"] is not None:
                            mm(pS[p][:, 0:E1], z["kns0"][:],
                                             z["vc"], start=(cb == 0),
                                             stop=False, skip_group_check=True)
                            mm(pS[p][:, E1 + 1:2 * E1 + 1],
                                             z["kns1"][:], z["vc"],
                                             start=False,
                                             stop=(cb == NCHUNK - 1),
                                             skip_group_check=True)
                            if cb < NCHUNK - 1:
                                s01 = ssbp.tile([D, 2 * E1 + 2], MT, tag="ssb")
                                nc.scalar.copy(s01[:], pS[p][:])
                                S_sbuf[p] = s01
                        # numerator (cols 0..63) + denominator (col 64)
                        po = poutp.tile([C, E1], F32, tag="po")
                        mm(po[:], z["atm"][:], z["vc"],
                                         start=True,
                                         stop=(cb == 0 or z["prev_S"] is None))
                        if cb > 0 and z["prev_S"] is not None:
                            mm(po[:], z["qcT"],
                                             z["prev_S"][:, 0:E1],
                                             start=False, stop=False,
                                             skip_group_check=True)
                            mm(po[:], z["qrcT"],
                                             z["prev_S"][:, E1 + 1:2 * E1 + 1],
                                             start=False, stop=True,
                                             skip_group_check=True)

                        if host_norm:
                            # ship numerator and denominator; host divides
                            nc.scalar.copy(z["st"]["outs"][:, z["j"], :],
                                           po[:, 0:E1])
                        else:
                            # out[t,:] = num[t,:] / den[t]
                            dinv = dinvp.tile([C, 1], F32, tag="dinv")
                            nc.vector.reciprocal(dinv[:], po[:, E:E1])
                            nc.scalar.mul(z["st"]["outs"][:, z["j"], :],
                                          po[:, 0:E], dinv[:])

                        if z["j"] == z["slen"] - 1:
                            base = p * N + z["c0"] * C
                            rows = slice(base, base + z["slen"] * C)
                            nc.sync.dma_start(
                                out[rows, :].rearrange("(n p) e -> p n e", p=C),
                                z["st"]["outs"][:])

    nc.compile()
    return nc




F16 = mybir.dt.float16
F8 = mybir.dt.float8e4
U8 = mybir.dt.uint8


def build_kernel16(repeat=1, loop_k=None, dma_only=False, probe_pe_only=False,
                   slab=SLAB, slab_bufs=SLAB_BUFS):
    """fp16 variant. All inputs ship as fp16; natural-layout tensors
    (kn/krn/v1) and the output use a chunk-major [C, nchunk*f] DRAM layout so
    every DMA descriptor is a contiguous >=512B run. fp16 matmuls run at 1
    cycle/row on the PE (vs 4 for fp32), accumulation stays f32 in PSUM.
    Host un-permutes the output and upcasts to f32."""
    nc = bacc.Bacc("TRN2", target_bir_lowering=False, debug=False,
                   num_devices=N_CORES)

    NPC = PAIRS_PER_CORE
    qT = nc.dram_tensor("qT", [D, NROWS], F16, kind="ExternalInput").ap()
    kT = nc.dram_tensor("kT", [D, NROWS], F16, kind="ExternalInput").ap()
    qrT = nc.dram_tensor("qrT", [D, NROWS], F16, kind="ExternalInput").ap()
    krT = nc.dram_tensor("krT", [D, NROWS], F16, kind="ExternalInput").ap()
    kn = nc.dram_tensor("kn", [C, NPC * NCHUNK * D], F16, kind="ExternalInput").ap()
    krn = nc.dram_tensor("krn", [C, NPC * NCHUNK * D], F16, kind="ExternalInput").ap()
    v1 = nc.dram_tensor("v1", [C, NPC * NCHUNK * E1], F16, kind="ExternalInput").ap()
    mask = nc.dram_tensor("mask", [C, C], F32, kind="ExternalInput").ap()
    out = nc.dram_tensor("out", [C, NPC * NCHUNK * E], F16, kind="ExternalOutput").ap()

    plans = [{c0: slab for c0 in range(0, NCHUNK, slab)}] * NPC
    slab_of = []
    for pp in range(NPC):
        m = {}
        for c0, ln in plans[pp].items():
            for c in range(c0, c0 + ln):
                m[c] = (c0, ln)
        slab_of.append(m)

    with tile.TileContext(nc) as tc:
        with (
            tc.tile_pool(name="const", bufs=1) as constp,
            tc.tile_pool(name="slabs", bufs=slab_bufs) as slabs,
            tc.tile_pool(name="atm", bufs=4) as atmp,
            tc.tile_pool(name="ssb", bufs=8) as ssbp,
            tc.tile_pool(name="dinv", bufs=8) as dinvp,
            tc.tile_pool(name="pat", bufs=3, space="PSUM") as patp,
            tc.tile_pool(name="pout", bufs=3, space="PSUM") as poutp,
            tc.tile_pool(name="pst", bufs=(1 if pp else 2), space="PSUM") as pstp,
        ):
            mask_t = constp.tile([C, C], F32, tag="mask")
            nc.sync.dma_start(mask_t[:], mask[:])

            for rep in range(repeat):
              with (tc.For_i(0, loop_k, 1, hint_engines=(
                        mybir.EngineType.PE, mybir.EngineType.DVE,
                        mybir.EngineType.Activation, mybir.EngineType.SP))
                    if (loop_k is not None and loop_k > 1)
                    else _nullctx()):
                  pS = {}
                  for p in range(NPC):
                      pS[p] = pstp.tile([D, 2 * E1 + 2], F32, tag="pS",
                                        name=f"pS16_{rep}_{p}")

                  slab_t = [None] * NPC
                  S_sbuf = {}

                  fifo = []
                  for cc in range(NCHUNK + 1):
                    pending = {}
                    back = {}
                    if cc >= 1:
                        back = fifo.pop(0)
                    if cc < NCHUNK:
                        fifo.append(pending)
                    if cc < NCHUNK:
                      c = cc
                      for p in range(NPC):
                          c0, slen = slab_of[p][c]
                          if c == c0:
                              base = p * N + c * C
                              cols = slice(base, base + slen * C)
                              ncols = slice((p * NCHUNK + c) * D,
                                            (p * NCHUNK + c + slen) * D)
                              vcols = slice((p * NCHUNK + c) * E1,
                                            (p * NCHUNK + c + slen) * E1)
                              st = {"len": slen}
                              st["qT"] = slabs.tile([D, slen * C], F16, tag="qT", name=f"qTs16_{rep}_{p}_{c}")
                              nc.sync.dma_start(st["qT"][:], qT[:, cols])
                              st["kT"] = slabs.tile([D, slen * C], F16, tag="kT", name=f"kTs16_{rep}_{p}_{c}")
                              nc.sync.dma_start(st["kT"][:], kT[:, cols])
                              st["qrT"] = slabs.tile([D, slen * C], F16, tag="qrT", name=f"qrTs16_{rep}_{p}_{c}")
                              nc.sync.dma_start(st["qrT"][:], qrT[:, cols])
                              st["krT"] = slabs.tile([D, slen * C], F16, tag="krT", name=f"krTs16_{rep}_{p}_{c}")
                              nc.sync.dma_start(st["krT"][:], krT[:, cols])
                              st["kn"] = slabs.tile([C, slen * D], F16, tag="kn", name=f"kns16_{rep}_{p}_{c}")
                              nc.sync.dma_start(st["kn"][:], kn[:, ncols])
                              st["krn"] = slabs.tile([C, slen * D], F16, tag="krn", name=f"krns16_{rep}_{p}_{c}")
                              nc.sync.dma_start(st["krn"][:], krn[:, ncols])
                              st["v1"] = slabs.tile([C, slen * E1], F16, tag="v1", name=f"v1s16_{rep}_{p}_{c}")
                              nc.sync.dma_start(st["v1"][:], v1[:, vcols])
                              st["outs"] = slabs.tile([C, slen * E], F16, tag="outs", name=f"outs16_{rep}_{p}_{c}")
                              slab_t[p] = st

                          st = slab_t[p]
                          j = c - c0
                          if dma_only:
                              continue
                          z = dict(
                              qcT=st["qT"][:, j * C:(j + 1) * C],
                              kcT=st["kT"][:, j * C:(j + 1) * C],
                              qrcT=st["qrT"][:, j * C:(j + 1) * C],
                              krcT=st["krT"][:, j * C:(j + 1) * C],
                              knc=st["kn"][:, j * D:(j + 1) * D],
                              krnc=st["krn"][:, j * D:(j + 1) * D],
                              vc=st["v1"][:, j * E1:(j + 1) * E1],
                              st=st, j=j, c=c, c0=c0, slen=slen,
                              prev_S=S_sbuf.get(p))
                          pending[p] = z

                      if probe_pe_only and pending:
                          for p, z in sorted(pending.items()):
                              pat0 = patp.tile([C, C], F32, tag="pat")
                              nc.tensor.matmul(pat0[:], z["kcT"], z["qcT"], start=True, stop=False)
                              nc.tensor.matmul(pat0[:], z["krcT"], z["qrcT"], start=False, stop=True)
                              po = poutp.tile([C, E1], F32, tag="po")
                              nc.tensor.matmul(po[:], z["qcT"], mask_t[:, 0:E1].bitcast(F16)[:, 0:E1], start=True, stop=False, skip_group_check=True)
                              nc.tensor.matmul(po[:], z["qrcT"], mask_t[:, 0:E1].bitcast(F16)[:, 0:E1], start=False, stop=False, skip_group_check=True)
                              nc.tensor.matmul(po[:], z["kcT"], mask_t[:, 0:E1].bitcast(F16)[:, 0:E1], start=False, stop=True, skip_group_check=True)
                              nc.tensor.matmul(pS[p][:, 0:E1], z["knc"], z["vc"], start=(z["c"] == 0), stop=False, skip_group_check=True)
                              nc.tensor.matmul(pS[p][:, E1 + 1:2 * E1 + 1], z["krnc"], z["vc"], start=False, stop=(z["c"] == NCHUNK - 1), skip_group_check=True)
                          continue

                      if pending and not dma_only:
                        ps = sorted(pending.keys())
                        for br in range(2):
                            for p in ps:
                                z = pending[p]
                                if br == 0:
                                    nc.tensor.matmul(pS[p][:, 0:E1], z["knc"],
                                                     z["vc"], start=(c == 0),
                                                     stop=False,
                                                     skip_group_check=True)
                                else:
                                    nc.tensor.matmul(pS[p][:, E1 + 1:2 * E1 + 1],
                                                     z["krnc"], z["vc"],
                                                     start=False,
                                                     stop=(c == NCHUNK - 1),
                                                     skip_group_check=True)
                        for p in ps:
                            if c < NCHUNK - 1:
                                s01 = ssbp.tile([D, 2 * E1 + 2], F16, tag="ssb",
                                                name=f"s01h_{rep}_{p}_{c}")
                                nc.scalar.copy(s01[:], pS[p][:])
                                S_sbuf[p] = s01
                        pats = {}
                        for p in ps:
                            pats[p] = patp.tile([C, C], F32, tag="pat",
                                                name=f"path_{rep}_{p}_{c}")
                        for br in range(2):
                            for p in ps:
                                z = pending[p]
                                if br == 0:
                                    nc.tensor.matmul(pats[p][:], z["kcT"],
                                                     z["qcT"], start=True,
                                                     stop=False,
                                                     skip_group_check=True)
                                else:
                                    nc.tensor.matmul(pats[p][:], z["krcT"],
                                                     z["qrcT"], start=False,
                                                     stop=True,
                                                     skip_group_check=True)
                        for p in ps:
                            atm = atmp.tile([C, C], F16, tag="atm",
                                            name=f"atmh_{rep}_{p}_{c}")
                            nc.vector.tensor_mul(atm[:], pats[p][:], mask_t[:])
                            pending[p]["atm"] = atm

                    if back and not dma_only and not probe_pe_only:
                        items = sorted(back.items())
                        pos = {}
                        for p, z in items:
                            pos[p] = poutp.tile([C, E1], F32, tag="po",
                                                name=f"poh_{rep}_{p}_{z['c']}")
                        for p, z in items:
                            nc.tensor.matmul(pos[p][:], z["atm"][:], z["vc"],
                                             start=True,
                                             stop=(z["c"] == 0 or z["prev_S"] is None),
                                             skip_group_check=True)
                        for p, z in items:
                            if z["c"] > 0 and z["prev_S"] is not None:
                                nc.tensor.matmul(pos[p][:], z["qcT"],
                                                 z["prev_S"][:, 0:E1],
                                                 start=False, stop=False,
                                                 skip_group_check=True)
                        for p, z in items:
                            if z["c"] > 0 and z["prev_S"] is not None:
                                nc.tensor.matmul(pos[p][:], z["qrcT"],
                                                 z["prev_S"][:, E1 + 1:2 * E1 + 1],
                                                 start=False, stop=True,
                                                 skip_group_check=True)
                        for p, z in items:
                            po = pos[p]
                            dinv = dinvp.tile([C, 1], F32, tag="dinv",
                                              name=f"dinvh_{rep}_{p}_{z['c']}")
                            nc.vector.reciprocal(dinv[:], po[:, E:E1])
                            nc.scalar.mul(
                                z["st"]["outs"][:, z["j"] * E:(z["j"] + 1) * E],
                                po[:, 0:E], dinv[:])
                            if z["j"] == z["slen"] - 1:
                                ocols = slice((p * NCHUNK + z["c0"]) * E,
                                              (p * NCHUNK + z["c0"] + z["slen"]) * E)
                                nc.sync.dma_start(out[:, ocols],
                                                  z["st"]["outs"][:])

    nc.compile()
    return nc


def build_kernel16b(repeat=1, loop_k=None, dma_only=False, probe_pe_pure=False,
                    slab=8, slab_bufs=3):
    """fp16 + packed-DMA variant: per (pair, slab) ONE load of the merged
    transposed block [qT|kT|qrT|krT], ONE load of the merged natural block
    [kn|krn|v1], ONE store of the output block. At slab=8 that is 12 DMA
    instructions per iteration (vs 57 in v1), sidestepping the ~625ns/DMA
    HWDGE descriptor-generation serialization that dominated the v1 floor.

    probe_pe_pure: run the full per-chunk matmul bundle on tiles loaded once
    outside the loop — a clean PE-only floor with no DMA dependencies."""
    nc = bacc.Bacc("TRN2", target_bir_lowering=False, debug=False,
                   num_devices=N_CORES)

    NPC = PAIRS_PER_CORE
    NSLAB = NCHUNK // slab
    SC = slab * C
    KVW = slab * (2 * D + E1)       # merged natural-block width per slab
    qk = nc.dram_tensor("qk", [D, NPC * NSLAB * 4 * SC], F16,
                        kind="ExternalInput").ap()
    kv = nc.dram_tensor("kv", [C, NPC * NSLAB * KVW], F16,
                        kind="ExternalInput").ap()
    mask = nc.dram_tensor("mask", [C, C], F32, kind="ExternalInput").ap()
    out = nc.dram_tensor("out", [C, NPC * NCHUNK * E], F16,
                         kind="ExternalOutput").ap()

    with tile.TileContext(nc) as tc:
        with (
            tc.tile_pool(name="const", bufs=1) as constp,
            tc.tile_pool(name="slabs", bufs=slab_bufs) as slabs,
            tc.tile_pool(name="atm", bufs=4) as atmp,
            tc.tile_pool(name="ssb", bufs=8) as ssbp,
            tc.tile_pool(name="dinv", bufs=8) as dinvp,
            tc.tile_pool(name="pat", bufs=3, space="PSUM") as patp,
            tc.tile_pool(name="pout", bufs=3, space="PSUM") as poutp,
            tc.tile_pool(name="pst", bufs=(1 if pp else 2), space="PSUM") as pstp,
        ):
            mask_t = constp.tile([C, C], F32, tag="mask")
            nc.sync.dma_start(mask_t[:], mask[:])

            pure = {}
            if probe_pe_pure:
                # one fixed tile set, loaded once; the loop's MMs reference it
                pure["qk"] = constp.tile([D, 4 * SC], F16, tag="pqk", name="pqk")
                nc.sync.dma_start(pure["qk"][:], qk[:, 0:4 * SC])
                pure["kv"] = constp.tile([C, KVW], F16, tag="pkv", name="pkv")
                nc.sync.dma_start(pure["kv"][:], kv[:, 0:KVW])
                pure["atm"] = constp.tile([C, C], F16, tag="patm", name="patm")
                nc.vector.tensor_copy(pure["atm"][:], mask_t[:])
                pure["s01"] = constp.tile([D, 2 * E1 + 2], F16, tag="ps01", name="ps01")
                nc.vector.tensor_copy(pure["s01"][:], pure["kv"][:, 0:2 * E1 + 2])

            for rep in range(repeat):
              with (tc.For_i(0, loop_k, 1, hint_engines=(
                        mybir.EngineType.PE, mybir.EngineType.DVE,
                        mybir.EngineType.Activation, mybir.EngineType.SP))
                    if (loop_k is not None and loop_k > 1)
                    else _nullctx()):
                  if probe_pe_pure:
                      # 7-MM bundle x NCHUNK x NPC on fixed tiles
                      pqk, pkv = pure["qk"], pure["kv"]
                      for c in range(NCHUNK):
                        for p in range(NPC):
                          j = c % slab
                          qcT = pqk[:, 0 * SC + j * C:0 * SC + (j + 1) * C]
                          kcT = pqk[:, 1 * SC + j * C:1 * SC + (j + 1) * C]
                          qrcT = pqk[:, 2 * SC + j * C:2 * SC + (j + 1) * C]
                          krcT = pqk[:, 3 * SC + j * C:3 * SC + (j + 1) * C]
                          knc = pkv[:, j * D:(j + 1) * D]
                          krnc = pkv[:, slab * D + j * D:slab * D + (j + 1) * D]
                          vc = pkv[:, 2 * slab * D + j * E1:2 * slab * D + (j + 1) * E1]
                          pS = pstp.tile([D, 2 * E1 + 2], F32, tag="pS")
                          nc.tensor.matmul(pS[:, 0:E1], knc, vc, start=True, stop=False, skip_group_check=True)
                          nc.tensor.matmul(pS[:, E1 + 1:2 * E1 + 1], krnc, vc, start=False, stop=True, skip_group_check=True)
                          pat0 = patp.tile([C, C], F32, tag="pat")
                          nc.tensor.matmul(pat0[:], kcT, qcT, start=True, stop=False)
                          nc.tensor.matmul(pat0[:], krcT, qrcT, start=False, stop=True)
                          po = poutp.tile([C, E1], F32, tag="po")
                          nc.tensor.matmul(po[:], pure["atm"][:, 0:C], vc, start=True, stop=False, skip_group_check=True)
                          nc.tensor.matmul(po[:], qcT, pure["s01"][:, 0:E1], start=False, stop=False, skip_group_check=True)
                          nc.tensor.matmul(po[:], qrcT, pure["s01"][:, E1 + 1:2 * E1 + 1], start=False, stop=True, skip_group_check=True)
                      continue

                  pS = {}
                  for p in range(NPC):
                      pS[p] = pstp.tile([D, 2 * E1 + 2], F32, tag="pS",
                                        name=f"pSb_{rep}_{p}")

                  slab_t = [None] * NPC
                  S_sbuf = {}

                  fifo = []
                  for cc in range(NCHUNK + 1):
                    pending = {}
                    back = {}
                    if cc >= 1:
                        back = fifo.pop(0)
                    if cc < NCHUNK:
                        fifo.append(pending)
                    if cc < NCHUNK:
                      c = cc
                      for p in range(NPC):
                          c0 = (c // slab) * slab
                          si = c // slab
                          if c == c0:
                              qbase = (p * NSLAB + si) * 4 * SC
                              kbase = (p * NSLAB + si) * KVW
                              st = {}
                              st["kv"] = slabs.tile([C, KVW], F16, tag="kv",
                                                    name=f"kvs_{rep}_{p}_{c}")
                              nc.sync.dma_start(st["kv"][:],
                                                kv[:, kbase:kbase + KVW])
                              st["qk"] = slabs.tile([D, 4 * SC], F16, tag="qk",
                                                    name=f"qks_{rep}_{p}_{c}")
                              nc.sync.dma_start(st["qk"][:],
                                                qk[:, qbase:qbase + 4 * SC])
                              st["outs"] = slabs.tile([C, slab * E], F16,
                                                      tag="outs",
                                                      name=f"outsb_{rep}_{p}_{c}")
                              slab_t[p] = st

                          st = slab_t[p]
                          j = c - c0
                          if dma_only:
                              continue
                          z = dict(
                              qcT=st["qk"][:, 0 * SC + j * C:0 * SC + (j + 1) * C],
                              kcT=st["qk"][:, 1 * SC + j * C:1 * SC + (j + 1) * C],
                              qrcT=st["qk"][:, 2 * SC + j * C:2 * SC + (j + 1) * C],
                              krcT=st["qk"][:, 3 * SC + j * C:3 * SC + (j + 1) * C],
                              knc=st["kv"][:, j * D:(j + 1) * D],
                              krnc=st["kv"][:, slab * D + j * D:slab * D + (j + 1) * D],
                              vc=st["kv"][:, 2 * slab * D + j * E1:2 * slab * D + (j + 1) * E1],
                              st=st, j=j, c=c, c0=c0, slen=slab,
                              prev_S=S_sbuf.get(p))
                          pending[p] = z

                      if pending and not dma_only:
                        ps = sorted(pending.keys())
                        for br in range(2):
                            for p in ps:
                                z = pending[p]
                                if br == 0:
                                    nc.tensor.matmul(pS[p][:, 0:E1], z["knc"],
                                                     z["vc"], start=(c == 0),
                                                     stop=False,
                                                     skip_group_check=True)
                                else:
                                    nc.tensor.matmul(pS[p][:, E1 + 1:2 * E1 + 1],
                                                     z["krnc"], z["vc"],
                                                     start=False,
                                                     stop=(c == NCHUNK - 1),
                                                     skip_group_check=True)
                        for p in ps:
                            if c < NCHUNK - 1:
                                s01 = ssbp.tile([D, 2 * E1 + 2], F16, tag="ssb",
                                                name=f"s01b_{rep}_{p}_{c}")
                                nc.scalar.copy(s01[:], pS[p][:])
                                S_sbuf[p] = s01
                        pats = {}
                        for p in ps:
                            pats[p] = patp.tile([C, C], F32, tag="pat",
                                                name=f"patb_{rep}_{p}_{c}")
                        for br in range(2):
                            for p in ps:
                                z = pending[p]
                                if br == 0:
                                    nc.tensor.matmul(pats[p][:], z["kcT"],
                                                     z["qcT"], start=True,
                                                     stop=False,
                                                     skip_group_check=True)
                                else:
                                    nc.tensor.matmul(pats[p][:], z["krcT"],
                                                     z["qrcT"], start=False,
                                                     stop=True,
                                                     skip_group_check=True)
                        for p in ps:
                            atm = atmp.tile([C, C], F16, tag="atm",
                                            name=f"atmb_{rep}_{p}_{c}")
                            nc.vector.tensor_mul(atm[:], pats[p][:], mask_t[:])
                            pending[p]["atm"] = atm

                    if back and not dma_only:
                        items = sorted(back.items())
                        pos = {}
                        for p, z in items:
                            pos[p] = poutp.tile([C, E1], F32, tag="po",
                                                name=f"pob_{rep}_{p}_{z['c']}")
                        for p, z in items:
                            nc.tensor.matmul(pos[p][:], z["atm"][:], z["vc"],
                                             start=True,
                                             stop=(z["c"] == 0 or z["prev_S"] is None),
                                             skip_group_check=True)
                        for p, z in items:
                            if z["c"] > 0 and z["prev_S"] is not None:
                                nc.tensor.matmul(pos[p][:], z["qcT"],
                                                 z["prev_S"][:, 0:E1],
                                                 start=False, stop=False,
                                                 skip_group_check=True)
                        for p, z in items:
                            if z["c"] > 0 and z["prev_S"] is not None:
                                nc.tensor.matmul(pos[p][:], z["qrcT"],
                                                 z["prev_S"][:, E1 + 1:2 * E1 + 1],
                                                 start=False, stop=True,
                                                 skip_group_check=True)
                        for p, z in items:
                            po = pos[p]
                            dinv = dinvp.tile([C, 1], F32, tag="dinv",
                                              name=f"dinvb_{rep}_{p}_{z['c']}")
                            nc.vector.reciprocal(dinv[:], po[:, E:E1])
                            nc.scalar.mul(
                                z["st"]["outs"][:, z["j"] * E:(z["j"] + 1) * E],
                                po[:, 0:E], dinv[:])
                            if z["j"] == z["slen"] - 1:
                                ocols = slice((p * NCHUNK + z["c0"]) * E,
                                              (p * NCHUNK + z["c0"] + z["slen"]) * E)
                                nc.sync.dma_start(out[:, ocols],
                                                  z["st"]["outs"][:])

    nc.compile()
    return nc


def build_kernel16c(repeat=1, loop_k=None, dma_only=False, slab=8,
                    slab_bufs=3, recip_dev=False):
    """v3: fp16 + packed DMA (as 16b) + pair-merged PSUM banks.

    Both (b,h) pairs handled by a core share single PSUM banks for AT, for
    num|den, and for the scan state, so the causal mask, the state
    evacuation, and the num/den evacuation each run as ONE wide DVE/ACT op
    per chunk instead of one per pair. The division happens on the host
    (kernel ships num and den); no reciprocal / scale ops on device.
    """
    nc = bacc.Bacc("TRN2", target_bir_lowering=False, debug=False,
                   num_devices=N_CORES)

    NPC = PAIRS_PER_CORE
    NSLAB = NCHUNK // slab
    SC = slab * C
    KVW = slab * (2 * D + E1)
    OW = 2 * E1                      # per-chunk output cols (both pairs)
    qk = nc.dram_tensor("qk", [D, NPC * NSLAB * 4 * SC], F16,
                        kind="ExternalInput").ap()
    kv = nc.dram_tensor("kv", [C, NPC * NSLAB * KVW], F16,
                        kind="ExternalInput").ap()
    mask2 = nc.dram_tensor("mask2", [C, 2 * C], F32, kind="ExternalInput").ap()
    out = nc.dram_tensor("out", [C, NCHUNK * OW], F16,
                         kind="ExternalOutput").ap()

    with tile.TileContext(nc) as tc:
        with (
            tc.tile_pool(name="const", bufs=1) as constp,
            tc.tile_pool(name="slabs", bufs=slab_bufs) as slabs,
            tc.tile_pool(name="atm", bufs=4) as atmp,
            tc.tile_pool(name="ssb", bufs=4) as ssbp,
            tc.tile_pool(name="pat", bufs=3, space="PSUM") as patp,
            tc.tile_pool(name="pout", bufs=3, space="PSUM") as poutp,
            tc.tile_pool(name="pst", bufs=(1 if pp else 2), space="PSUM") as pstp,
        ):
            mask_t = constp.tile([C, 2 * C], F32, tag="mask")
            nc.sync.dma_start(mask_t[:], mask2[:])

            fixed = None
            if probe_nodma:
                # one 8-chunk slab-set per pair, loaded once; the loop reuses
                # it for both halves -> compute rhythm with no load DMAs
                fixed = []
                fl = 8
                for p in range(NPC):
                    fqk = constp.tile([D, 4 * fl * C], F16, tag=f"fqk{p}",
                                      name=f"fqk{p}")
                    nc.sync.dma_start(fqk[:], qk[:, 0:4 * fl * C])
                    fkv = constp.tile([C, fl * (2 * D + E1)], F16,
                                      tag=f"fkv{p}", name=f"fkv{p}")
                    nc.sync.dma_start(fkv[:], kv[:, 0:fl * (2 * D + E1)])
                    fixed.append({"qk": fqk, "kv": fkv, "ln": fl})

            for rep in range(repeat):
              with (tc.For_i(0, loop_k, 1, hint_engines=(
                        mybir.EngineType.PE, mybir.EngineType.DVE,
                        mybir.EngineType.Activation, mybir.EngineType.SP))
                    if (loop_k is not None and loop_k > 1)
                    else _nullctx()):
                  pSb = pstp.tile([D, 2 * NPC * SW], F32, tag="pS",
                                  name=f"pSc_{rep}")

                  slab_t = [None] * NPC
                  outs_t = [None]
                  S_sbuf = [None]

                  pend = None
                  for cc in range(NCHUNK + 1):
                    back = pend
                    pend = None
                    if cc < NCHUNK:
                      c = cc
                      si = c // slab
                      c0 = si * slab
                      j = c - c0
                      sl = {}
                      for p in range(NPC):
                          if c == c0:
                              qbase = (p * NSLAB + si) * 4 * SC
                              kbase = (p * NSLAB + si) * KVW
                              st = {}
                              st["kv"] = slabs.tile([C, KVW], F16, tag="kv",
                                                    name=f"kvc_{rep}_{p}_{c}")
                              nc.sync.dma_start(st["kv"][:],
                                                kv[:, kbase:kbase + KVW])
                              st["qk"] = slabs.tile([D, 4 * SC], F16, tag="qk",
                                                    name=f"qkc_{rep}_{p}_{c}")
                              nc.sync.dma_start(st["qk"][:],
                                                qk[:, qbase:qbase + 4 * SC])
                              slab_t[p] = st
                          st = slab_t[p]
                          sl[p] = dict(
                              qcT=st["qk"][:, 0 * SC + j * C:0 * SC + (j + 1) * C],
                              kcT=st["qk"][:, 1 * SC + j * C:1 * SC + (j + 1) * C],
                              qrcT=st["qk"][:, 2 * SC + j * C:2 * SC + (j + 1) * C],
                              krcT=st["qk"][:, 3 * SC + j * C:3 * SC + (j + 1) * C],
                              knc=st["kv"][:, j * D:(j + 1) * D],
                              krnc=st["kv"][:, slab * D + j * D:slab * D + (j + 1) * D],
                              vc=st["kv"][:, 2 * slab * D + j * E1:2 * slab * D + (j + 1) * E1])
                      if c == c0:
                          outs_t[0] = slabs.tile([C, slab * OW], F16, tag="outs",
                                                 name=f"outsc_{rep}_{c}")
                      if dma_only:
                          if c - c0 == slab - 1:
                              ocols = slice(c0 * OW, (c0 + slab) * OW)
                              nc.sync.dma_start(out[:, ocols], outs_t[0][:])
                          continue

                      prev_S = S_sbuf[0]

                      # state updates: 4 regions of one PSUM bank
                      for br in range(2):
                          for p in range(NPC):
                              z = sl[p]
                              reg = pSb[:, (2 * p + br) * SW:(2 * p + br) * SW + E1]
                              nc.tensor.matmul(
                                  reg, z["knc"] if br == 0 else z["krnc"],
                                  z["vc"],
                                  start=(c == 0 and br == 0 and p == 0),
                                  stop=(c == NCHUNK - 1 and br == 1 and p == NPC - 1),
                                  skip_group_check=True)
                      if c < NCHUNK - 1:
                          s01 = ssbp.tile([D, 2 * NPC * SW], F16, tag="ssb",
                                          name=f"s01c_{rep}_{c}")
                          nc.scalar.copy(s01[:], pSb[:])
                          S_sbuf[0] = s01

                      # AT for both pairs into one bank, one mask op
                      patb = patp.tile([C, 2 * C], F32, tag="pat",
                                       name=f"patc_{rep}_{c}")
                      for br in range(2):
                          for p in range(NPC):
                              z = sl[p]
                              reg = patb[:, p * C:(p + 1) * C]
                              nc.tensor.matmul(
                                  reg, z["kcT"] if br == 0 else z["krcT"],
                                  z["qcT"] if br == 0 else z["qrcT"],
                                  start=(br == 0 and p == 0),
                                  stop=(br == 1 and p == NPC - 1),
                                  skip_group_check=True)
                      atm = atmp.tile([C, 2 * C], F16, tag="atm",
                                      name=f"atmc_{rep}_{c}")
                      nc.vector.tensor_mul(atm[:], patb[:], mask_t[:])

                      pend = dict(atm=atm, sl=sl, c=c, c0=c0, j=j,
                                  prev_S=prev_S, outs=outs_t[0])

                    if back is not None and not dma_only:
                        cb = back["c"]
                        pob = poutp.tile([C, NPC * PW], F32, tag="po",
                                         name=f"poc_{rep}_{cb}")
                        for p in range(NPC):
                            z = back["sl"][p]
                            reg = pob[:, p * PW:p * PW + E1]
                            nc.tensor.matmul(
                                reg, back["atm"][:, p * C:(p + 1) * C],
                                z["vc"], start=(p == 0),
                                stop=(cb == 0 and p == NPC - 1),
                                skip_group_check=True)
                        if cb > 0:
                            pv = back["prev_S"]
                            if pp:
                                terms = [t for t in pv if t is not None]
                            else:
                                terms = [pv]
                            for ti, term in enumerate(terms):
                                last_t = (ti == len(terms) - 1)
                                for br in range(2):
                                    for p in range(NPC):
                                        z = back["sl"][p]
                                        reg = pob[:, p * PW:p * PW + E1]
                                        nc.tensor.matmul(
                                            reg,
                                            z["qcT"] if br == 0 else z["qrcT"],
                                            term[:, (2 * p + br) * SW:(2 * p + br) * SW + E1],
                                            start=False,
                                            stop=(last_t and br == 1 and p == NPC - 1),
                                            skip_group_check=True)

                        # evacuate num|den for both pairs in one op
                        src = pob[:].rearrange("p (g w) -> p g w", g=NPC)[:, :, 0:E1]
                        dst = back["outs"][:, back["j"] * OW:(back["j"] + 1) * OW]
                        dst = dst.rearrange("p (g w) -> p g w", g=NPC)
                        if cb % 2 == 0:
                            nc.vector.tensor_copy(dst, src)
                        else:
                            nc.scalar.copy(dst, src)
                        if back["j"] == slab - 1:
                            ocols = slice(back["c0"] * OW,
                                          (back["c0"] + slab) * OW)
                            nc.sync.dma_start(out[:, ocols], back["outs"][:])

    nc.compile()
    return nc


def _plan_slabs(plan):
    """plan: list of slab lengths summing to NCHUNK -> per-chunk lookup."""
    slabs, c0 = [], 0
    for ln in plan:
        slabs.append((c0, ln))
        c0 += ln
    assert c0 == NCHUNK
    of = {}
    for si, (c0, ln) in enumerate(slabs):
        for c in range(c0, c0 + ln):
            of[c] = (si, c0, ln)
    return slabs, of


def build_kernel16d(repeat=1, loop_k=None, dma_only=False, slab=8,
                    slab_bufs=3, pipe=2, store_q="gpsimd", plan=None,
                    probe_nodma=False, stag=False, add_eng="vector",
                    kn8=False, pp=False, load_split=False):
    """v4: 16c + latency-chain fixes.

    - The scan state lives as an SBUF fp16 running sum: each chunk's outer
      product goes to a FRESH PSUM bank (no in-place PSUM accumulation), and
      ACT folds it into the running state (s01 += pS). This breaks the
      PE->ACT->PE anti-dependency cycle that serialized v3 (~1us/chunk).
    - Back stage (num/den matmuls + evacuation) trails by `pipe` chunks so
      every cross-engine hop has >= 1 full stage of slack.
    - Output stores go on the Pool (SWDGE) queue so the SP load queue never
      head-of-line blocks on end-of-iteration stores.
    """
    nc = bacc.Bacc("TRN2", target_bir_lowering=False, debug=False,
                   num_devices=N_CORES)

    NPC = PAIRS_PER_CORE
    if plan is None:
        plan = [slab] * (NCHUNK // slab)
    slabs_l, slab_of = _plan_slabs(plan)
    # per-(pair, slab) base offsets in the packed DRAM tensors
    QKW_of = [4 * ln * C for _, ln in slabs_l]
    # kv block: kn|krn (fp8 bytes when kn8) + v1 (fp16); widths in ELEMENTS
    # of the kv dram dtype (uint8 when kn8, fp16 otherwise)
    KD = D if kn8 else D          # kn col width per chunk in dram elements
    KVW_of = [ln * (2 * D + 2 * E1) if kn8 else ln * (2 * D + E1)
              for _, ln in slabs_l]
    KVT = U8 if kn8 else F16
    qk_tot = sum(QKW_of)
    kv_tot = sum(KVW_of)
    qk_base = [[p * qk_tot + sum(QKW_of[:si]) for si in range(len(slabs_l))]
               for p in range(NPC)]
    kv_base = [[p * kv_tot + sum(KVW_of[:si]) for si in range(len(slabs_l))]
               for p in range(NPC)]
    OW = 2 * E1
    qk = nc.dram_tensor("qk", [D, NPC * qk_tot], F16,
                        kind="ExternalInput").ap()
    kv = nc.dram_tensor("kv", [C, NPC * kv_tot], KVT,
                        kind="ExternalInput").ap()
    mask2 = nc.dram_tensor("mask2", [C, 2 * C], F32, kind="ExternalInput").ap()
    out = nc.dram_tensor("out", [C, NCHUNK * OW], F16,
                         kind="ExternalOutput").ap()

    store_eng = {"gpsimd": nc.gpsimd, "sync": nc.sync}[store_q]

    with tile.TileContext(nc) as tc:
        with (
            tc.tile_pool(name="const", bufs=1) as constp,
            tc.tile_pool(name="slabs", bufs=slab_bufs) as slabs,
            tc.tile_pool(name="atm", bufs=pipe + 2) as atmp,
            tc.tile_pool(name="ssb", bufs=pipe + 3) as ssbp,
            tc.tile_pool(name="pat", bufs=3, space="PSUM") as patp,
            tc.tile_pool(name="pout", bufs=3, space="PSUM") as poutp,
            tc.tile_pool(name="pst", bufs=(1 if pp else 2), space="PSUM") as pstp,
        ):
            mask_t = constp.tile([C, 2 * C], F32, tag="mask")
            nc.sync.dma_start(mask_t[:], mask2[:])

            fixed = None
            if probe_nodma:
                # one 8-chunk slab-set per pair, loaded once; the loop reuses
                # it for both halves -> compute rhythm with no load DMAs
                fixed = []
                fl = 8
                for p in range(NPC):
                    fqk = constp.tile([D, 4 * fl * C], F16, tag=f"fqk{p}",
                                      name=f"fqk{p}")
                    nc.sync.dma_start(fqk[:], qk[:, 0:4 * fl * C])
                    fkv = constp.tile([C, fl * (2 * D + E1)], F16,
                                      tag=f"fkv{p}", name=f"fkv{p}")
                    nc.sync.dma_start(fkv[:], kv[:, 0:fl * (2 * D + E1)])
                    fixed.append({"qk": fqk, "kv": fkv, "ln": fl})

            for rep in range(repeat):
              with (tc.For_i(0, loop_k, 1, staggered_reset=stag,
                             hint_engines=(
                        mybir.EngineType.PE, mybir.EngineType.DVE,
                        mybir.EngineType.Activation, mybir.EngineType.SP,
                        mybir.EngineType.Pool))
                    if (loop_k is not None and loop_k > 1)
                    else _nullctx()):
                  slab_t = [None] * NPC
                  outs_t = [None]
                  S_sbuf = [None]
                  pS_pp = [None, None]    # ping-pong PSUM state banks
                  S_snap = [None, None]   # latest SBUF snapshot per bank
                  if pp:
                      pS_pp[0] = pstp.tile([D, 2 * NPC * SW], F32, tag="pSX",
                                           name=f"pSX_{rep}")
                      pS_pp[1] = pstp.tile([D, 2 * NPC * SW], F32, tag="pSY",
                                           name=f"pSY_{rep}")

                  fifo = []
                  for cc in range(NCHUNK + pipe):
                    back = None
                    if cc >= pipe and fifo:
                        back = fifo.pop(0)
                    if cc < NCHUNK:
                      c = cc
                      si, c0, ln = slab_of[c]
                      j = c - c0
                      sl = {}
                      for p in range(NPC):
                          if probe_nodma:
                              slab_t[p] = fixed[p]
                              c0, ln = (0 if c < 8 else 8), 8
                              j = c - c0
                          elif c == c0:
                              qbase = qk_base[p][si]
                              kbase = kv_base[p][si]
                              dmae = (nc.scalar if (load_split and p == 1)
                                      else nc.sync)
                              st = {"ln": ln}
                              st["kv"] = slabs.tile([C, KVW_of[si]], KVT, tag="kv",
                                                    name=f"kvd_{rep}_{p}_{c}")
                              dmae.dma_start(st["kv"][:],
                                             kv[:, kbase:kbase + KVW_of[si]])
                              st["qk"] = slabs.tile([D, QKW_of[si]], F16, tag="qk",
                                                    name=f"qkd_{rep}_{p}_{c}")
                              dmae.dma_start(st["qk"][:],
                                             qk[:, qbase:qbase + QKW_of[si]])
                              slab_t[p] = st
                          st = slab_t[p]
                          SC = st["ln"] * C
                          lnp = st["ln"]
                          if kn8:
                              knc = st["kv"][:, j * D:(j + 1) * D].bitcast(F8)
                              krnc = st["kv"][:, lnp * D + j * D:lnp * D + (j + 1) * D].bitcast(F8)
                              vc = st["kv"][:, 2 * lnp * D + j * 2 * E1:2 * lnp * D + (j + 1) * 2 * E1].bitcast(F16)
                          else:
                              knc = st["kv"][:, j * D:(j + 1) * D]
                              krnc = st["kv"][:, lnp * D + j * D:lnp * D + (j + 1) * D]
                              vc = st["kv"][:, 2 * lnp * D + j * E1:2 * lnp * D + (j + 1) * E1]
                          sl[p] = dict(
                              qcT=st["qk"][:, 0 * SC + j * C:0 * SC + (j + 1) * C],
                              kcT=st["qk"][:, 1 * SC + j * C:1 * SC + (j + 1) * C],
                              qrcT=st["qk"][:, 2 * SC + j * C:2 * SC + (j + 1) * C],
                              krcT=st["qk"][:, 3 * SC + j * C:3 * SC + (j + 1) * C],
                              knc=knc, krnc=krnc, vc=vc)
                      if probe_nodma:
                          ln = 8
                      if c == c0:
                          outs_t[0] = slabs.tile([C, ln * OW], F16, tag="outs",
                                                 name=f"outsd_{rep}_{c}")
                      if dma_only:
                          if j == ln - 1:
                              ocols = slice(c0 * OW, (c0 + ln) * OW)
                              store_eng.dma_start(
                                  out[:, ocols],
                                  slab_t[0]["kv"][:, 0:2 * ln * OW].bitcast(F16)
                                  if kn8 else slab_t[0]["kv"][:, 0:ln * OW])
                          continue

                      if pp:
                          # ping-pong: bank c%2 accumulates in place; the
                          # snapshot is a PLAIN copy (ACT-capable) and stays
                          # valid for two chunks
                          B = c % 2
                          bank = pS_pp[B]
                          prev_S = (None if c == 0 else
                                    (S_snap[1 - B], S_snap[B] if c >= 2 else None))
                          for br in range(2):
                              for p in range(NPC):
                                  z = sl[p]
                                  reg = bank[:, (2 * p + br) * SW:(2 * p + br) * SW + E1]
                                  nc.tensor.matmul(
                                      reg, z["knc"] if br == 0 else z["krnc"],
                                      z["vc"],
                                      start=(c < 2 and br == 0 and p == 0),
                                      stop=(c >= NCHUNK - 2 and br == 1 and p == NPC - 1),
                                      skip_group_check=True)
                          if c < NCHUNK - 1:
                              s01 = ssbp.tile([D, 2 * NPC * SW], F16, tag="ssb",
                                              name=f"s01d_{rep}_{c}")
                              if c % 2 == 0:
                                  nc.scalar.copy(s01[:], bank[:])
                              else:
                                  nc.vector.tensor_copy(s01[:], bank[:])
                              S_snap[B] = s01
                      else:
                          prev_S = S_sbuf[0]

                          # fresh per-chunk outer product (one PSUM group)
                          pSc = pstp.tile([D, 2 * NPC * SW], F32, tag="pS",
                                          name=f"pSd_{rep}_{c}")
                          for br in range(2):
                              for p in range(NPC):
                                  z = sl[p]
                                  reg = pSc[:, (2 * p + br) * SW:(2 * p + br) * SW + E1]
                                  nc.tensor.matmul(
                                      reg, z["knc"] if br == 0 else z["krnc"],
                                      z["vc"],
                                      start=(br == 0 and p == 0),
                                      stop=(br == 1 and p == NPC - 1),
                                      skip_group_check=True)
                          if c < NCHUNK - 1:
                              s01 = ssbp.tile([D, 2 * NPC * SW], F16, tag="ssb",
                                              name=f"s01d_{rep}_{c}")
                              addeng = getattr(nc, add_eng)
                              if prev_S is None:
                                  addeng.tensor_copy(s01[:], pSc[:])
                              else:
                                  addeng.tensor_add(s01[:], pSc[:], prev_S[:])
                              S_sbuf[0] = s01

                      patb = patp.tile([C, 2 * C], F32, tag="pat",
                                       name=f"patd_{rep}_{c}")
                      for br in range(2):
                          for p in range(NPC):
                              z = sl[p]
                              reg = patb[:, p * C:(p + 1) * C]
                              nc.tensor.matmul(
                                  reg, z["kcT"] if br == 0 else z["krcT"],
                                  z["qcT"] if br == 0 else z["qrcT"],
                                  start=(br == 0 and p == 0),
                                  stop=(br == 1 and p == NPC - 1),
                                  skip_group_check=True)
                      atm = atmp.tile([C, 2 * C], F16, tag="atm",
                                      name=f"atmd_{rep}_{c}")
                      nc.vector.tensor_mul(atm[:], patb[:], mask_t[:])

                      fifo.append(dict(atm=atm, sl=sl, c=c, c0=c0, j=j,
                                       ln=ln, prev_S=prev_S, outs=outs_t[0]))

                    if back is not None and not dma_only:
                        cb = back["c"]
                        pob = poutp.tile([C, NPC * PW], F32, tag="po",
                                         name=f"pod_{rep}_{cb}")
                        for p in range(NPC):
                            z = back["sl"][p]
                            reg = pob[:, p * PW:p * PW + E1]
                            nc.tensor.matmul(
                                reg, back["atm"][:, p * C:(p + 1) * C],
                                z["vc"], start=(p == 0),
                                stop=(cb == 0 and p == NPC - 1),
                                skip_group_check=True)
                        if cb > 0:
                            pv = back["prev_S"]
                            if pp:
                                terms = [t for t in pv if t is not None]
                            else:
                                terms = [pv]
                            for ti, term in enumerate(terms):
                                last_t = (ti == len(terms) - 1)
                                for br in range(2):
                                    for p in range(NPC):
                                        z = back["sl"][p]
                                        reg = pob[:, p * PW:p * PW + E1]
                                        nc.tensor.matmul(
                                            reg,
                                            z["qcT"] if br == 0 else z["qrcT"],
                                            term[:, (2 * p + br) * SW:(2 * p + br) * SW + E1],
                                            start=False,
                                            stop=(last_t and br == 1 and p == NPC - 1),
                                            skip_group_check=True)

                        src = pob[:].rearrange("p (g w) -> p g w", g=NPC)[:, :, 0:E1]
                        dst = back["outs"][:, back["j"] * OW:(back["j"] + 1) * OW]
                        dst = dst.rearrange("p (g w) -> p g w", g=NPC)
                        if pp and cb % 2 == 0:
                            nc.vector.tensor_copy(dst, src)
                        else:
                            nc.scalar.copy(dst, src)
                        if back["j"] == back["ln"] - 1:
                            ocols = slice(back["c0"] * OW,
                                          (back["c0"] + back["ln"]) * OW)
                            store_eng.dma_start(out[:, ocols], back["outs"][:])

    nc.compile()
    return nc


# Final tuned configuration (see session notes): fp16 everywhere, fp8-e4m3
# for the state-path k/k_rot, packed per-(pair,slab) DMA blocks with a
# tapered-tail slab plan, SBUF-accumulated scan state, 3-deep back stage,
# stores on the Pool/SWDGE queue.
BEST = dict(plan=[4, 4, 4, 2, 2], slab_bufs=10, pipe=2, kn8=True, pp=True)


def bench_build(loop_k=None, **over):
    kw = dict(BEST)
    kw.update(over)
    return build_kernel16d(loop_k=loop_k, **kw)


def bench_in_maps(q, k, q_rot, k_rot, v):
    in_maps = _prepare_in_maps16b(q, k, q_rot, k_rot, v, vscale=1.0 / 16.0,
                                  plan=BEST["plan"], kn8=BEST["kn8"])
    for m in in_maps:
        msk = m.pop("mask")
        m["mask2"] = np.ascontiguousarray(np.concatenate([msk, msk], axis=1))
    return in_maps


def kernel16d(q, k, q_rot, k_rot, v, horizon=128, slab=8, **run_kwargs):
    q = np.asarray(q)
    k = np.asarray(k)
    q_rot = np.asarray(q_rot)
    k_rot = np.asarray(k_rot)
    v = np.asarray(v)
    b, h, n, d = q.shape
    e = v.shape[-1]
    assert (b * h, n, d, e) == (N_CORES * PAIRS_PER_CORE, N, D, E)

    if "nc16d" not in _cached:
        _cached["nc16d"] = bench_build()
    nc = _cached["nc16d"]

    in_maps = bench_in_maps(q, k, q_rot, k_rot, v)
    res = run_bass_kernel_spmd(nc, in_maps, core_ids=list(range(N_CORES)),
                               **run_kwargs)

    outf = np.empty((b * h, n, e), dtype=np.float32)
    for i in range(N_CORES):
        o = res.results[i]["out"]
        o = o.reshape(C, NCHUNK, PAIRS_PER_CORE, E1).astype(np.float32)
        for p in range(PAIRS_PER_CORE):
            num = o[:, :, p, 0:E].transpose(1, 0, 2).reshape(n, E)
            den = o[:, :, p, E].transpose(1, 0).reshape(n, 1)
            outf[PAIRS_PER_CORE * i + p] = num / den
    if run_kwargs:
        kernel16d.last_results = res
    return outf.reshape(b, h, n, e)


def kernel16c(q, k, q_rot, k_rot, v, horizon=128, slab=8, **run_kwargs):
    q = np.asarray(q)
    k = np.asarray(k)
    q_rot = np.asarray(q_rot)
    k_rot = np.asarray(k_rot)
    v = np.asarray(v)
    b, h, n, d = q.shape
    e = v.shape[-1]
    assert (b * h, n, d, e) == (N_CORES * PAIRS_PER_CORE, N, D, E)

    key = f"nc16c_{slab}"
    if key not in _cached:
        _cached[key] = build_kernel16c(slab=slab)
    nc = _cached[key]

    # v (and the fused ones column) are pre-scaled by 1/16 so the shipped
    # fp16 num|den never overflow (den reaches ~65536 unscaled); the host
    # division num/den cancels the scale exactly.
    in_maps = _prepare_in_maps16b(q, k, q_rot, k_rot, v, slab=slab,
                                  vscale=1.0 / 16.0)
    for m in in_maps:
        msk = m.pop("mask")
        m["mask2"] = np.ascontiguousarray(np.concatenate([msk, msk], axis=1))
    res = run_bass_kernel_spmd(nc, in_maps, core_ids=list(range(N_CORES)),
                               **run_kwargs)

    outf = np.empty((b * h, n, e), dtype=np.float32)
    for i in range(N_CORES):
        o = res.results[i]["out"]  # [C, NCHUNK * 2 * E1] fp16
        o = o.reshape(C, NCHUNK, PAIRS_PER_CORE, E1).astype(np.float32)
        for p in range(PAIRS_PER_CORE):
            num = o[:, :, p, 0:E].transpose(1, 0, 2).reshape(n, E)
            den = o[:, :, p, E].transpose(1, 0).reshape(n, 1)
            outf[PAIRS_PER_CORE * i + p] = num / den
    if run_kwargs:
        kernel16c.last_results = res
    return outf.reshape(b, h, n, e)


def _prepare_in_maps16b(q, k, q_rot, k_rot, v, slab=8, vscale=1.0, plan=None,
                        kn8=False):
    b, h, n, d = q.shape
    e = v.shape[-1]
    nbh = b * h
    if plan is None:
        plan = [slab] * (NCHUNK // slab)
    qf = q.reshape(nbh, n, d)
    kf = k.reshape(nbh, n, d)
    qrf = q_rot.reshape(nbh, n, d)
    krf = k_rot.reshape(nbh, n, d)
    vf = v.reshape(nbh, n, e)
    mask = np.triu(np.ones((C, C), dtype=np.float32))

    def chunk_major(x, nch):
        f = x.shape[-1]
        return x.reshape(nch, C, f).transpose(1, 0, 2).reshape(C, nch * f)

    in_maps = []
    for i in range(N_CORES):
        sel = [PAIRS_PER_CORE * i + p for p in range(PAIRS_PER_CORE)]
        qkblks, kvblks = [], []
        for s in sel:
            qT, kT, qrT, krT = (x[s].T.astype(np.float16)
                                for x in (qf, kf, qrf, krf))
            v1s = (vscale * np.concatenate(
                [vf[s], np.ones((n, 1), vf.dtype)], axis=1)).astype(np.float16)
            kns = kf[s].astype(np.float16)
            krns = krf[s].astype(np.float16)
            c0 = 0
            for ln in plan:
                cs = slice(c0 * C, (c0 + ln) * C)
                c0 += ln
                qkblks.append(np.concatenate(
                    [qT[:, cs], kT[:, cs], qrT[:, cs], krT[:, cs]], axis=1))
                if kn8:
                    import ml_dtypes
                    kn_b = chunk_major(kns[cs], ln).astype(
                        ml_dtypes.float8_e4m3fn).view(np.uint8)
                    krn_b = chunk_major(krns[cs], ln).astype(
                        ml_dtypes.float8_e4m3fn).view(np.uint8)
                    v1_b = chunk_major(v1s[cs], ln).view(np.uint8)
                    kvblks.append(np.concatenate([kn_b, krn_b, v1_b], axis=1))
                else:
                    kvblks.append(np.concatenate(
                        [chunk_major(kns[cs], ln), chunk_major(krns[cs], ln),
                         chunk_major(v1s[cs], ln)], axis=1))
        in_maps.append(dict(
            qk=np.ascontiguousarray(np.concatenate(qkblks, axis=1)),
            kv=np.ascontiguousarray(np.concatenate(kvblks, axis=1)),
            mask=mask))
    return in_maps


def kernel16b(q, k, q_rot, k_rot, v, horizon=128, slab=8, **run_kwargs):
    q = np.asarray(q)
    k = np.asarray(k)
    q_rot = np.asarray(q_rot)
    k_rot = np.asarray(k_rot)
    v = np.asarray(v)
    b, h, n, d = q.shape
    e = v.shape[-1]
    assert (b * h, n, d, e) == (N_CORES * PAIRS_PER_CORE, N, D, E)

    key = f"nc16b_{slab}"
    if key not in _cached:
        _cached[key] = build_kernel16b(slab=slab)
    nc = _cached[key]

    in_maps = _prepare_in_maps16b(q, k, q_rot, k_rot, v, slab=slab)
    res = run_bass_kernel_spmd(nc, in_maps, core_ids=list(range(N_CORES)),
                               **run_kwargs)

    outf = np.empty((b * h, n, e), dtype=np.float32)
    for i in range(N_CORES):
        o = res.results[i]["out"]  # [C, PAIRS*NCHUNK*E] fp16
        o = o.reshape(C, PAIRS_PER_CORE, NCHUNK, E).astype(np.float32)
        for p in range(PAIRS_PER_CORE):
            outf[PAIRS_PER_CORE * i + p] = o[:, p].transpose(1, 0, 2).reshape(n, e)
    if run_kwargs:
        kernel16b.last_results = res
    return outf.reshape(b, h, n, e)


def _prepare_in_maps16(q, k, q_rot, k_rot, v):
    b, h, n, d = q.shape
    e = v.shape[-1]
    nbh = b * h
    qf = q.reshape(nbh, n, d)
    kf = k.reshape(nbh, n, d)
    qrf = q_rot.reshape(nbh, n, d)
    krf = k_rot.reshape(nbh, n, d)
    vf = v.reshape(nbh, n, e)
    mask = np.triu(np.ones((C, C), dtype=np.float32))

    def chunk_major(x):
        # [n, f] -> [C, NCHUNK * f]: column-major-by-chunk on-chip layout
        f = x.shape[-1]
        return x.reshape(NCHUNK, C, f).transpose(1, 0, 2).reshape(C, NCHUNK * f)

    in_maps = []
    for i in range(N_CORES):
        sel = [PAIRS_PER_CORE * i + p for p in range(PAIRS_PER_CORE)]
        qT = np.concatenate([qf[s].T for s in sel], axis=1).astype(np.float16)
        kT = np.concatenate([kf[s].T for s in sel], axis=1).astype(np.float16)
        qrT = np.concatenate([qrf[s].T for s in sel], axis=1).astype(np.float16)
        krT = np.concatenate([krf[s].T for s in sel], axis=1).astype(np.float16)
        kn = np.concatenate([chunk_major(kf[s]) for s in sel], axis=1).astype(np.float16)
        krn = np.concatenate([chunk_major(krf[s]) for s in sel], axis=1).astype(np.float16)
        v1 = np.concatenate(
            [chunk_major(np.concatenate(
                [vf[s], np.ones((n, 1), vf.dtype)], axis=1)) for s in sel],
            axis=1).astype(np.float16)
        in_maps.append(dict(qT=np.ascontiguousarray(qT),
                            kT=np.ascontiguousarray(kT),
                            qrT=np.ascontiguousarray(qrT),
                            krT=np.ascontiguousarray(krT),
                            kn=np.ascontiguousarray(kn),
                            krn=np.ascontiguousarray(krn),
                            v1=np.ascontiguousarray(v1),
                            mask=mask))
    return in_maps


def kernel16(q, k, q_rot, k_rot, v, horizon=128, **run_kwargs):
    q = np.asarray(q)
    k = np.asarray(k)
    q_rot = np.asarray(q_rot)
    k_rot = np.asarray(k_rot)
    v = np.asarray(v)
    b, h, n, d = q.shape
    e = v.shape[-1]
    assert (b * h, n, d, e) == (N_CORES * PAIRS_PER_CORE, N, D, E)

    if "nc16" not in _cached:
        _cached["nc16"] = build_kernel16()
    nc = _cached["nc16"]

    in_maps = _prepare_in_maps16(q, k, q_rot, k_rot, v)
    res = run_bass_kernel_spmd(nc, in_maps, core_ids=list(range(N_CORES)),
                               **run_kwargs)

    outf = np.empty((b * h, n, e), dtype=np.float32)
    for i in range(N_CORES):
        o = res.results[i]["out"]  # [C, PAIRS*NCHUNK*E] fp16
        o = o.reshape(C, PAIRS_PER_CORE, NCHUNK, E).astype(np.float32)
        for p in range(PAIRS_PER_CORE):
            outf[PAIRS_PER_CORE * i + p] = o[:, p].transpose(1, 0, 2).reshape(n, e)
    if run_kwargs:
        kernel16.last_results = res
    return outf.reshape(b, h, n, e)


# Column strides inside shared PSUM banks (8-byte aligned regions)
PW = 72            # per-pair region width in the output bank (>= E1)
SW = 66            # per-(pair,branch) region width in the state bank (>= E1)


def build_kernel_m(repeat=1, loop_k=None):
    """Pair-merged variant: both (b,h) pairs handled per core share single
    PSUM banks for AT, numerator/denominator, and state, so the causal mask,
    the state evacuation, and the reciprocal each run as ONE wide
    vector/scalar op per chunk instead of one per pair. Cuts the DVE/ACT
    instruction count (and their fixed per-op drain cost) roughly in half."""
    nc = bacc.Bacc("TRN2", target_bir_lowering=False, debug=False,
                   num_devices=N_CORES)

    MT = F32  # typed-f32r rejected by walrus codegen (odd-N ISA check)

    def mm(out_ap, lhsT_ap, rhs_ap, **kw):
        if mm_f32r:
            lhsT_ap = lhsT_ap.bitcast(F32R)
            rhs_ap = rhs_ap.bitcast(F32R)
        return nc.tensor.matmul(out_ap, lhsT_ap, rhs_ap, **kw)

    qT = nc.dram_tensor("qT", [D, NROWS], MT, kind="ExternalInput").ap()
    kT = nc.dram_tensor("kT", [D, NROWS], MT, kind="ExternalInput").ap()
    qrT = nc.dram_tensor("qrT", [D, NROWS], MT, kind="ExternalInput").ap()
    krT = nc.dram_tensor("krT", [D, NROWS], MT, kind="ExternalInput").ap()
    kn = nc.dram_tensor("kn", [NROWS, D], MT, kind="ExternalInput").ap()
    krn = nc.dram_tensor("krn", [NROWS, D], MT, kind="ExternalInput").ap()
    v1 = nc.dram_tensor("v1", [NROWS, E1], MT, kind="ExternalInput").ap()
    mask2 = nc.dram_tensor("mask2", [C, 2 * C], F32, kind="ExternalInput").ap()
    out = nc.dram_tensor("out", [NROWS, E], F32, kind="ExternalOutput").ap()

    NP = PAIRS_PER_CORE  # 2

    with tile.TileContext(nc) as tc:
        with (
            tc.tile_pool(name="const", bufs=1) as constp,
            tc.tile_pool(name="slabs", bufs=6) as slabs,
            tc.tile_pool(name="atm", bufs=3) as atmp,
            tc.tile_pool(name="ssb", bufs=4) as ssbp,
            tc.tile_pool(name="dinv", bufs=8) as dinvp,
            tc.tile_pool(name="pat", bufs=3, space="PSUM") as patp,
            tc.tile_pool(name="pout", bufs=3, space="PSUM") as poutp,
            tc.tile_pool(name="pst", bufs=1, space="PSUM") as pstp,
        ):
            mask_t = constp.tile([C, 2 * C], F32, tag="mask")
            nc.sync.dma_start(mask_t[:], mask2[:])

            fixed = None
            if probe_nodma:
                # one 8-chunk slab-set per pair, loaded once; the loop reuses
                # it for both halves -> compute rhythm with no load DMAs
                fixed = []
                fl = 8
                for p in range(NPC):
                    fqk = constp.tile([D, 4 * fl * C], F16, tag=f"fqk{p}",
                                      name=f"fqk{p}")
                    nc.sync.dma_start(fqk[:], qk[:, 0:4 * fl * C])
                    fkv = constp.tile([C, fl * (2 * D + E1)], F16,
                                      tag=f"fkv{p}", name=f"fkv{p}")
                    nc.sync.dma_start(fkv[:], kv[:, 0:fl * (2 * D + E1)])
                    fixed.append({"qk": fqk, "kv": fkv, "ln": fl})

            for rep in range(repeat):
              with (tc.For_i(0, loop_k, 1, hint_engines=(
                        mybir.EngineType.PE, mybir.EngineType.DVE,
                        mybir.EngineType.Activation, mybir.EngineType.SP))
                    if (loop_k is not None and loop_k > 1)
                    else _nullctx()):
                  # one state bank: region (p, br) at cols (2p+br)*SW
                  pSt = pstp.tile([D, 2 * NP * SW], F32, tag="pS",
                                  name=f"pSm_{rep}")

                  slab_t = [None] * NP
                  S_sbuf = [None]     # boxed: current [D, 4*SW] sbuf state

                  pending = None
                  for cc in range(NCHUNK + 1):
                    back = pending
                    pending = None
                    if cc < NCHUNK:
                      c = cc
                      sl = {}
                      for p in range(NP):
                          if c % SLAB == 0:
                              base = p * N + c * C
                              cols = slice(base, base + SLAB * C)
                              st = {}
                              st["qT"] = slabs.tile([D, slen * C], F32, tag="qT", name=f"qTs_{rep}_{p}_{c}")
                              nc.sync.dma_start(st["qT"][:], qT[:, cols])
                              st["kT"] = slabs.tile([D, slen * C], F32, tag="kT", name=f"kTs_{rep}_{p}_{c}")
                              nc.sync.dma_start(st["kT"][:], kT[:, cols])
                              st["qrT"] = slabs.tile([D, slen * C], F32, tag="qrT", name=f"qrTs_{rep}_{p}_{c}")
                              nc.sync.dma_start(st["qrT"][:], qrT[:, cols])
                              st["krT"] = slabs.tile([D, slen * C], F32, tag="krT", name=f"krTs_{rep}_{p}_{c}")
                              nc.sync.dma_start(st["krT"][:], krT[:, cols])
                              st["kn"] = slabs.tile([C, slen, D], F32, tag="kn", name=f"kns_{rep}_{p}_{c}")
                              nc.sync.dma_start(
                                  st["kn"][:],
                                  kn[cols, :].rearrange("(n p) d -> p n d", p=C))
                              st["krn"] = slabs.tile([C, slen, D], F32, tag="krn", name=f"krns_{rep}_{p}_{c}")
                              nc.sync.dma_start(
                                  st["krn"][:],
                                  krn[cols, :].rearrange("(n p) d -> p n d", p=C))
                              st["v1"] = slabs.tile([C, slen, E1], F32, tag="v1", name=f"v1s_{rep}_{p}_{c}")
                              nc.sync.dma_start(
                                  st["v1"][:],
                                  v1[cols, :].rearrange("(n p) e -> p n e", p=C))
                              st["outs"] = slabs.tile([C, SLAB, E], F32, tag="outs", name=f"outs_{rep}_{p}_{c}")
                              slab_t[p] = st

                          st = slab_t[p]
                          j = c - c0
                          sl[p] = dict(
                              st=st, j=j,
                              qcT=st["qT"][:, j * C:(j + 1) * C],
                              kcT=st["kT"][:, j * C:(j + 1) * C],
                              qrcT=st["qrT"][:, j * C:(j + 1) * C],
                              krcT=st["krT"][:, j * C:(j + 1) * C],
                              knc=st["kn"][:, j, :],
                              krnc=st["krn"][:, j, :],
                              vc=st["v1"][:, j, :],
                          )

                      prev_S = S_sbuf[0]

                      # state updates, all four into one bank
                      for p in range(NP):
                          z = sl[p]
                          nc.tensor.matmul(
                              pSt[:, (2 * p) * SW:(2 * p) * SW + E1],
                              z["knc"], z["vc"],
                              start=(c == 0 and p == 0), stop=False,
                              skip_group_check=True)
                          nc.tensor.matmul(
                              pSt[:, (2 * p + 1) * SW:(2 * p + 1) * SW + E1],
                              z["krnc"], z["vc"],
                              start=False,
                              stop=(c == NCHUNK - 1 and p == NP - 1),
                              skip_group_check=True)
                      if c < NCHUNK - 1:
                          s01 = ssbp.tile([D, 2 * NP * SW], F32, tag="ssb")
                          nc.scalar.copy(s01[:], pSt[:])
                          S_sbuf[0] = s01

                      # AT for both pairs into one bank, one mask op
                      patb = patp.tile([C, 2 * C], F32, tag="pat")
                      for p in range(NP):
                          z = sl[p]
                          reg = patb[:, p * C:(p + 1) * C]
                          nc.tensor.matmul(reg, z["kcT"], z["qcT"],
                                           start=True, stop=False,
                                           skip_group_check=True)
                          nc.tensor.matmul(reg, z["krcT"], z["qrcT"],
                                           start=False, stop=True,
                                           skip_group_check=True)
                      atm = atmp.tile([C, 2 * C], F32, tag="atm")
                      nc.vector.tensor_mul(atm[:], patb[:], mask_t[:])

                      pending = dict(atm=atm, sl=sl, c=c, prev_S=prev_S)

                    if back is not None:
                        cb = back["c"]
                        pob = poutp.tile([C, NP * PW], F32, tag="po")
                        for p in range(NP):
                            z = back["sl"][p]
                            reg = pob[:, p * PW:p * PW + E1]
                            only = (cb == 0)
                            nc.tensor.matmul(
                                reg, back["atm"][:, p * C:(p + 1) * C],
                                z["vc"], start=True, stop=only,
                                skip_group_check=True)
                            if cb > 0:
                                pv = back["prev_S"]
                                nc.tensor.matmul(
                                    reg, z["qcT"],
                                    pv[:, (2 * p) * SW:(2 * p) * SW + E1],
                                    start=False, stop=False,
                                    skip_group_check=True)
                                nc.tensor.matmul(
                                    reg, z["qrcT"],
                                    pv[:, (2 * p + 1) * SW:(2 * p + 1) * SW + E1],
                                    start=False, stop=True,
                                    skip_group_check=True)

                        # one reciprocal for both pairs' denominators
                        dinv = dinvp.tile([C, NP], F32, tag="dinv")
                        nc.vector.reciprocal(
                            dinv[:], pob[:, E:NP * PW:PW])
                        for p in range(NP):
                            z = back["sl"][p]
                            nc.scalar.mul(z["st"]["outs"][:, z["j"], :],
                                          pob[:, p * PW:p * PW + E],
                                          dinv[:, p:p + 1])
                            if z["j"] == SLAB - 1:
                                base = p * N + (cb - SLAB + 1) * C
                                rows = slice(base, base + SLAB * C)
                                nc.sync.dma_start(
                                    out[rows, :].rearrange(
                                        "(n p) e -> p n e", p=C),
                                    z["st"]["outs"][:])

    nc.compile()
    return nc



def _prepare_in_maps(q, k, q_rot, k_rot, v, transpose_k=False, merged=False):
    b, h, n, d = q.shape
    e = v.shape[-1]
    nbh = b * h
    qf = np.ascontiguousarray(q.reshape(nbh, n, d).astype(np.float32))
    kf = np.ascontiguousarray(k.reshape(nbh, n, d).astype(np.float32))
    qrf = np.ascontiguousarray(q_rot.reshape(nbh, n, d).astype(np.float32))
    krf = np.ascontiguousarray(k_rot.reshape(nbh, n, d).astype(np.float32))
    vf = np.ascontiguousarray(v.reshape(nbh, n, e).astype(np.float32))
    mask = np.triu(np.ones((C, C), dtype=np.float32))

    in_maps = []
    for i in range(N_CORES):
        sel = [PAIRS_PER_CORE * i + p for p in range(PAIRS_PER_CORE)]
        qT = np.ascontiguousarray(
            np.concatenate([qf[s].T for s in sel], axis=1))
        kT = np.ascontiguousarray(
            np.concatenate([kf[s].T for s in sel], axis=1))
        qrT = np.ascontiguousarray(
            np.concatenate([qrf[s].T for s in sel], axis=1))
        krT = np.ascontiguousarray(
            np.concatenate([krf[s].T for s in sel], axis=1))
        knat = np.ascontiguousarray(np.concatenate([kf[s] for s in sel], axis=0))
        krnat = np.ascontiguousarray(np.concatenate([krf[s] for s in sel], axis=0))
        vcat = np.concatenate([vf[s] for s in sel], axis=0)
        v1 = np.ascontiguousarray(
            np.concatenate([vcat, np.ones((vcat.shape[0], 1), np.float32)],
                           axis=1))
        m = dict(qT=qT, kT=kT, qrT=qrT, krT=krT, v1=v1)
        if merged:
            m["mask2"] = np.ascontiguousarray(np.concatenate([mask, mask], axis=1))
        else:
            m["mask"] = mask
        if transpose_k:
            m["ident"] = np.eye(C, dtype=np.float32)
        else:
            m["kn"] = knat
            m["krn"] = krnat
        in_maps.append(m)
    return in_maps


def kernel_f32(q, k, q_rot, k_rot, v, horizon=128, **run_kwargs):
    q = np.asarray(q)
    k = np.asarray(k)
    q_rot = np.asarray(q_rot)
    k_rot = np.asarray(k_rot)
    v = np.asarray(v)
    b, h, n, d = q.shape
    e = v.shape[-1]
    assert (b * h, n, d, e) == (N_CORES * PAIRS_PER_CORE, N, D, E), \
        "kernel is hardcoded for b*h=16, n=2048, d=128, e=64"

    if "nc" not in _cached:
        _cached["nc"] = build_kernel()
    nc = _cached["nc"]

    in_maps = _prepare_in_maps(q, k, q_rot, k_rot, v)
    res = run_bass_kernel_spmd(nc, in_maps, core_ids=list(range(N_CORES)),
                               **run_kwargs)

    outf = np.empty((b * h, n, e), dtype=np.float32)
    for i in range(N_CORES):
        o = res.results[i]["out"].reshape(PAIRS_PER_CORE, n, e)
        for p in range(PAIRS_PER_CORE):
            outf[PAIRS_PER_CORE * i + p] = o[p]
    if run_kwargs:
        kernel_f32.last_results = res
    return outf.reshape(b, h, n, e)


def kernel(q, k, q_rot, k_rot, v, horizon=128, **run_kwargs):
    return kernel16d(q, k, q_rot, k_rot, v, horizon, **run_kwargs)


if __name__ == "__main__":
    rng = np.random.default_rng(0)
    q = rng.random((2, 8, N, D), dtype=np.float32)
    k = rng.random((2, 8, N, D), dtype=np.float32)
    qr = rng.standard_normal((2, 8, N, D), dtype=np.float32)
    kr = rng.standard_normal((2, 8, N, D), dtype=np.float32)
    v = rng.random((2, 8, N, E), dtype=np.float32)
    o = kernel(q, k, qr, kr, v, 128)
    print("ok", o.shape, o.dtype, np.abs(o).mean())

